# revision 1
# baseline (speedup 1.0000x reference)
"""Trainium2 Bass kernel for nn_NeuronAttention (moe_routing).

Sharding: data-parallel over batch B=8 across 8 NeuronCores (one batch row
per core); weights replicated; no collectives.

Per-core computation uses two layouts: "T-domain" [feature, token] for PE
GEMMs (contraction on partitions) and "N-domain" [token, small-free] for
routing math (softmax, top-k, Householder-chain recursion) on DVE/ACT.
The Householder chains are evaluated in 64-dim dot-space: with
d0 = xc@P.T, G = P@P.T, a = 1/(||P_k||^2+EPS), the 4 selected reflections
reduce to the scalar recursion beta_i = 2a_i(d0_i - sum_{j<i} beta_j G_ij)
and a rank-64 correction xc - (sum_i beta_i e_{idx_i})@P.

Precision: true fp32 (PE 4-pass) on every GEMM feeding router scores
(top-k gaps in this data go down to ~2e-6; FP22/bf16 would flip
selections), float32r (1-pass) only for the final output GEMM.
"""

import numpy as np

B, S, D, R = 8, 1024, 1024, 512
NPROC, TOPK = 64, 4
H, DH = 8, 64
EPS = 1e-8
NCORES = 8
TCN = 8   # token chunks of 128
KCN = 8   # D chunks of 128
RCN = 4   # rank chunks of 128
DEBUG = False
F32R_OUT = False
PHASES = 4
SUBB = 3
ROUTES = 3

_BUILT = {}


def _finish(nc):
    # sim-only partial builds skip the walrus wait-split
    return nc


def _apply_tile_drain_patch():
    """walrus here rejects >1 sync-wait on CTRL-class instructions; split
    Tile's kernel-tail drain waits into a chain of single-wait nops."""
    import concourse.mybir as mybir
    from concourse.tile import TileContext
    from concourse.vector_clock import ScopedClock

    if getattr(TileContext, "_drain_patched", False):
        return

    def _patched(self, tick_clock, wait_clock):
        probe = self.nc.sync.nop()
        wait_clock.add_sem_waits(
            probe.ins, ScopedClock({None: tick_clock.global_clock}))
        si = probe.ins.sync_info
        waits = list(si.on_wait) if si is not None else []
        updates = list(si.on_update) if si is not None else []
        if len(waits) > 1:
            probe.ins.sync_info = mybir.SyncInfo(
                on_update=updates, on_wait=waits[:1])
            for ofs in range(1, len(waits)):
                extra = self.nc.sync.nop()
                extra.ins.sync_info = mybir.SyncInfo(
                    on_update=[], on_wait=waits[ofs:ofs + 1])
        self.nc.sync.drain()
        self.nc.all_engine_barrier()
        assert self.sems is not None
        popped = self.nc._tile_sem_poison_stack.pop()
        assert popped is self._sem_poison
        self.nc.clear_and_free_semaphores(list(self.sems.allocated().values()))
        self.nc.all_engine_barrier()

    TileContext._drain_and_barrier = _patched
    TileContext._drain_patched = True


def _split_sync_waits(nc):
    """walrus here accepts at most 1 sync-wait per instruction; hoist
    extra waits onto same-engine NoOps inserted just before."""
    import concourse.mybir as mybir

    ctr = [0]
    for f in nc.m.functions:
        for bb in f.blocks:
            insts = bb.instructions
            out = []
            for inst in insts:
                si = inst.sync_info
                if si is not None and len(si.on_wait) > 1:
                    waits = list(si.on_wait)
                    for w in waits[:-1]:
                        ctr[0] += 1
                        nop = mybir.InstNoOp(
                            name=f"I-sw{ctr[0]}", ins=[], outs=[])
                        nop.engine = inst.engine
                        nop.sync_info = mybir.SyncInfo(
                            on_update=[], on_wait=[w])
                        out.append(nop)
                    inst.sync_info = mybir.SyncInfo(
                        on_update=list(si.on_update), on_wait=[waits[-1]])
                out.append(inst)
            bb.instructions = out


def _pack_kc(a, nchunk, chunk):
    # [nchunk*chunk, N] -> [chunk, nchunk*N], chunk-major partitions
    n = a.shape[1]
    return np.ascontiguousarray(
        a.reshape(nchunk, chunk, n).transpose(1, 0, 2).reshape(chunk, nchunk * n)
    ).astype(np.float32)


def prep_weights(inputs):
    f = {k: np.asarray(v, np.float64) for k, v in inputs.items()}
    P = f["process_hh"]
    G = P @ P.T
    alpha2 = 2.0 / ((P * P).sum(1) + EPS)
    ihh, ohh = f["input_hh"], f["output_hh"]
    base_in, base_out = f["base_input"], f["base_output"]
    Bo = ohh @ base_out.T

    w = {}
    w["BI"] = _pack_kc(base_in, KCN, 128)
    W4 = np.concatenate([f["q_in_router"].T, f["k_in_router"].T,
                         f["v_in_router"].T, ihh.T], axis=1)
    w["W4"] = _pack_kc(W4, KCN, 128)
    for nm, wp in (("WDRQ", "q_proc_router"), ("WDRK", "k_proc_router"),
                   ("WDRV", "v_proc_router"), ("WDRO", "o_proc_router")):
        w[nm] = _pack_kc(np.concatenate([P.T, f[wp].T], axis=1), RCN, 128)
    w["WDRO2"] = _pack_kc(
        np.concatenate([f["o_out_router"].T, Bo.T,
                        np.zeros((512, 128))], axis=1), RCN, 128)
    w["NEGBH"] = (-(ihh @ base_in)).astype(np.float32)
    w["NEGP"] = (-P).astype(np.float32)
    w["GIN"] = (ihh @ ihh.T).astype(np.float32)
    w["GOUT"] = (ohh @ ohh.T).astype(np.float32)
    BD = np.zeros((256, 196))
    for i in range(4):
        if i < 3:
            BD[64 * i:64 * i + 64, 64 * i:64 * i + 64] = -G
        BD[64 * i:64 * i + 64, 192 + i] = alpha2
    w["BD"] = _pack_kc(BD, 2, 128)
    w["BOUT"] = _pack_kc(base_out, RCN, 128)
    w["NEGOHH"] = (-ohh).astype(np.float32)
    w["NEGPBO"] = (-(P @ base_out)).astype(np.float32)
    w["NEGPOW"] = (-(P @ np.concatenate(
        [f["o_out_router"].T, Bo.T], axis=1))).astype(np.float32)
    w["NPBOHH"] = np.concatenate(
        [-(P @ base_out), -ohh], axis=0).astype(np.float32)
    return w


def build():
    import concourse.bass as bass
    import concourse.mybir as mybir
    from concourse.tile import TileContext
    from concourse.masks import make_identity

    _apply_tile_drain_patch()
    dt = mybir.dt
    op = mybir.AluOpType
    act = mybir.ActivationFunctionType

    nc = bass.Bass()
    XTd = nc.dram_tensor("XT", (128, KCN * 1024), dt.float32, kind="ExternalInput")
    wd = {}
    for nm, shape in (
        ("BI", (128, KCN * 512)), ("W4", (128, KCN * 256)),
        ("WDRQ", (128, RCN * 128)), ("WDRK", (128, RCN * 128)),
        ("WDRV", (128, RCN * 128)), ("WDRO", (128, RCN * 128)),
        ("WDRO2", (128, RCN * 256)),
        ("NEGBH", (64, 512)), ("NEGP", (64, 512)),
        ("GIN", (64, 64)), ("GOUT", (64, 64)),
        ("BD", (128, 2 * 196)), ("BOUT", (128, RCN * 1024)),
        ("NEGOHH", (64, 1024)), ("NEGPBO", (64, 1024)), ("NEGPOW", (64, 128)), ("NPBOHH", (128, 1024)),
    ):
        wd[nm] = nc.dram_tensor(nm, shape, dt.float32, kind="ExternalInput")
    OUTd = nc.dram_tensor("OUT", (1024, 1024), dt.float32, kind="ExternalOutput")
    dbg = {}
    if DEBUG:
        for nm, shape in (
            ("DSF", (8, 128, 256)), ("DXB", (4, 128, 1024)),
            ("DXCQ", (4, 128, 1024)), ("DRQ", (128, 8, 64)),
            ("DD0Q", (128, 8, 64)), ("DM8Q", (128, 8, 8)),
            ("DCQ", (128, 8, 64)), ("DQT", (4, 128, 1024)),
            ("DVP", (8, 128, 520)), ("DAO", (8, 128, 512)),
            ("DX4", (4, 128, 1024)), ("DBETAQ", (128, 8, 4)),
        ):
            dbg[nm] = nc.dram_tensor(nm, shape, dt.float32, kind="ExternalOutput")

    f32r = dt.float32r

    with TileContext(nc) as tc:
        with (
            tc.tile_pool(name="w", bufs=1) as pw,
            tc.tile_pool(name="live", bufs=1) as pl,
        ):
            W = {}
            for nm, dram in wd.items():
                if nm in ("BOUT", "NEGOHH", "WDRO2", "NEGPBO", "NEGPOW", "NPBOHH"):
                    continue
                t = pw.tile(list(dram.shape), dt.float32, tag=nm)
                nc.sync.dma_start(out=t[:], in_=dram[:])
                W[nm] = t
            ident = pw.tile([128, 128], dt.float32, tag="ident", name="ident")
            make_identity(nc, ident[:])

            BI = W["BI"][:].rearrange("p (k n) -> p k n", k=KCN)
            W4 = W["W4"][:].rearrange("p (k n) -> p k n", k=KCN)
            BD = W["BD"][:].rearrange("p (k n) -> p k n", k=2)
            WDR = {r: W["WDR" + r][:].rearrange("p (k n) -> p k n", k=RCN)
                   for r in ("Q", "K", "V", "O")}
            XTv = XTd[:].rearrange("p (k n) -> p k n", k=KCN)

            # persistent activations
            AOT = [pl.tile([128, 1024], dt.float32, tag=f"aot{rc}", name=f"aot{rc}")
                   for rc in range(RCN)]

            # ---------- shared helpers ----------

            def dot64(acc, a, b, pool, nm):
                scr = pool.tile([128, 64], dt.float32, tag="scr_sh",
                                name=f"scr_{nm}", bufs=4)
                nc.vector.tensor_mul(scr[:], a, b)
                nc.vector.tensor_reduce(acc, scr[:], mybir.AxisListType.X,
                                        op.add)
            def softmax_front(pool, ppt, ppv, name, s_all, f_all, gram,
                              cht_dtype=None, cht_ap=None):
                """Batched over all 8 token chunks. s_all/f_all are
                [128, 8, 64] APs. Returns CHT [64, 1024] (chat^T)."""
                E = pool.tile([128, TCN * 64], dt.float32, tag="E_sh",
                              name=f"E_{name}", bufs=2)
                Ev = E[:].rearrange("p (t n) -> p t n", t=TCN)
                ET = pool.tile([64, 1024], dt.float32, tag="ET_sh",
                               name=f"ET_{name}", bufs=1)
                CH = pool.tile([128, TCN * 64], dt.float32, tag="CH_sh",
                               name=f"CH_{name}", bufs=2)
                CHv = CH[:].rearrange("p (t n) -> p t n", t=TCN)
                CHT = cht_ap if cht_ap is not None else pool.tile(
                    [64, 1024], cht_dtype or dt.float32,
                    tag="CHT_sh", name=f"CHT_{name}", bufs=1)
                SC = pool.tile([128, 8 * 8], dt.float32, tag=f"sc1_{name}", name=f"sc1_{name}")
                SCv = SC[:].rearrange("p (t n) -> p t n", t=8)
                scr = pool.tile([128, 512], dt.float32, tag="scr_sh",
                                name=f"scr_{name}", bufs=4)

                nc.scalar.activation(Ev[:, :, :], s_all, act.Exp)
                Z8 = SCv[:, :, 0:1]
                nc.vector.tensor_reduce(Z8, Ev[:, :, :],
                                        mybir.AxisListType.X, op.add)
                # u = e @ Gin per chunk, packed into one PSUM bank
                pu = ppv.tile([128, 512], dt.float32, tag="ps_u", name="ps_u")
                for t in range(TCN):
                    pt = ppt.tile([128, 128], dt.float32, tag="ps_t", name="ps_t")
                    nc.tensor.transpose(pt[0:64, :], Ev[:, t, :], ident[:])
                    nc.scalar.copy(ET[:, 128 * t:128 * (t + 1)], pt[0:64, :])
                    nc.tensor.matmul(pu[:, 64 * t:64 * (t + 1)],
                                     ET[:, 128 * t:128 * (t + 1)], gram,
                                     start=True, stop=True)
                puv = pu[:].rearrange("p (t n) -> p t n", t=TCN)
                pacc, qacc = SCv[:, :, 1:2], SCv[:, :, 2:3]
                nc.vector.tensor_mul(scr[:], Ev[:, :, :], f_all)
                nc.vector.tensor_reduce(
                    pacc, scr[:].rearrange("p (t n) -> p t n", t=TCN),
                    mybir.AxisListType.X, op.add)
                nc.vector.tensor_mul(scr[:], Ev[:, :, :], puv)
                nc.vector.tensor_reduce(
                    qacc, scr[:].rearrange("p (t n) -> p t n", t=TCN),
                    mybir.AxisListType.X, op.add)
                z2, den = SCv[:, :, 3:4], SCv[:, :, 4:5]
                rec, gam = SCv[:, :, 5:6], SCv[:, :, 6:7]
                nc.vector.tensor_mul(z2, Z8, Z8)
                nc.vector.scalar_tensor_tensor(out=den, in0=z2, scalar=EPS,
                                               in1=qacc, op0=op.mult, op1=op.add)
                nc.vector.reciprocal(rec, den)
                nc.vector.scalar_tensor_tensor(out=gam, in0=pacc, scalar=2.0,
                                               in1=rec, op0=op.mult, op1=op.mult)
                nc.vector.tensor_mul(CHv[:, :, :], Ev[:, :, :],
                                     gam.to_broadcast((128, TCN, 64)))
                for t in range(TCN):
                    pt2 = ppt.tile([128, 128], dt.float32, tag="ps_t", name="ps_t")
                    nc.tensor.transpose(pt2[0:64, :], CHv[:, t, :], ident[:])
                    nc.scalar.copy(CHT[:, 128 * t:128 * (t + 1)], pt2[0:64, :])
                return CHT

            def hh_chain_a(pool, pps, ppt, name, wdr, src_tiles,
                           dbg_pfx=None, ct_ap=None):
                """Stage a: D0/R GEMM, top-4 one-hots, d0 selects, -G row
                gather. Returns tile dict for hh_chain_b."""
                D0 = pool.tile([128, TCN * 64], dt.float32, tag="D0_sh",
                               name=f"D0_{name}", bufs=2)
                D0v = D0[:].rearrange("p (t n) -> p t n", t=TCN)
                RS = pool.tile([128, TCN * 64], dt.float32, tag="RS_sh",
                               name=f"RS_{name}", bufs=2)
                RSv = RS[:].rearrange("p (t n) -> p t n", t=TCN)
                M8 = pool.tile([128, TCN * 8], dt.float32, tag="M8_sh",
                               name=f"M8_{name}", bufs=1)
                M8v = M8[:].rearrange("p (t n) -> p t n", t=TCN)
                OH = pool.tile([128, TCN * 256], dt.float32, tag="OH_sh",
                               name=f"OH_{name}", bufs=2)
                OHv = OH[:].rearrange("p (t n) -> p t n", t=TCN)
                OHT = pool.tile([128, 256], dt.float32, tag="OHT_sh",
                                name=f"OHT_{name}", bufs=2)
                BT = pool.tile([128, TCN * 196], dt.float32, tag="BT_sh",
                               name=f"BT_{name}", bufs=2)
                BTv = BT[:].rearrange("p (t n) -> p t n", t=TCN)
                DSA = pool.tile([128, TCN * 24], dt.float32, tag="DS_sh2",
                                name=f"DS_{name}", bufs=2)
                DSAv = DSA[:].rearrange("p (t n) -> p t n", t=TCN)
                BE = pool.tile([128, TCN * 4], dt.float32, tag=f"BE_{name}", name=f"BE_{name}")
                BEv = BE[:].rearrange("p (t n) -> p t n", t=TCN)
                CC = pool.tile([128, TCN * 64], dt.float32, tag="CC_sh",
                               name=f"CC_{name}", bufs=2)
                CCv = CC[:].rearrange("p (t n) -> p t n", t=TCN)
                CT = ct_ap if ct_ap is not None else pool.tile(
                    [64, 1024], dt.float32, tag="CT_sh",
                    name=f"CT_{name}", bufs=2)
                scr = pool.tile([128, 512], dt.float32, tag="scr_sh",
                                name=f"scr_{name}2", bufs=4)
                scrv = scr[:].rearrange("p (t n) -> p t n", t=TCN)

                # D0/R: pack 4 chunks per PSUM bank
                psd = [pps.tile([128, 512], dt.float32, tag="ps_sf",
                                name="ps_sf", bufs=2)
                       for _ in range(2)]
                for t in range(TCN):
                    for rc in range(RCN):
                        nc.tensor.matmul(
                            psd[t // 4][:, 128 * (t % 4):128 * (t % 4 + 1)],
                            src_tiles[rc][:, 128 * t:128 * (t + 1)],
                            wdr[:, rc, :],
                            start=(rc == 0), stop=(rc == RCN - 1))
                for half in range(2):
                    pv = psd[half][:].rearrange("p (t n) -> p t n", t=4)
                    nc.scalar.copy(D0v[:, 4 * half:4 * half + 4, :],
                                   pv[:, :, 0:64])
                    nc.scalar.copy(RSv[:, 4 * half:4 * half + 4, :],
                                   pv[:, :, 64:128])
                for t in range(TCN):
                    nc.vector.max(M8v[:, t, :], RSv[:, t, :])
                # one-hots + d0 selects (batched over chunks)
                for i in range(4):
                    nc.vector.tensor_tensor(
                        OHv[:, :, 64 * i:64 * (i + 1)], RSv[:, :, :],
                        M8v[:, :, i:i + 1].to_broadcast((128, TCN, 64)),
                        op.is_equal)
                    nc.vector.tensor_mul(scr[:], OHv[:, :, 64 * i:64 * (i + 1)],
                                         D0v[:, :, :])
                    nc.vector.tensor_reduce(DSAv[:, :, i:i + 1], scrv,
                                            mybir.AxisListType.X, op.add)
                # gather -G rows + 2alpha via transposed one-hot GEMM
                for t in range(TCN):
                    psb = pps.tile([128, 196], dt.float32, tag="ps_mm",
                                   name="ps_b", bufs=3)
                    for half in range(2):
                        pt = ppt.tile([128, 128], dt.float32, tag="ps_t", name="ps_t")
                        nc.tensor.transpose(
                            pt[:], OHv[:, t, 128 * half:128 * (half + 1)],
                            ident[:])
                        nc.scalar.copy(OHT[:, 128 * half:128 * (half + 1)], pt[:])
                        nc.tensor.matmul(
                            psb[:], OHT[:, 128 * half:128 * (half + 1)],
                            BD[:, half, :], start=(half == 0), stop=(half == 1))
                    nc.scalar.copy(BTv[:, t, :], psb[:])
                if DEBUG and dbg_pfx:
                    nc.sync.dma_start(out=dbg[f"DR{dbg_pfx}"][:], in_=RSv[:, :, :])
                    nc.sync.dma_start(out=dbg[f"DD0{dbg_pfx}"][:], in_=D0v[:, :, :])
                    nc.sync.dma_start(out=dbg[f"DM8{dbg_pfx}"][:], in_=M8v[:, :, :])
                return dict(OHv=OHv, BTv=BTv, DSAv=DSAv, BEv=BEv,
                            CCv=CCv, CC=CC, CT=CT, scr=scr, scrv=scrv)

            def hh_chain_b(st):
                OHv, BTv, DSAv = st["OHv"], st["BTv"], st["DSAv"]
                BEv, CCv, CC = st["BEv"], st["CCv"], st["CC"]
                CT, scr, scrv = st["CT"], st["scr"], st["scrv"]
                # pair values -G[idx_i, idx_j]
                pair = {}
                pidx = 4
                for i in range(1, 4):
                    for j in range(i):
                        nc.vector.tensor_mul(
                            scr[:], OHv[:, :, 64 * i:64 * (i + 1)],
                            BTv[:, :, 64 * j:64 * (j + 1)])
                        nc.vector.tensor_reduce(
                            DSAv[:, :, pidx:pidx + 1], scrv,
                            mybir.AxisListType.X, op.add)
                        pair[(i, j)] = DSAv[:, :, pidx:pidx + 1]
                        pidx += 1
                # recursion (batched [128, 8] ops)
                be = [BEv[:, :, i:i + 1] for i in range(4)]
                a2 = [BTv[:, :, 192 + i:193 + i] for i in range(4)]
                nc.vector.tensor_mul(be[0], DSAv[:, :, 0:1], a2[0])
                tmp = 10
                for i in range(1, 4):
                    cur = DSAv[:, :, i:i + 1]
                    for j in range(i):
                        t1 = DSAv[:, :, tmp:tmp + 1]; tmp += 1
                        nc.vector.tensor_mul(t1, pair[(i, j)], be[j])
                        t2 = DSAv[:, :, tmp:tmp + 1]; tmp += 1
                        nc.vector.tensor_add(t2, t1, cur)
                        cur = t2
                    nc.vector.tensor_mul(be[i], cur, a2[i])
                # c = sum beta_i * onehot_i
                nc.vector.tensor_mul(CCv[:, :, :], OHv[:, :, 0:64],
                                     be[0].to_broadcast((128, TCN, 64)))
                for i in range(1, 4):
                    nc.vector.tensor_mul(
                        scr[:], OHv[:, :, 64 * i:64 * (i + 1)],
                        be[i].to_broadcast((128, TCN, 64)))
                    nc.vector.tensor_add(CC[:], CC[:], scr[:])
                for t in range(TCN):
                    ptc = ppt.tile([128, 128], dt.float32, tag="ps_t", name="ps_t")
                    nc.tensor.transpose(ptc[0:64, :], CCv[:, t, :], ident[:])
                    nc.scalar.copy(CT[:, 128 * t:128 * (t + 1)], ptc[0:64, :])
                return CT

            def hh_chain(pool, pps, ppt, name, wdr, src_tiles,
                         dbg_pfx=None, ct_ap=None):
                st = hh_chain_a(pool, pps, ppt, name, wdr, src_tiles,
                                dbg_pfx, ct_ap=ct_ap)
                return hh_chain_b(st)

            def neg_corr(pps, lhs64, ct, dst_tiles, src_tiles):
                """dst = src + lhs64.T @ ct  (K=64 correction GEMM + add)."""
                for rc in range(RCN):
                    for th in range(2):
                        ps = pps.tile([128, 512], dt.float32, tag="ps_mm",
                                      name="ps_mm", bufs=3)
                        nc.tensor.matmul(
                            ps[:], lhs64[0:64, 128 * rc:128 * (rc + 1)],
                            ct[0:64, 512 * th:512 * (th + 1)],
                            start=True, stop=True)
                        nc.vector.tensor_add(
                            dst_tiles[rc][:, 512 * th:512 * (th + 1)],
                            src_tiles[rc][:, 512 * th:512 * (th + 1)], ps[:])

            # ================= phase A: SF + xbT =================
            _pbc_cm = tc.tile_pool(name="bc", bufs=1)
            pbc = _pbc_cm.__enter__()
            XC = {r: [pbc.tile([128, 1024], dt.float32, tag=f"xc{r}{rc}", name=f"xc{r}{rc}")
                      for rc in range(RCN)] for r in "qk"}
            VP = [pbc.tile([128, 520], dt.float32, tag=f"vp{kc}", name=f"vp{kc}")
                  for kc in range(KCN)]
            with tc.tile_pool(name="phb", bufs=1) as pb:
                XC["v"] = None  # aliased to XB below (in-place)
                SFt = pb.tile([128, TCN * 256], dt.float32, tag="BT_sh",
                              name="sf", bufs=2)
                SFv = SFt[:].rearrange("p (t n) -> p t n", t=TCN)
                XB = [pb.tile([128, 1024], dt.float32, tag=f"xb{rc}", name=f"xb{rc}")
                      for rc in range(RCN)]
                with (
                    tc.tile_pool(name="psA", bufs=4, space="PSUM") as psA,
                    tc.tile_pool(name="pxt", bufs=2) as px,
                ):
                    for sweep in range(2):
                        ps_sf = [psA.tile([128, 256], dt.float32, tag="ps_asf",
                                          name="ps_asf") for _ in range(4)]
                        ps_xb = [psA.tile([128, 512], dt.float32, tag="ps_axb",
                                          name="ps_axb") for _ in range(4)]
                        for kc in range(KCN):
                            xt = px.tile([128, 1024], dt.float32, tag="xtc",
                                         name="xtc")
                            nc.sync.dma_start(out=xt[:], in_=XTv[:, kc, :])
                            for ti in range(4):
                                nc.tensor.matmul(
                                    ps_sf[ti][:],
                                    xt[:, 128 * ti + 512 * sweep:
                                       128 * (ti + 1) + 512 * sweep],
                                    W4[:, kc, :],
                                    start=(kc == 0), stop=(kc == KCN - 1))
                            for i in range(4):
                                rc, th = 2 * sweep + i // 2, i % 2
                                nc.tensor.matmul(
                                    ps_xb[i][:],
                                    BI[:, kc, 128 * rc:128 * (rc + 1)],
                                    xt[:, 512 * th:512 * (th + 1)],
                                    start=(kc == 0), stop=(kc == KCN - 1))
                        for ti in range(4):
                            t = 4 * sweep + ti
                            nc.scalar.copy(SFv[:, t, :], ps_sf[ti][:])
                            if DEBUG:
                                nc.sync.dma_start(out=dbg["DSF"][t],
                                                  in_=SFv[:, t, :])
                        for i in range(4):
                            rc, th = 2 * sweep + i // 2, i % 2
                            nc.scalar.copy(
                                XB[rc][:, 512 * th:512 * (th + 1)], ps_xb[i][:])
                    if DEBUG:
                        for rc in range(RCN):
                            nc.sync.dma_start(out=dbg["DXB"][rc], in_=XB[rc][:])

                # ============ phase B: compress routes ============
                if PHASES < 2:
                    return _finish(nc)
                XC["v"] = XB
                with (
                    tc.tile_pool(name="psB", bufs=1, space="PSUM") as pps,
                    tc.tile_pool(name="psBt", bufs=2, space="PSUM") as ppt,
                    tc.tile_pool(name="psBv", bufs=1, space="PSUM") as ppv,
                ):
                    CTs = {}
                    for ri, r in enumerate("qkv"[:ROUTES]):
                        CHT = softmax_front(
                            pb, ppt, ppv, r,
                            SFv[:, :, 64 * ri:64 * ri + 64],
                            SFv[:, :, 192:256], W["GIN"][:])
                        neg_corr(pps, W["NEGBH"][:], CHT, XC[r], XB)
                        if DEBUG and r == "q":
                            for rc in range(RCN):
                                nc.sync.dma_start(out=dbg["DXCQ"][rc],
                                                  in_=XC[r][rc][:])
                    sts = {}
                    for ri, r in enumerate("qkv"[:ROUTES]):
                        if SUBB < 2:
                            continue
                        sts[r] = hh_chain_a(pb, pps, ppt, r, WDR["QKV"[ri]],
                                            XC[r],
                                            dbg_pfx=("Q" if r == "q" else None))
                    for ri, r in enumerate("qkv"[:ROUTES]):
                        if SUBB < 2 or SUBB < 3:
                            continue
                        CTs[r] = hh_chain_b(sts[r])
                        neg_corr(pps, W["NEGP"][:], CTs[r], XC[r], XC[r])
                    if DEBUG:
                        for rc in range(RCN):
                            nc.sync.dma_start(out=dbg["DQT"][rc],
                                              in_=XC["q"][rc][:])
                    # V -> N-domain V' with per-head 65-col blocks + ones
                    for rc in range(RCN if SUBB >= 3 else 0):
                        for t in range(TCN):
                            pt = ppt.tile([128, 128], dt.float32, tag="ps_t", name="ps_t")
                            nc.tensor.transpose(
                                pt[:], XC["v"][rc][:, 128 * t:128 * (t + 1)],
                                ident[:])
                            dst = bass.AP(
                                VP[t].tensor, VP[t].offset + 65 * (2 * rc),
                                [VP[t].ap[0], [65, 2], [1, 64]])
                            nc.scalar.copy(
                                dst, pt[:].rearrange("p (h n) -> p h n", h=2))
                    for t in range(TCN):
                        ones = VP[t][:].rearrange(
                            "p (h n) -> p h n", h=H)[:, :, 64:65]
                        nc.vector.memset(ones, 1.0)
                    if DEBUG:
                        for t in range(TCN):
                            nc.sync.dma_start(out=dbg["DVP"][t], in_=VP[t][:])

            # ================= phase C: attention =================
            if PHASES < 3:
                _pbc_cm.__exit__(None, None, None)
                return _finish(nc)
            with tc.tile_pool(name="att", bufs=1) as pa:
                AO = [pa.tile([128, 512], dt.float32, tag=f"ao{qt}", name=f"ao{qt}")
                      for qt in range(TCN)]
                def new_pt(hi):
                    t = pa.tile([128, KCN * 512], dt.float32, tag=f"pt{hi}",
                                name=f"pt{hi}", bufs=2)
                    return t[:].rearrange("p (k n) -> p k n", k=KCN)
                RSE = pa.tile([128, 8], dt.float32, tag="rse", name="rse")
                with (
                    tc.tile_pool(name="psC", bufs=3, space="PSUM") as pps,
                    tc.tile_pool(name="psCt", bufs=2, space="PSUM") as ppt,
                    tc.tile_pool(name="psCv", bufs=3, space="PSUM") as ppv,
                ):
                    for hp in range(4):
                        for qh in range(2):
                            for hi, h in enumerate((2 * hp, 2 * hp + 1)):
                                hr = 64 * hi
                                ptv = new_pt(hi)
                                for kc in range(KCN):
                                    ps = pps.tile([128, 512], dt.float32,
                                                  tag="ps_mm", name="ps_mm")
                                    nc.tensor.matmul(
                                        ps[:],
                                        XC["k"][hp][hr:hr + 64,
                                                    128 * kc:128 * (kc + 1)],
                                        XC["q"][hp][hr:hr + 64,
                                                    512 * qh:512 * (qh + 1)],
                                        start=True, stop=True)
                                    nc.scalar.activation(
                                        ptv[:, kc, :], ps[:], act.Exp,
                                        scale=0.125)
                                for qc in range(4):
                                    qt = 4 * qh + qc
                                    pv = ppv.tile([128, 65], dt.float32,
                                                  tag="ps_pv", name="ps_pv")
                                    for kc in range(KCN):
                                        nc.tensor.matmul(
                                            pv[:],
                                            ptv[:, kc,
                                                128 * qc:128 * (qc + 1)],
                                            VP[kc][:, 65 * h:65 * h + 65],
                                            start=(kc == 0),
                                            stop=(kc == KCN - 1))
                                    rse = RSE[:, h:h + 1]
                                    nc.vector.reciprocal(rse, pv[:, 64:65])
                                    nc.vector.tensor_scalar_mul(
                                        AO[qt][:, 64 * h:64 * (h + 1)],
                                        pv[:, 0:64], rse)
                    for rc in range(RCN):
                        for qt in range(TCN):
                            pt = ppt.tile([128, 128], dt.float32, tag="ps_t", name="ps_t")
                            nc.tensor.transpose(
                                pt[:], AO[qt][:, 128 * rc:128 * (rc + 1)],
                                ident[:])
                            nc.scalar.copy(
                                AOT[rc][:, 128 * qt:128 * (qt + 1)], pt[:])
                    if DEBUG:
                        for qt in range(TCN):
                            nc.sync.dma_start(out=dbg["DAO"][qt], in_=AO[qt][:])

            _pbc_cm.__exit__(None, None, None)
            # ================= phase D: expand =================
            if PHASES < 4:
                return _finish(nc)
            with tc.tile_pool(name="exp", bufs=1) as pe:
                xdt = f32r if F32R_OUT else dt.float32
                BOUTt = pe.tile([128, RCN * 1024], xdt, tag="boutw", name="boutw")
                nc.sync.dma_start(out=BOUTt[:], in_=wd["BOUT"][:].bitcast(xdt))
                BOUT = BOUTt[:].rearrange("p (k n) -> p k n", k=RCN)
                NEGOHHt = pe.tile([64, 1024], xdt, tag="negohhw", name="negohhw")
                nc.sync.dma_start(out=NEGOHHt[:], in_=wd["NEGOHH"][:].bitcast(xdt))
                WDRO2t = pe.tile([128, RCN * 256], xdt, tag="wdro2w", name="wdro2w")
                nc.sync.dma_start(out=WDRO2t[:], in_=wd["WDRO2"][:].bitcast(xdt))
                WDRO2 = WDRO2t[:].rearrange("p (k n) -> p k n", k=RCN)
                NEGPBOt = pe.tile([64, 1024], xdt, tag="negpbow", name="negpbow")
                nc.sync.dma_start(out=NEGPBOt[:], in_=wd["NEGPBO"][:].bitcast(xdt))
                NEGPOWt = pe.tile([64, 128], xdt, tag="negpoww", name="negpoww")
                nc.sync.dma_start(out=NEGPOWt[:], in_=wd["NEGPOW"][:].bitcast(xdt))
                NPBOHHt = pe.tile([128, 1024], xdt, tag="npbohhw", name="npbohhw")
                nc.sync.dma_start(out=NPBOHHt[:], in_=wd["NPBOHH"][:].bitcast(xdt))
                STK = pe.tile([128, 1024], xdt, tag="stk", name="stk")
                OUT1 = [pe.tile([128, 1024], dt.float32, tag=f"out1{dc}",
                                name=f"out1{dc}") for dc in range(KCN)]
                SO = pe.tile([128, TCN * 128], dt.float32, tag="so", name="so")
                SOv = SO[:].rearrange("p (t n) -> p t n", t=TCN)
                with (
                    tc.tile_pool(name="psD", bufs=1, space="PSUM") as pps,
                    tc.tile_pool(name="psDt", bufs=2, space="PSUM") as ppt,
                    tc.tile_pool(name="psDv", bufs=1, space="PSUM") as ppv,
                ):
                    CTo = hh_chain(pe, pps, ppt, "o", WDR["O"], AOT,
                                   ct_ap=STK[0:64, :])
                    so_n = 128
                    for t in range(TCN):
                        ps = pps.tile([128, so_n], dt.float32, tag="ps_sf",
                                      name="ps_sf", bufs=2)
                        for rc in range(RCN):
                            nc.tensor.matmul(
                                ps[:], AOT[rc][:, 128 * t:128 * (t + 1)],
                                WDRO2[:, rc, 0:so_n],
                                start=(rc == 0), stop=False)
                        nc.tensor.matmul(
                            ps[:], CTo[0:64, 128 * t:128 * (t + 1)],
                            NEGPOWt[0:64, :], start=False, stop=True)
                        nc.scalar.copy(SOv[:, t, :], ps[:, 0:128])
                    # ao @ base_out — emitted late so it back-fills PE stalls
                    for dc in range(KCN):
                        for th in range(2):
                            ps = pps.tile([128, 512], dt.float32, tag="ps_mm",
                                          name="ps_mm", bufs=3)
                            for rc in range(RCN):
                                nc.tensor.matmul(
                                    ps[:], BOUT[:, rc, 128 * dc:128 * (dc + 1)],
                                    AOT[rc][:, 512 * th:512 * (th + 1)],
                                    start=(rc == 0), stop=(rc == RCN - 1))
                            nc.scalar.copy(
                                OUT1[dc][:, 512 * th:512 * (th + 1)], ps[:])
                    CHoT = softmax_front(
                        pe, ppt, ppv, "o",
                        SOv[:, :, 0:64], SOv[:, :, 64:128], W["GOUT"][:],
                        cht_dtype=xdt, cht_ap=STK[64:128, :])
                    for dc in range(KCN):
                        for th in range(2):
                            ps = pps.tile([128, 512], dt.float32, tag="ps_mm",
                                          name="ps_mm", bufs=3)
                            nc.tensor.matmul(
                                ps[:], NPBOHHt[:, 128 * dc:128 * (dc + 1)],
                                STK[:, 512 * th:512 * (th + 1)],
                                start=True, stop=True)
                            ot = pe.tile([128, 512], dt.float32, tag="outsb",
                                         name="outsb", bufs=3)
                            nc.vector.tensor_add(
                                ot[:], OUT1[dc][:, 512 * th:512 * (th + 1)],
                                ps[:])
                            nc.sync.dma_start(
                                out=OUTd[128 * dc:128 * (dc + 1),
                                         512 * th:512 * (th + 1)],
                                in_=ot[:])
    _split_sync_waits(nc)
    return nc


def get_built():
    if "nc" not in _BUILT:
        _BUILT["nc"] = build()
    return _BUILT["nc"]


def kernel(**inputs):
    from concourse.bass_utils import run_bass_kernel_spmd

    x = np.asarray(inputs["x"], np.float32)
    w = prep_weights(inputs)
    nc = get_built()
    in_maps = []
    for c in range(NCORES):
        m = dict(w)
        m["XT"] = _pack_kc(np.ascontiguousarray(x[c].T), KCN, 128)
        in_maps.append(m)
    res = run_bass_kernel_spmd(nc, in_maps, core_ids=list(range(NCORES)))
    out = np.stack([res.results[c]["OUT"].T for c in range(NCORES)], axis=0)
    return out.astype(np.float32)


def run_timed(inputs, trace=False):
    from concourse.bass_utils import run_bass_kernel_spmd
    x = np.asarray(inputs["x"], np.float32)
    w = prep_weights(inputs)
    nc = get_built()
    in_maps = []
    for c in range(NCORES):
        m = dict(w)
        m["XT"] = _pack_kc(np.ascontiguousarray(x[c].T), KCN, 128)
        in_maps.append(m)
    return run_bass_kernel_spmd(nc, in_maps, core_ids=list(range(NCORES)),
                                trace=trace)



# revision 19
# speedup vs baseline: 1.4933x; 1.4933x over previous
"""Trainium2 Bass kernel for nn_NeuronAttention (moe_routing).

Sharding: data-parallel over batch B=8 across 8 NeuronCores (one batch row
per core); weights replicated; no collectives.

Per-core computation uses two layouts: "T-domain" [feature, token] for PE
GEMMs (contraction on partitions) and "N-domain" [token, small-free] for
routing math (softmax, top-k, Householder-chain recursion) on DVE/ACT.
The Householder chains are evaluated in 64-dim dot-space: with
d0 = xc@P.T, G = P@P.T, a = 1/(||P_k||^2+EPS), the 4 selected reflections
reduce to the scalar recursion beta_i = 2a_i(d0_i - sum_{j<i} beta_j G_ij)
and a rank-64 correction xc - (sum_i beta_i e_{idx_i})@P.

Precision plan (hw-measured: f32r keeps ~13 mantissa bits, fp16 11, and a
full-m12 emulation of this pipeline gives 5e-3 rel err vs the 2e-2 gate):
  - proc-router score GEMMs (feed top-k) stay true fp32 4-pass, reading
    fp32 xc tiles, so selections match the reference almost everywhere;
  - phase-A GEMMs (x@[routers|hh], x@base_in) run 1-pass f32r on host-
    pre-rounded operands;
  - the in-softmax chat correction runs f32r (chat rounded at ~2^-13);
  - attention QK runs fp16 on post-chain fp16 copies, exp'd scores are
    stored f32r (full fp32 exponent range - no overflow), and PV streams
    the exp'd matrix as the f32r moving operand, producing attention
    output directly in T-domain [feature, token];
  - softmax denominators use an appended all-ones stationary column and
    an exact reciprocal + 0/1-matmul partition-broadcast;
  - all post-selection / output GEMMs (one-hot gathers, chain corrections,
    x@base_out, final Householder correction) run fp16 1-pass.
"""

import numpy as np

B, S, D, R = 8, 1024, 1024, 512
NPROC, TOPK = 64, 4
H, DH = 8, 64
EPS = 1e-8
NCORES = 8
TCN = 8   # token chunks of 128
KCN = 8   # D chunks of 128
RCN = 4   # rank chunks of 128

_BUILT = {}


def _apply_tile_drain_patch():
    """walrus here rejects >1 sync-wait on CTRL-class instructions; split
    Tile's kernel-tail drain waits into a chain of single-wait nops."""
    import concourse.mybir as mybir
    from concourse.tile import TileContext
    from concourse.vector_clock import ScopedClock

    if getattr(TileContext, "_drain_patched", False):
        return

    def _patched(self, tick_clock, wait_clock):
        probe = self.nc.sync.nop()
        wait_clock.add_sem_waits(
            probe.ins, ScopedClock({None: tick_clock.global_clock}))
        si = probe.ins.sync_info
        waits = list(si.on_wait) if si is not None else []
        updates = list(si.on_update) if si is not None else []
        if len(waits) > 1:
            probe.ins.sync_info = mybir.SyncInfo(
                on_update=updates, on_wait=waits[:1])
            for ofs in range(1, len(waits)):
                extra = self.nc.sync.nop()
                extra.ins.sync_info = mybir.SyncInfo(
                    on_update=[], on_wait=waits[ofs:ofs + 1])
        self.nc.sync.drain()
        self.nc.all_engine_barrier()
        assert self.sems is not None
        popped = self.nc._tile_sem_poison_stack.pop()
        assert popped is self._sem_poison
        self.nc.clear_and_free_semaphores(list(self.sems.allocated().values()))
        self.nc.all_engine_barrier()

    TileContext._drain_and_barrier = _patched
    TileContext._drain_patched = True


def _split_sync_waits(nc):
    """walrus here accepts at most 1 sync-wait per instruction; hoist
    extra waits onto same-engine NoOps inserted just before."""
    import concourse.mybir as mybir

    ctr = [0]
    for f in nc.m.functions:
        for bb in f.blocks:
            insts = bb.instructions
            out = []
            for inst in insts:
                si = inst.sync_info
                if si is not None and len(si.on_wait) > 1:
                    waits = list(si.on_wait)
                    for w in waits[:-1]:
                        ctr[0] += 1
                        nop = mybir.InstNoOp(
                            name=f"I-sw{ctr[0]}", ins=[], outs=[])
                        nop.engine = inst.engine
                        nop.sync_info = mybir.SyncInfo(
                            on_update=[], on_wait=[w])
                        out.append(nop)
                    inst.sync_info = mybir.SyncInfo(
                        on_update=list(si.on_update), on_wait=[waits[-1]])
                out.append(inst)
            bb.instructions = out


def _pack_kc(a, nchunk, chunk, dtype=np.float32):
    # [nchunk*chunk, N] -> [chunk, nchunk*N], chunk-major partitions
    n = a.shape[1]
    return np.ascontiguousarray(
        a.reshape(nchunk, chunk, n).transpose(1, 0, 2).reshape(chunk, nchunk * n)
    ).astype(dtype)


def _r12(a, mbits=12):
    """Round to the f32r grid (12 explicit mantissa bits, RNE) so the PE's
    1-pass f32r read is exact."""
    a = np.asarray(a, np.float64)
    m, e = np.frexp(a)
    return np.ldexp(np.round(m * 2.0**mbits) / 2.0**mbits, e).astype(np.float32)


def prep_weights(inputs):
    f = {k: np.asarray(v, np.float64) for k, v in inputs.items()}
    P = f["process_hh"]
    G = P @ P.T
    alpha2 = 2.0 / ((P * P).sum(1) + EPS)
    ihh, ohh = f["input_hh"], f["output_hh"]
    base_in, base_out = f["base_input"], f["base_output"]
    Bo = ohh @ base_out.T

    w = {}
    w["BI"] = _r12(_pack_kc(base_in, KCN, 128))
    W4 = np.concatenate([f["q_in_router"].T, f["k_in_router"].T,
                         f["v_in_router"].T, ihh.T], axis=1)
    w["W4"] = _r12(_pack_kc(W4, KCN, 128))
    for nm, wp in (("WDRQ", "q_proc_router"), ("WDRK", "k_proc_router"),
                   ("WDRV", "v_proc_router"), ("WDRO", "o_proc_router")):
        w[nm] = _pack_kc(np.concatenate([P.T, f[wp].T], axis=1), RCN, 128)
    w["WDRO2"] = _pack_kc(
        np.concatenate([f["o_out_router"].T, Bo.T], axis=1), RCN, 128,
        np.float16)
    w["NEGBH"] = _r12(-(ihh @ base_in))
    w["NEGP"] = (-P).astype(np.float16)
    w["GIN"] = (ihh @ ihh.T).astype(np.float16)
    w["GOUT"] = (ohh @ ohh.T).astype(np.float16)
    BD = np.zeros((256, 196))
    for i in range(4):
        if i < 3:
            BD[64 * i:64 * i + 64, 64 * i:64 * i + 64] = -G
        BD[64 * i:64 * i + 64, 192 + i] = alpha2
    w["BD"] = _pack_kc(BD, 2, 128, np.float16)
    w["BOUT"] = _pack_kc(base_out, RCN, 128, np.float16)
    w["NEGPOW"] = (-(P @ np.concatenate(
        [f["o_out_router"].T, Bo.T], axis=1))).astype(np.float16)[:, 0:128]
    w["NPBOHH"] = np.concatenate(
        [-(P @ base_out), -ohh], axis=0).astype(np.float16)
    return w


def build():
    import concourse.bass as bass
    import concourse.mybir as mybir
    from concourse.tile import TileContext
    from concourse.masks import make_identity

    _apply_tile_drain_patch()
    dt = mybir.dt
    op = mybir.AluOpType
    act = mybir.ActivationFunctionType
    f32r = dt.float32r
    f16 = dt.float16

    nc = bass.Bass()
    XTd = nc.dram_tensor("XT", (128, KCN * 1024), dt.float32, kind="ExternalInput")
    wd = {}
    for nm, shape, wdt in (
        ("BI", (128, KCN * 512), dt.float32), ("W4", (128, KCN * 256), dt.float32),
        ("WDRQ", (128, RCN * 128), dt.float32), ("WDRK", (128, RCN * 128), dt.float32),
        ("WDRV", (128, RCN * 128), dt.float32), ("WDRO", (128, RCN * 128), dt.float32),
        ("WDRO2", (128, RCN * 128), f16),
        ("NEGBH", (64, 512), dt.float32), ("NEGP", (64, 512), f16),
        ("GIN", (64, 64), f16), ("GOUT", (64, 64), f16),
        ("BD", (128, 2 * 196), f16), ("BOUT", (128, RCN * 1024), f16),
        ("NEGPOW", (64, 128), f16), ("NPBOHH", (128, 1024), f16),
    ):
        wd[nm] = nc.dram_tensor(nm, shape, wdt, kind="ExternalInput")
    OUTd = nc.dram_tensor("OUT", (1024, 1024), dt.float32, kind="ExternalOutput")

    with TileContext(nc) as tc:
        with (
            tc.tile_pool(name="w", bufs=1) as pw,
            tc.tile_pool(name="live", bufs=1) as pl,
        ):
            W = {}
            for nm, dram in wd.items():
                if nm in ("BOUT", "WDRO2", "NEGPOW", "NPBOHH", "BI", "W4"):
                    continue
                if nm == "NEGBH":
                    # consumed by an f32r matmul; host pre-rounds, DMA as f32r
                    t = pw.tile(list(dram.shape), f32r, tag=nm)
                    nc.sync.dma_start(out=t[:], in_=dram[:].bitcast(f32r))
                else:
                    t = pw.tile(list(dram.shape), dram.dtype, tag=nm)
                    nc.sync.dma_start(out=t[:], in_=dram[:])
                W[nm] = t
            ident = pw.tile([128, 128], dt.float32, tag="ident", name="ident")
            make_identity(nc, ident[:])
            ident16 = pw.tile([128, 128], f16, tag="ident16", name="ident16")
            nc.scalar.copy(ident16[:], ident[:])
            ones32 = pw.tile([128, 8], dt.float32, tag="ones32", name="ones32")
            nc.vector.memset(ones32[:], 1.0)

            BD = W["BD"][:].rearrange("p (k n) -> p k n", k=2)
            WDR = {r: W["WDR" + r][:].rearrange("p (k n) -> p k n", k=RCN)
                   for r in ("Q", "K", "V", "O")}
            XTv = XTd[:].rearrange("p (k n) -> p k n", k=KCN)

            # persistent activations: attention output, T-domain
            AOT = [pl.tile([128, 1024], dt.float32, tag=f"aot{rc}", name=f"aot{rc}")
                   for rc in range(RCN)]
            AOT16 = [pl.tile([128, 1024], f16, tag=f"aot16{rc}", name=f"aot16{rc}")
                     for rc in range(RCN)]

            # ---------- shared helpers ----------

            def softmax_front(pool, ppt, ppv, name, s_all, f_all, gram,
                              cht_dtype=None, cht_ap=None):
                """Batched over all 8 token chunks. s_all/f_all are
                [128, 8, 64] APs. Returns CHT [64, 1024] (chat^T)."""
                E = pool.tile([128, TCN * 64], f16, tag="E_sh",
                              name=f"E_{name}", bufs=2)
                Ev = E[:].rearrange("p (t n) -> p t n", t=TCN)
                ET = pool.tile([64, 1024], f16, tag="ET_sh",
                               name=f"ET_{name}", bufs=1)
                CH = pool.tile([128, TCN * 64], f16, tag="CH_sh",
                               name=f"CH_{name}", bufs=2)
                CHv = CH[:].rearrange("p (t n) -> p t n", t=TCN)
                CHT = cht_ap if cht_ap is not None else pool.tile(
                    [64, 1024], cht_dtype or f32r,
                    tag="CHT_sh", name=f"CHT_{name}", bufs=1)
                SC = pool.tile([128, 8 * 8], dt.float32, tag=f"sc1_{name}",
                               name=f"sc1_{name}")
                SCv = SC[:].rearrange("p (t n) -> p t n", t=8)
                scr = pool.tile([128, 512], dt.float32, tag="scr_sh",
                                name=f"scr_{name}", bufs=2)

                nc.scalar.activation(Ev[:, :, :], s_all, act.Exp)
                Z8 = SCv[:, :, 0:1]
                nc.vector.tensor_reduce(Z8, Ev[:, :, :],
                                        mybir.AxisListType.X, op.add)
                # u = e @ Gin per chunk, packed into one PSUM bank
                pu = ppv.tile([128, 512], dt.float32, tag="ps_u", name="ps_u")
                for t in range(TCN):
                    pt = ppt.tile([128, 128], f16, tag="ps_t16", name="ps_t16")
                    nc.tensor.transpose(pt[0:64, :], Ev[:, t, :], ident16[:])
                    nc.scalar.copy(ET[:, 128 * t:128 * (t + 1)], pt[0:64, :])
                    nc.tensor.matmul(pu[:, 64 * t:64 * (t + 1)],
                                     ET[:, 128 * t:128 * (t + 1)], gram,
                                     start=True, stop=True)
                puv = pu[:].rearrange("p (t n) -> p t n", t=TCN)
                pacc, qacc = SCv[:, :, 1:2], SCv[:, :, 2:3]
                nc.vector.tensor_mul(scr[:], Ev[:, :, :], f_all)
                nc.vector.tensor_reduce(
                    pacc, scr[:].rearrange("p (t n) -> p t n", t=TCN),
                    mybir.AxisListType.X, op.add)
                nc.vector.tensor_mul(scr[:], Ev[:, :, :], puv)
                nc.vector.tensor_reduce(
                    qacc, scr[:].rearrange("p (t n) -> p t n", t=TCN),
                    mybir.AxisListType.X, op.add)
                z2, den = SCv[:, :, 3:4], SCv[:, :, 4:5]
                rec, gam = SCv[:, :, 5:6], SCv[:, :, 6:7]
                nc.vector.tensor_mul(z2, Z8, Z8)
                nc.vector.scalar_tensor_tensor(out=den, in0=z2, scalar=EPS,
                                               in1=qacc, op0=op.mult, op1=op.add)
                nc.vector.reciprocal(rec, den)
                nc.vector.scalar_tensor_tensor(out=gam, in0=pacc, scalar=2.0,
                                               in1=rec, op0=op.mult, op1=op.mult)
                nc.vector.tensor_mul(CHv[:, :, :], Ev[:, :, :],
                                     gam.to_broadcast((128, TCN, 64)))
                for t in range(TCN):
                    pt2 = ppt.tile([128, 128], f16, tag="ps_t16", name="ps_t16")
                    nc.tensor.transpose(pt2[0:64, :], CHv[:, t, :], ident16[:])
                    nc.scalar.copy(CHT[:, 128 * t:128 * (t + 1)], pt2[0:64, :])
                return CHT

            def hh_chain_a(pool, pps, ppt, name, wdr, src_tiles, ct_ap=None):
                """Stage a: D0/R GEMM (true fp32 - feeds top-k), top-4
                one-hots, d0 selects, -G row gather. Returns tile dict."""
                D0 = pool.tile([128, TCN * 64], f16, tag="D0_sh",
                               name=f"D0_{name}", bufs=2)
                D0v = D0[:].rearrange("p (t n) -> p t n", t=TCN)
                RS = pool.tile([128, TCN * 64], dt.float32, tag="RS_sh",
                               name=f"RS_{name}", bufs=2)
                RSv = RS[:].rearrange("p (t n) -> p t n", t=TCN)
                M8 = pool.tile([128, TCN * 8], dt.float32, tag="M8_sh",
                               name=f"M8_{name}", bufs=1)
                M8v = M8[:].rearrange("p (t n) -> p t n", t=TCN)
                OH = pool.tile([128, TCN * 256], f16, tag="OH_sh",
                               name=f"OH_{name}", bufs=1)
                OHv = OH[:].rearrange("p (t n) -> p t n", t=TCN)
                OHT = pool.tile([128, 256], f16, tag="OHT_sh",
                                name=f"OHT_{name}", bufs=2)
                BT = pool.tile([128, TCN * 196], f16, tag="BT_sh",
                               name=f"BT_{name}", bufs=2)
                BTv = BT[:].rearrange("p (t n) -> p t n", t=TCN)
                DSA = pool.tile([128, TCN * 24], dt.float32, tag="DS_sh2",
                                name=f"DS_{name}", bufs=2)
                DSAv = DSA[:].rearrange("p (t n) -> p t n", t=TCN)
                BE = pool.tile([128, TCN * 4], dt.float32, tag=f"BE_{name}",
                               name=f"BE_{name}")
                BEv = BE[:].rearrange("p (t n) -> p t n", t=TCN)
                CC = pool.tile([128, TCN * 64], f16, tag="CC_sh",
                               name=f"CC_{name}", bufs=2)
                CCv = CC[:].rearrange("p (t n) -> p t n", t=TCN)
                CT = ct_ap if ct_ap is not None else pool.tile(
                    [64, 1024], f16, tag="CT_sh", name=f"CT_{name}", bufs=1)
                scr = pool.tile([128, 512], f16, tag="scr16_sh",
                                name=f"scr16_{name}", bufs=2)
                scrv = scr[:].rearrange("p (t n) -> p t n", t=TCN)

                # D0/R: pack 4 chunks per PSUM bank (true fp32, 4-pass)
                psd = [pps.tile([128, 512], dt.float32, tag="ps_sf",
                                name="ps_sf", bufs=2)
                       for _ in range(2)]
                for t in range(TCN):
                    for rc in range(RCN):
                        nc.tensor.matmul(
                            psd[t // 4][:, 128 * (t % 4):128 * (t % 4 + 1)],
                            src_tiles[rc][:, 128 * t:128 * (t + 1)],
                            wdr[:, rc, :],
                            start=(rc == 0), stop=(rc == RCN - 1))
                for half in range(2):
                    pv = psd[half][:].rearrange("p (t n) -> p t n", t=4)
                    nc.scalar.copy(D0v[:, 4 * half:4 * half + 4, :],
                                   pv[:, :, 0:64])
                    nc.scalar.copy(RSv[:, 4 * half:4 * half + 4, :],
                                   pv[:, :, 64:128])
                for t in range(TCN):
                    nc.vector.max(M8v[:, t, :], RSv[:, t, :])
                # one-hots + d0 selects (batched over chunks)
                for i in range(4):
                    nc.vector.tensor_tensor(
                        OHv[:, :, 64 * i:64 * (i + 1)], RSv[:, :, :],
                        M8v[:, :, i:i + 1].to_broadcast((128, TCN, 64)),
                        op.is_equal)
                    nc.vector.tensor_mul(scr[:], OHv[:, :, 64 * i:64 * (i + 1)],
                                         D0v[:, :, :])
                    nc.vector.tensor_reduce(DSAv[:, :, i:i + 1], scrv,
                                            mybir.AxisListType.X, op.add)
                # gather -G rows + 2alpha via transposed one-hot GEMM (fp16)
                for t in range(TCN):
                    psb = pps.tile([128, 196], dt.float32, tag="ps_mm",
                                   name="ps_b", bufs=3)
                    for half in range(2):
                        pt = ppt.tile([128, 128], f16, tag="ps_t16",
                                      name="ps_t16")
                        nc.tensor.transpose(
                            pt[:], OHv[:, t, 128 * half:128 * (half + 1)],
                            ident16[:])
                        nc.scalar.copy(OHT[:, 128 * half:128 * (half + 1)], pt[:])
                        nc.tensor.matmul(
                            psb[:], OHT[:, 128 * half:128 * (half + 1)],
                            BD[:, half, :], start=(half == 0), stop=(half == 1))
                    nc.scalar.copy(BTv[:, t, :], psb[:])
                return dict(OHv=OHv, BTv=BTv, DSAv=DSAv, BEv=BEv,
                            CCv=CCv, CC=CC, CT=CT, scr=scr, scrv=scrv)

            def hh_chain_b(ppt, st):
                OHv, BTv, DSAv = st["OHv"], st["BTv"], st["DSAv"]
                BEv, CCv, CC = st["BEv"], st["CCv"], st["CC"]
                CT, scr, scrv = st["CT"], st["scr"], st["scrv"]
                # pair values -G[idx_i, idx_j]
                pair = {}
                pidx = 4
                for i in range(1, 4):
                    for j in range(i):
                        nc.vector.tensor_mul(
                            scr[:], OHv[:, :, 64 * i:64 * (i + 1)],
                            BTv[:, :, 64 * j:64 * (j + 1)])
                        nc.vector.tensor_reduce(
                            DSAv[:, :, pidx:pidx + 1], scrv,
                            mybir.AxisListType.X, op.add)
                        pair[(i, j)] = DSAv[:, :, pidx:pidx + 1]
                        pidx += 1
                # recursion (batched [128, 8] ops)
                be = [BEv[:, :, i:i + 1] for i in range(4)]
                a2 = [BTv[:, :, 192 + i:193 + i] for i in range(4)]
                nc.vector.tensor_mul(be[0], DSAv[:, :, 0:1], a2[0])
                tmp = 10
                for i in range(1, 4):
                    cur = DSAv[:, :, i:i + 1]
                    for j in range(i):
                        t1 = DSAv[:, :, tmp:tmp + 1]; tmp += 1
                        nc.vector.tensor_mul(t1, pair[(i, j)], be[j])
                        t2 = DSAv[:, :, tmp:tmp + 1]; tmp += 1
                        nc.vector.tensor_add(t2, t1, cur)
                        cur = t2
                    nc.vector.tensor_mul(be[i], cur, a2[i])
                # c = sum beta_i * onehot_i
                nc.vector.tensor_mul(CCv[:, :, :], OHv[:, :, 0:64],
                                     be[0].to_broadcast((128, TCN, 64)))
                for i in range(1, 4):
                    nc.vector.tensor_mul(
                        scr[:], OHv[:, :, 64 * i:64 * (i + 1)],
                        be[i].to_broadcast((128, TCN, 64)))
                    nc.vector.tensor_add(CC[:], CC[:], scr[:])
                for t in range(TCN):
                    ptc = ppt.tile([128, 128], f16, tag="ps_t16", name="ps_t16")
                    nc.tensor.transpose(ptc[0:64, :], CCv[:, t, :], ident16[:])
                    nc.scalar.copy(CT[:, 128 * t:128 * (t + 1)], ptc[0:64, :])
                return CT

            def hh_chain(pool, pps, ppt, name, wdr, src_tiles, ct_ap=None):
                st = hh_chain_a(pool, pps, ppt, name, wdr, src_tiles,
                                ct_ap=ct_ap)
                return hh_chain_b(ppt, st)

            # ================= phase A: SF + xbT (f32r 1-pass) =============
            _pbc_cm = tc.tile_pool(name="bc", bufs=1)
            pbc = _pbc_cm.__enter__()
            # post-chain fp16 activations for attention
            XA = {r: [pbc.tile([128, 1024], f16, tag=f"xa{r}{rc}",
                               name=f"xa{r}{rc}")
                      for rc in range(RCN)] for r in "qkv"}
            VP = [pbc.tile([128, 520], f32r, tag=f"vp{kc}", name=f"vp{kc}")
                  for kc in range(KCN)]
            with tc.tile_pool(name="phb", bufs=1) as pb:
                SFt = pb.tile([128, TCN * 256], f16, tag="sf",
                              name="sf", bufs=1)
                SFv = SFt[:].rearrange("p (t n) -> p t n", t=TCN)
                XB = [pb.tile([128, 1024], dt.float32, tag=f"xb{rc}",
                              name=f"xb{rc}")
                      for rc in range(RCN)]
                # xc buffers rotate across routes (2 in flight)
                XC = {r: [pb.tile([128, 1024], dt.float32, tag=f"xc{rc}",
                                  name=f"xc{r}{rc}", bufs=2)
                          for rc in range(RCN)] for r in "qkv"}
                with (
                    tc.tile_pool(name="pha", bufs=1) as pa_,
                    tc.tile_pool(name="phx", bufs=3) as px,
                    tc.tile_pool(name="psA", bufs=4, space="PSUM") as psA,
                ):
                    BIt = pa_.tile([128, KCN * 512], f32r, tag="BIw", name="BIw")
                    nc.sync.dma_start(out=BIt[:], in_=wd["BI"][:].bitcast(f32r))
                    W4t = pa_.tile([128, KCN * 256], f32r, tag="W4w", name="W4w")
                    nc.sync.dma_start(out=W4t[:], in_=wd["W4"][:].bitcast(f32r))
                    BI = BIt[:].rearrange("p (k n) -> p k n", k=KCN)
                    W4 = W4t[:].rearrange("p (k n) -> p k n", k=KCN)
                    for sweep in range(2):
                        ps_sf = [psA.tile([128, 256], dt.float32, tag="ps_asf",
                                          name="ps_asf") for _ in range(4)]
                        ps_xb = [psA.tile([128, 512], dt.float32, tag="ps_axb",
                                          name="ps_axb") for _ in range(4)]
                        for kc in range(KCN):
                            xt = px.tile([128, 1024], f32r, tag="xtc",
                                         name="xtc")
                            nc.sync.dma_start(out=xt[:],
                                              in_=XTv[:, kc, :].bitcast(f32r))
                            for ti in range(4):
                                nc.tensor.matmul(
                                    ps_sf[ti][:],
                                    xt[:, 128 * ti + 512 * sweep:
                                       128 * (ti + 1) + 512 * sweep],
                                    W4[:, kc, :],
                                    start=(kc == 0), stop=(kc == KCN - 1))
                            for i in range(4):
                                rc, th = 2 * sweep + i // 2, i % 2
                                nc.tensor.matmul(
                                    ps_xb[i][:],
                                    BI[:, kc, 128 * rc:128 * (rc + 1)],
                                    xt[:, 512 * th:512 * (th + 1)],
                                    start=(kc == 0), stop=(kc == KCN - 1))
                        for ti in range(4):
                            t = 4 * sweep + ti
                            nc.scalar.copy(SFv[:, t, :], ps_sf[ti][:])
                        for i in range(4):
                            rc, th = 2 * sweep + i // 2, i % 2
                            nc.scalar.copy(
                                XB[rc][:, 512 * th:512 * (th + 1)], ps_xb[i][:])

                # ============ phase B: compress routes ============
                with (
                    tc.tile_pool(name="psB", bufs=1, space="PSUM") as pps,
                    tc.tile_pool(name="psBt", bufs=2, space="PSUM") as ppt,
                    tc.tile_pool(name="psBv", bufs=1, space="PSUM") as ppv,
                ):
                    for ri, r in enumerate("qkv"):
                        CHT = softmax_front(
                            pb, ppt, ppv, r,
                            SFv[:, :, 64 * ri:64 * ri + 64],
                            SFv[:, :, 192:256], W["GIN"][:])
                        # chat correction: f32r 1-pass GEMM + DVE add
                        for rc in range(RCN):
                            for th in range(2):
                                ps = pps.tile([128, 512], dt.float32,
                                              tag="ps_mm", name="ps_mm", bufs=3)
                                nc.tensor.matmul(
                                    ps[:],
                                    W["NEGBH"][:][0:64, 128 * rc:128 * (rc + 1)],
                                    CHT[0:64, 512 * th:512 * (th + 1)],
                                    start=True, stop=True)
                                nc.vector.tensor_add(
                                    XC[r][rc][:, 512 * th:512 * (th + 1)],
                                    XB[rc][:, 512 * th:512 * (th + 1)], ps[:])
                        CT = hh_chain(pb, pps, ppt, r, WDR["QKV"[ri]], XC[r])
                        # chain correction: fp16 1-pass GEMM + DVE add -> fp16
                        for rc in range(RCN):
                            for th in range(2):
                                ps = pps.tile([128, 512], dt.float32,
                                              tag="ps_mm", name="ps_mm", bufs=3)
                                nc.tensor.matmul(
                                    ps[:], W["NEGP"][:][
                                        0:64, 128 * rc:128 * (rc + 1)],
                                    CT[0:64, 512 * th:512 * (th + 1)],
                                    start=True, stop=True)
                                nc.vector.tensor_add(
                                    XA[r][rc][:, 512 * th:512 * (th + 1)],
                                    XC[r][rc][:, 512 * th:512 * (th + 1)],
                                    ps[:])
                    # V -> N-domain V' (f32r) with per-head 65-col blocks+ones
                    for rc in range(RCN):
                        for t in range(TCN):
                            pt = ppt.tile([128, 128], f16, tag="ps_t16",
                                          name="ps_t16")
                            nc.tensor.transpose(
                                pt[:], XA["v"][rc][:, 128 * t:128 * (t + 1)],
                                ident16[:])
                            dst = bass.AP(
                                VP[t].tensor, VP[t].offset + 65 * (2 * rc),
                                [VP[t].ap[0], [65, 2], [1, 64]])
                            nc.scalar.copy(
                                dst, pt[:].rearrange("p (h n) -> p h n", h=2))
                    for t in range(TCN):
                        ones = VP[t][:].rearrange(
                            "p (h n) -> p h n", h=H)[:, :, 64:65]
                        nc.scalar.copy(ones, ones32[:, 0:8].rearrange(
                            "p (h n) -> p h n", h=H))

            # ================= phase C: attention =================
            with tc.tile_pool(name="att", bufs=1) as pa:
                # per-(head, token) softmax denominators, broadcast across each
                # 64-row head block by a stride-0-partition DMA from PSUM
                ZINV = [pa.tile([128, 1024], dt.float32, tag=f"zinv{rc}",
                                name=f"zinv{rc}") for rc in range(RCN)]
                ZRI = pa.tile([1, 16 * 512], dt.float32, tag="zri", name="zri")

                def new_pt(hi):
                    t = pa.tile([128, KCN * 512], f32r, tag=f"pt{hi}",
                                name=f"pt{hi}", bufs=1)
                    return t[:].rearrange("p (k n) -> p k n", k=KCN)
                with (
                    tc.tile_pool(name="psC", bufs=3, space="PSUM") as pps,
                    tc.tile_pool(name="psCv", bufs=2, space="PSUM") as ppv,
                ):
                    for hp in range(4):
                        for qh in range(2):
                            for hi, h in enumerate((2 * hp, 2 * hp + 1)):
                                hr = 64 * hi
                                ptv = new_pt(hi)
                                for kc in range(KCN):
                                    ps = pps.tile([128, 512], dt.float32,
                                                  tag="ps_mm", name="ps_mm")
                                    nc.tensor.matmul(
                                        ps[:],
                                        XA["k"][hp][hr:hr + 64,
                                                    128 * kc:128 * (kc + 1)],
                                        XA["q"][hp][hr:hr + 64,
                                                    512 * qh:512 * (qh + 1)],
                                        start=True, stop=True)
                                    nc.scalar.activation(
                                        ptv[:, kc, :], ps[:], act.Exp,
                                        scale=0.125)
                                pv65 = ppv.tile([128, 512], dt.float32,
                                                tag="ps_pv", name="ps_pv")
                                for kc in range(KCN):
                                    nc.tensor.matmul(
                                        pv65[0:65, :],
                                        VP[kc][:, 65 * h:65 * h + 65],
                                        ptv[:, kc, :],
                                        start=(kc == 0), stop=(kc == KCN - 1))
                                nc.scalar.copy(
                                    AOT[hp][hr:hr + 64,
                                            512 * qh:512 * (qh + 1)],
                                    pv65[0:64, :])
                                zofs = 512 * (2 * h + qh)
                                nc.vector.reciprocal(
                                    ZRI[0:1, zofs:zofs + 512], pv65[64:65, :])
                                zsrc = bass.AP(
                                    ZRI.tensor, ZRI.offset + zofs,
                                    [ZRI.ap[0], [0, 64], [1, 512]])
                                nc.sync.dma_start(
                                    out=ZINV[hp][64 * hi:64 * (hi + 1),
                                                 512 * qh:512 * (qh + 1)],
                                    in_=zsrc)
                    # normalize + fp16 cast
                    for rc in range(RCN):
                        for th in range(2):
                            nc.vector.tensor_mul(
                                AOT[rc][:, 512 * th:512 * (th + 1)],
                                AOT[rc][:, 512 * th:512 * (th + 1)],
                                ZINV[rc][:, 512 * th:512 * (th + 1)])
                    for rc in range(RCN):
                        nc.gpsimd.tensor_copy(AOT16[rc][:], AOT[rc][:])

            _pbc_cm.__exit__(None, None, None)
            # ================= phase D: expand =================
            with tc.tile_pool(name="exp", bufs=1) as pe:
                BOUTt = pe.tile([128, RCN * 1024], f16, tag="boutw", name="boutw")
                nc.sync.dma_start(out=BOUTt[:], in_=wd["BOUT"][:])
                BOUT = BOUTt[:].rearrange("p (k n) -> p k n", k=RCN)
                WDRO2t = pe.tile([128, RCN * 128], f16, tag="wdro2w",
                                 name="wdro2w")
                nc.sync.dma_start(out=WDRO2t[:], in_=wd["WDRO2"][:])
                WDRO2 = WDRO2t[:].rearrange("p (k n) -> p k n", k=RCN)
                NEGPOWt = pe.tile([64, 128], f16, tag="negpoww", name="negpoww")
                nc.sync.dma_start(out=NEGPOWt[:], in_=wd["NEGPOW"][:])
                NPBOHHt = pe.tile([128, 1024], f16, tag="npbohhw",
                                  name="npbohhw")
                nc.sync.dma_start(out=NPBOHHt[:], in_=wd["NPBOHH"][:])
                STK = pe.tile([128, 1024], f16, tag="stk", name="stk")
                OUT1 = [pe.tile([128, 1024], dt.float32, tag=f"out1{dc}",
                                name=f"out1{dc}") for dc in range(KCN)]
                SO = pe.tile([128, TCN * 128], dt.float32, tag="so", name="so")
                SOv = SO[:].rearrange("p (t n) -> p t n", t=TCN)
                with (
                    tc.tile_pool(name="psD", bufs=1, space="PSUM") as pps,
                    tc.tile_pool(name="psDt", bufs=2, space="PSUM") as ppt,
                    tc.tile_pool(name="psDv", bufs=1, space="PSUM") as ppv,
                ):
                    CTo = hh_chain(pe, pps, ppt, "o", WDR["O"], AOT,
                                   ct_ap=STK[0:64, :])
                    for t in range(TCN):
                        ps = pps.tile([128, 128], dt.float32, tag="ps_sf",
                                      name="ps_sf", bufs=2)
                        for rc in range(RCN):
                            nc.tensor.matmul(
                                ps[:], AOT16[rc][:, 128 * t:128 * (t + 1)],
                                WDRO2[:, rc, :],
                                start=(rc == 0), stop=False)
                        nc.tensor.matmul(
                            ps[:], CTo[0:64, 128 * t:128 * (t + 1)],
                            NEGPOWt[0:64, :], start=False, stop=True)
                        nc.scalar.copy(SOv[:, t, :], ps[:, 0:128])
                    # ao @ base_out - emitted late so it back-fills PE stalls
                    for dc in range(KCN):
                        for th in range(2):
                            ps = pps.tile([128, 512], dt.float32, tag="ps_mm",
                                          name="ps_mm", bufs=3)
                            for rc in range(RCN):
                                nc.tensor.matmul(
                                    ps[:], BOUT[:, rc, 128 * dc:128 * (dc + 1)],
                                    AOT16[rc][:, 512 * th:512 * (th + 1)],
                                    start=(rc == 0), stop=(rc == RCN - 1))
                            nc.scalar.copy(
                                OUT1[dc][:, 512 * th:512 * (th + 1)], ps[:])
                    softmax_front(
                        pe, ppt, ppv, "o",
                        SOv[:, :, 0:64], SOv[:, :, 64:128], W["GOUT"][:],
                        cht_dtype=f16, cht_ap=STK[64:128, :])
                    for dc in range(KCN):
                        for th in range(2):
                            ps = pps.tile([128, 512], dt.float32, tag="ps_mm",
                                          name="ps_mm", bufs=3)
                            nc.tensor.matmul(
                                ps[:], NPBOHHt[:][:, 128 * dc:128 * (dc + 1)],
                                STK[:, 512 * th:512 * (th + 1)],
                                start=True, stop=True)
                            ot = pe.tile([128, 512], dt.float32, tag="outsb",
                                         name="outsb", bufs=3)
                            nc.vector.tensor_add(
                                ot[:], OUT1[dc][:, 512 * th:512 * (th + 1)],
                                ps[:])
                            nc.sync.dma_start(
                                out=OUTd[128 * dc:128 * (dc + 1),
                                         512 * th:512 * (th + 1)],
                                in_=ot[:])
    _split_sync_waits(nc)
    return nc


def get_built():
    if "nc" not in _BUILT:
        _BUILT["nc"] = build()
    return _BUILT["nc"]


def _in_maps(inputs):
    x = np.asarray(inputs["x"], np.float32)
    w = prep_weights(inputs)
    in_maps = []
    for c in range(NCORES):
        m = dict(w)
        m["XT"] = _r12(_pack_kc(np.ascontiguousarray(x[c].T), KCN, 128))
        in_maps.append(m)
    return in_maps


def kernel(**inputs):
    from concourse.bass_utils import run_bass_kernel_spmd

    nc = get_built()
    res = run_bass_kernel_spmd(nc, _in_maps(inputs),
                               core_ids=list(range(NCORES)))
    out = np.stack([res.results[c]["OUT"].T for c in range(NCORES)], axis=0)
    return out.astype(np.float32)


def run_timed(inputs, trace=False):
    from concourse.bass_utils import run_bass_kernel_spmd
    nc = get_built()
    return run_bass_kernel_spmd(nc, _in_maps(inputs),
                                core_ids=list(range(NCORES)), trace=trace)


# revision 26
# speedup vs baseline: 1.6471x; 1.1030x over previous
"""Trainium2 Bass kernel for nn_NeuronAttention (moe_routing).

Sharding: data-parallel over batch B=8 across 8 NeuronCores (one batch row
per core); weights replicated; no collectives.

Per-core computation uses two layouts: "T-domain" [feature, token] for PE
GEMMs (contraction on partitions) and "N-domain" [token, small-free] for
routing math (softmax, top-k, Householder-chain recursion) on DVE/ACT.
The Householder chains are evaluated in 64-dim dot-space: with
d0 = xc@P.T, G = P@P.T, a = 1/(||P_k||^2+EPS), the 4 selected reflections
reduce to the scalar recursion beta_i = 2a_i(d0_i - sum_{j<i} beta_j G_ij)
and a rank-64 correction xc - (sum_i beta_i e_{idx_i})@P.

Precision plan (hw-measured: f32r keeps ~13 mantissa bits, fp16 11, and a
full-m12 emulation of this pipeline gives 5e-3 rel err vs the 2e-2 gate):
  - proc-router score GEMMs (feed top-k) stay true fp32 4-pass, reading
    fp32 xc tiles, so selections match the reference almost everywhere;
  - phase-A GEMMs (x@[routers|hh], x@base_in) run 1-pass f32r on host-
    pre-rounded operands;
  - the in-softmax chat correction runs f32r (chat rounded at ~2^-13);
  - attention QK runs fp16 on post-chain fp16 copies, exp'd scores are
    stored f32r (full fp32 exponent range - no overflow), and PV streams
    the exp'd matrix as the f32r moving operand, producing attention
    output directly in T-domain [feature, token];
  - softmax denominators use an appended all-ones stationary column and
    an exact reciprocal + 0/1-matmul partition-broadcast;
  - all post-selection / output GEMMs (one-hot gathers, chain corrections,
    x@base_out, final Householder correction) run fp16 1-pass.
"""

import numpy as np

B, S, D, R = 8, 1024, 1024, 512
NPROC, TOPK = 64, 4
H, DH = 8, 64
EPS = 1e-8
NCORES = 8
TCN = 8   # token chunks of 128
KCN = 8   # D chunks of 128
RCN = 4   # rank chunks of 128

_BUILT = {}


def _apply_tile_drain_patch():
    """walrus here rejects >1 sync-wait on CTRL-class instructions; split
    Tile's kernel-tail drain waits into a chain of single-wait nops."""
    import concourse.mybir as mybir
    from concourse.tile import TileContext
    from concourse.vector_clock import ScopedClock

    if getattr(TileContext, "_drain_patched", False):
        return

    def _patched(self, tick_clock, wait_clock):
        probe = self.nc.sync.nop()
        wait_clock.add_sem_waits(
            probe.ins, ScopedClock({None: tick_clock.global_clock}))
        si = probe.ins.sync_info
        waits = list(si.on_wait) if si is not None else []
        updates = list(si.on_update) if si is not None else []
        if len(waits) > 1:
            probe.ins.sync_info = mybir.SyncInfo(
                on_update=updates, on_wait=waits[:1])
            for ofs in range(1, len(waits)):
                extra = self.nc.sync.nop()
                extra.ins.sync_info = mybir.SyncInfo(
                    on_update=[], on_wait=waits[ofs:ofs + 1])
        self.nc.sync.drain()
        self.nc.all_engine_barrier()
        assert self.sems is not None
        popped = self.nc._tile_sem_poison_stack.pop()
        assert popped is self._sem_poison
        self.nc.clear_and_free_semaphores(list(self.sems.allocated().values()))
        self.nc.all_engine_barrier()

    TileContext._drain_and_barrier = _patched
    TileContext._drain_patched = True


def _split_sync_waits(nc):
    """walrus here accepts at most 1 sync-wait per instruction; hoist
    extra waits onto same-engine NoOps inserted just before."""
    import concourse.mybir as mybir

    ctr = [0]
    for f in nc.m.functions:
        for bb in f.blocks:
            insts = bb.instructions
            out = []
            for inst in insts:
                si = inst.sync_info
                if si is not None and len(si.on_wait) > 1:
                    waits = list(si.on_wait)
                    for w in waits[:-1]:
                        ctr[0] += 1
                        nop = mybir.InstNoOp(
                            name=f"I-sw{ctr[0]}", ins=[], outs=[])
                        nop.engine = inst.engine
                        nop.sync_info = mybir.SyncInfo(
                            on_update=[], on_wait=[w])
                        out.append(nop)
                    inst.sync_info = mybir.SyncInfo(
                        on_update=list(si.on_update), on_wait=[waits[-1]])
                out.append(inst)
            bb.instructions = out


def _pack_kc(a, nchunk, chunk, dtype=np.float32):
    # [nchunk*chunk, N] -> [chunk, nchunk*N], chunk-major partitions
    n = a.shape[1]
    return np.ascontiguousarray(
        a.reshape(nchunk, chunk, n).transpose(1, 0, 2).reshape(chunk, nchunk * n)
    ).astype(dtype)


def _r12(a, mbits=12):
    """Round to the f32r grid (12 explicit mantissa bits, RNE) so the PE's
    1-pass f32r read is exact."""
    a = np.asarray(a, np.float64)
    m, e = np.frexp(a)
    return np.ldexp(np.round(m * 2.0**mbits) / 2.0**mbits, e).astype(np.float32)


def prep_weights(inputs):
    f = {k: np.asarray(v, np.float64) for k, v in inputs.items()}
    P = f["process_hh"]
    G = P @ P.T
    alpha2 = 2.0 / ((P * P).sum(1) + EPS)
    ihh, ohh = f["input_hh"], f["output_hh"]
    base_in, base_out = f["base_input"], f["base_output"]
    Bo = ohh @ base_out.T

    w = {}
    w["BI"] = _r12(_pack_kc(base_in, KCN, 128))
    W4 = np.concatenate([f["q_in_router"].T, f["k_in_router"].T,
                         f["v_in_router"].T, ihh.T], axis=1)
    w["W4"] = _r12(_pack_kc(W4, KCN, 128))
    for nm, wp in (("WDRQ", "q_proc_router"), ("WDRK", "k_proc_router"),
                   ("WDRV", "v_proc_router"), ("WDRO", "o_proc_router")):
        w[nm] = _pack_kc(np.concatenate([P.T, f[wp].T], axis=1), RCN, 128)
    w["WDRO2"] = _pack_kc(
        np.concatenate([f["o_out_router"].T, Bo.T], axis=1), RCN, 128,
        np.float16)
    w["NEGBH"] = _r12(-(ihh @ base_in))
    w["NEGP"] = (-P).astype(np.float16)
    w["GIN"] = (ihh @ ihh.T).astype(np.float16)
    w["GOUT"] = (ohh @ ohh.T).astype(np.float16)
    BD = np.zeros((256, 196))
    for i in range(4):
        if i < 3:
            BD[64 * i:64 * i + 64, 64 * i:64 * i + 64] = -G
        BD[64 * i:64 * i + 64, 192 + i] = alpha2
    w["BD"] = _pack_kc(BD, 2, 128, np.float16)
    w["BOUT"] = _pack_kc(base_out, RCN, 128, np.float16)
    w["NEGPOW"] = (-(P @ np.concatenate(
        [f["o_out_router"].T, Bo.T], axis=1))).astype(np.float16)[:, 0:128]
    w["NPBOHH"] = np.concatenate(
        [-(P @ base_out), -ohh], axis=0).astype(np.float16)
    return w


def build():
    import concourse.bass as bass
    import concourse.mybir as mybir
    from concourse.tile import TileContext
    from concourse.masks import make_identity

    _apply_tile_drain_patch()
    dt = mybir.dt
    op = mybir.AluOpType
    act = mybir.ActivationFunctionType
    f32r = dt.float32r
    f16 = dt.float16

    nc = bass.Bass()
    XTd = nc.dram_tensor("XT", (128, KCN * 1024), dt.float32, kind="ExternalInput")
    wd = {}
    for nm, shape, wdt in (
        ("BI", (128, KCN * 512), dt.float32), ("W4", (128, KCN * 256), dt.float32),
        ("WDRQ", (128, RCN * 128), dt.float32), ("WDRK", (128, RCN * 128), dt.float32),
        ("WDRV", (128, RCN * 128), dt.float32), ("WDRO", (128, RCN * 128), dt.float32),
        ("WDRO2", (128, RCN * 128), f16),
        ("NEGBH", (64, 512), dt.float32), ("NEGP", (64, 512), f16),
        ("GIN", (64, 64), f16), ("GOUT", (64, 64), f16),
        ("BD", (128, 2 * 196), f16), ("BOUT", (128, RCN * 1024), f16),
        ("NEGPOW", (64, 128), f16), ("NPBOHH", (128, 1024), f16),
    ):
        wd[nm] = nc.dram_tensor(nm, shape, wdt, kind="ExternalInput")
    OUTd = nc.dram_tensor("OUT", (1024, 1024), dt.float32, kind="ExternalOutput")

    with TileContext(nc) as tc:
        with (
            tc.tile_pool(name="w", bufs=1) as pw,
            tc.tile_pool(name="live", bufs=1) as pl,
        ):
            # small weights: issue on the Pool sequencer's DMA queue so the
            # SP queue serves BI/W4/xt first (PE's critical path at start)
            W = {}
            for nm, dram in wd.items():
                if nm in ("BOUT", "WDRO2", "NEGPOW", "NPBOHH", "BI", "W4"):
                    continue
                if nm == "NEGBH":
                    # consumed by an f32r matmul; host pre-rounds, DMA as f32r
                    t = pw.tile(list(dram.shape), f32r, tag=nm)
                    nc.gpsimd.dma_start(out=t[:], in_=dram[:].bitcast(f32r))
                else:
                    t = pw.tile(list(dram.shape), dram.dtype, tag=nm)
                    nc.gpsimd.dma_start(out=t[:], in_=dram[:])
                W[nm] = t
            ident = pw.tile([128, 128], dt.float32, tag="ident", name="ident")
            make_identity(nc, ident[:])
            ident16 = pw.tile([128, 128], f16, tag="ident16", name="ident16")
            nc.scalar.copy(ident16[:], ident[:])
            ones32 = pw.tile([128, 8], dt.float32, tag="ones32", name="ones32")
            nc.vector.memset(ones32[:], 1.0)

            BD = W["BD"][:].rearrange("p (k n) -> p k n", k=2)
            WDR = {r: W["WDR" + r][:].rearrange("p (k n) -> p k n", k=RCN)
                   for r in ("Q", "K", "V", "O")}
            XTv = XTd[:].rearrange("p (k n) -> p k n", k=KCN)

            # persistent activations: attention output, T-domain
            AOT = [pl.tile([128, 1024], dt.float32, tag=f"aot{rc}", name=f"aot{rc}")
                   for rc in range(RCN)]
            AOT16 = [pl.tile([128, 1024], f16, tag=f"aot16{rc}", name=f"aot16{rc}")
                     for rc in range(RCN)]

            # ---------- shared helpers ----------

            def softmax_front(pool, ppt, ppv, name, s_all, f_all, gram,
                              cht_dtype=None, cht_ap=None):
                """Batched over all 8 token chunks. s_all/f_all are
                [128, 8, 64] APs. Returns CHT [64, 1024] (chat^T)."""
                E = pool.tile([128, TCN * 64], f16, tag="E_sh",
                              name=f"E_{name}", bufs=2)
                Ev = E[:].rearrange("p (t n) -> p t n", t=TCN)
                ET = pool.tile([64, 1024], f16, tag="ET_sh",
                               name=f"ET_{name}", bufs=1)
                CH = pool.tile([128, TCN * 64], f16, tag="CH_sh",
                               name=f"CH_{name}", bufs=2)
                CHv = CH[:].rearrange("p (t n) -> p t n", t=TCN)
                CHT = cht_ap if cht_ap is not None else pool.tile(
                    [64, 1024], cht_dtype or f32r,
                    tag="CHT_sh", name=f"CHT_{name}", bufs=1)
                SC = pool.tile([128, 8 * 8], dt.float32, tag=f"sc1_{name}",
                               name=f"sc1_{name}")
                SCv = SC[:].rearrange("p (t n) -> p t n", t=8)
                scr = pool.tile([128, 512], dt.float32, tag="scr_sh",
                                name=f"scr_{name}", bufs=2)

                nc.scalar.activation(Ev[:, :, :], s_all, act.Exp)
                Z8 = SCv[:, :, 0:1]
                nc.vector.tensor_reduce(Z8, Ev[:, :, :],
                                        mybir.AxisListType.X, op.add)
                # u = e @ Gin per chunk, packed into one PSUM bank
                pu = ppv.tile([128, 512], dt.float32, tag="ps_u", name="ps_u")
                for t in range(TCN):
                    pt = ppt.tile([128, 128], f16, tag="ps_t16", name="ps_t16")
                    nc.tensor.transpose(pt[0:64, :], Ev[:, t, :], ident16[:])
                    nc.scalar.copy(ET[:, 128 * t:128 * (t + 1)], pt[0:64, :])
                    nc.tensor.matmul(pu[:, 64 * t:64 * (t + 1)],
                                     ET[:, 128 * t:128 * (t + 1)], gram,
                                     start=True, stop=True)
                puv = pu[:].rearrange("p (t n) -> p t n", t=TCN)
                pacc, qacc = SCv[:, :, 1:2], SCv[:, :, 2:3]
                nc.vector.tensor_mul(scr[:], Ev[:, :, :], f_all)
                nc.vector.tensor_reduce(
                    pacc, scr[:].rearrange("p (t n) -> p t n", t=TCN),
                    mybir.AxisListType.X, op.add)
                nc.vector.tensor_mul(scr[:], Ev[:, :, :], puv)
                nc.vector.tensor_reduce(
                    qacc, scr[:].rearrange("p (t n) -> p t n", t=TCN),
                    mybir.AxisListType.X, op.add)
                z2, den = SCv[:, :, 3:4], SCv[:, :, 4:5]
                rec, gam = SCv[:, :, 5:6], SCv[:, :, 6:7]
                nc.vector.tensor_mul(z2, Z8, Z8)
                nc.vector.scalar_tensor_tensor(out=den, in0=z2, scalar=EPS,
                                               in1=qacc, op0=op.mult, op1=op.add)
                nc.vector.reciprocal(rec, den)
                nc.vector.scalar_tensor_tensor(out=gam, in0=pacc, scalar=2.0,
                                               in1=rec, op0=op.mult, op1=op.mult)
                nc.vector.tensor_mul(CHv[:, :, :], Ev[:, :, :],
                                     gam.to_broadcast((128, TCN, 64)))
                for t in range(TCN):
                    pt2 = ppt.tile([128, 128], f16, tag="ps_t16", name="ps_t16")
                    nc.tensor.transpose(pt2[0:64, :], CHv[:, t, :], ident16[:])
                    nc.scalar.copy(CHT[:, 128 * t:128 * (t + 1)], pt2[0:64, :])
                return CHT

            def hh_chain_a(pool, pps, ppt, name, wdr, src_tiles, ct_ap=None):
                """Stage a: D0/R GEMM (true fp32 - feeds top-k), top-4
                one-hots, d0 selects, -G row gather. Returns tile dict."""
                D0 = pool.tile([128, TCN * 64], f16, tag="D0_sh",
                               name=f"D0_{name}", bufs=2)
                D0v = D0[:].rearrange("p (t n) -> p t n", t=TCN)
                RS = pool.tile([128, TCN * 64], dt.float32, tag="RS_sh",
                               name=f"RS_{name}", bufs=2)
                RSv = RS[:].rearrange("p (t n) -> p t n", t=TCN)
                M8 = pool.tile([128, TCN * 8], dt.float32, tag="M8_sh",
                               name=f"M8_{name}", bufs=1)
                M8v = M8[:].rearrange("p (t n) -> p t n", t=TCN)
                OH = pool.tile([128, TCN * 256], f16, tag="OH_sh",
                               name=f"OH_{name}", bufs=1)
                OHv = OH[:].rearrange("p (t n) -> p t n", t=TCN)
                OHT = pool.tile([128, 256], f16, tag="OHT_sh",
                                name=f"OHT_{name}", bufs=2)
                BT = pool.tile([128, TCN * 196], f16, tag="BT_sh",
                               name=f"BT_{name}", bufs=2)
                BTv = BT[:].rearrange("p (t n) -> p t n", t=TCN)
                DSA = pool.tile([128, TCN * 24], dt.float32, tag="DS_sh2",
                                name=f"DS_{name}", bufs=2)
                DSAv = DSA[:].rearrange("p (t n) -> p t n", t=TCN)
                BE = pool.tile([128, TCN * 4], dt.float32, tag=f"BE_{name}",
                               name=f"BE_{name}")
                BEv = BE[:].rearrange("p (t n) -> p t n", t=TCN)
                CC = pool.tile([128, TCN * 64], f16, tag="CC_sh",
                               name=f"CC_{name}", bufs=2)
                CCv = CC[:].rearrange("p (t n) -> p t n", t=TCN)
                CT = ct_ap if ct_ap is not None else pool.tile(
                    [64, 1024], f16, tag="CT_sh", name=f"CT_{name}", bufs=1)
                scr = pool.tile([128, 512], f16, tag="scr16_sh",
                                name=f"scr16_{name}", bufs=2)
                scrv = scr[:].rearrange("p (t n) -> p t n", t=TCN)

                # D0/R: pack 4 chunks per PSUM bank (true fp32, 4-pass)
                psd = [pps.tile([128, 512], dt.float32, tag="ps_sf",
                                name="ps_sf", bufs=2)
                       for _ in range(2)]
                for t in range(TCN):
                    for rc in range(RCN):
                        nc.tensor.matmul(
                            psd[t // 4][:, 128 * (t % 4):128 * (t % 4 + 1)],
                            src_tiles[rc][:, 128 * t:128 * (t + 1)],
                            wdr[:, rc, :],
                            start=(rc == 0), stop=(rc == RCN - 1))
                for half in range(2):
                    pv = psd[half][:].rearrange("p (t n) -> p t n", t=4)
                    nc.scalar.copy(D0v[:, 4 * half:4 * half + 4, :],
                                   pv[:, :, 0:64])
                    nc.scalar.copy(RSv[:, 4 * half:4 * half + 4, :],
                                   pv[:, :, 64:128])
                for t in range(TCN):
                    nc.vector.max(M8v[:, t, :], RSv[:, t, :])
                # one-hots + d0 selects (batched over chunks)
                for i in range(4):
                    nc.vector.tensor_tensor(
                        OHv[:, :, 64 * i:64 * (i + 1)], RSv[:, :, :],
                        M8v[:, :, i:i + 1].to_broadcast((128, TCN, 64)),
                        op.is_equal)
                    nc.vector.tensor_mul(scr[:], OHv[:, :, 64 * i:64 * (i + 1)],
                                         D0v[:, :, :])
                    nc.vector.tensor_reduce(DSAv[:, :, i:i + 1], scrv,
                                            mybir.AxisListType.X, op.add)
                # gather -G rows + 2alpha via transposed one-hot GEMM (fp16)
                for t in range(TCN):
                    psb = pps.tile([128, 196], dt.float32, tag="ps_mm",
                                   name="ps_b", bufs=3)
                    for half in range(2):
                        pt = ppt.tile([128, 128], f16, tag="ps_t16",
                                      name="ps_t16")
                        nc.tensor.transpose(
                            pt[:], OHv[:, t, 128 * half:128 * (half + 1)],
                            ident16[:])
                        nc.scalar.copy(OHT[:, 128 * half:128 * (half + 1)], pt[:])
                        nc.tensor.matmul(
                            psb[:], OHT[:, 128 * half:128 * (half + 1)],
                            BD[:, half, :], start=(half == 0), stop=(half == 1))
                    nc.scalar.copy(BTv[:, t, :], psb[:])
                return dict(OHv=OHv, BTv=BTv, DSAv=DSAv, BEv=BEv,
                            CCv=CCv, CC=CC, CT=CT, scr=scr, scrv=scrv)

            def hh_chain_b(ppt, st):
                OHv, BTv, DSAv = st["OHv"], st["BTv"], st["DSAv"]
                BEv, CCv, CC = st["BEv"], st["CCv"], st["CC"]
                CT, scr, scrv = st["CT"], st["scr"], st["scrv"]
                # pair values -G[idx_i, idx_j]
                pair = {}
                pidx = 4
                for i in range(1, 4):
                    for j in range(i):
                        nc.vector.tensor_mul(
                            scr[:], OHv[:, :, 64 * i:64 * (i + 1)],
                            BTv[:, :, 64 * j:64 * (j + 1)])
                        nc.vector.tensor_reduce(
                            DSAv[:, :, pidx:pidx + 1], scrv,
                            mybir.AxisListType.X, op.add)
                        pair[(i, j)] = DSAv[:, :, pidx:pidx + 1]
                        pidx += 1
                # recursion (batched [128, 8] ops)
                be = [BEv[:, :, i:i + 1] for i in range(4)]
                a2 = [BTv[:, :, 192 + i:193 + i] for i in range(4)]
                nc.vector.tensor_mul(be[0], DSAv[:, :, 0:1], a2[0])
                tmp = 10
                for i in range(1, 4):
                    cur = DSAv[:, :, i:i + 1]
                    for j in range(i):
                        t1 = DSAv[:, :, tmp:tmp + 1]; tmp += 1
                        nc.vector.tensor_mul(t1, pair[(i, j)], be[j])
                        t2 = DSAv[:, :, tmp:tmp + 1]; tmp += 1
                        nc.vector.tensor_add(t2, t1, cur)
                        cur = t2
                    nc.vector.tensor_mul(be[i], cur, a2[i])
                # c = sum beta_i * onehot_i
                nc.vector.tensor_mul(CCv[:, :, :], OHv[:, :, 0:64],
                                     be[0].to_broadcast((128, TCN, 64)))
                for i in range(1, 4):
                    nc.vector.tensor_mul(
                        scr[:], OHv[:, :, 64 * i:64 * (i + 1)],
                        be[i].to_broadcast((128, TCN, 64)))
                    nc.vector.tensor_add(CC[:], CC[:], scr[:])
                for t in range(TCN):
                    ptc = ppt.tile([128, 128], f16, tag="ps_t16", name="ps_t16")
                    nc.tensor.transpose(ptc[0:64, :], CCv[:, t, :], ident16[:])
                    nc.scalar.copy(CT[:, 128 * t:128 * (t + 1)], ptc[0:64, :])
                return CT

            def hh_chain(pool, pps, ppt, name, wdr, src_tiles, ct_ap=None):
                st = hh_chain_a(pool, pps, ppt, name, wdr, src_tiles,
                                ct_ap=ct_ap)
                return hh_chain_b(ppt, st)

            # ================= phase A: SF + xbT (f32r 1-pass) =============
            _pbc_cm = tc.tile_pool(name="bc", bufs=1)
            pbc = _pbc_cm.__enter__()
            # post-chain fp16 activations for attention
            XA = {r: [pbc.tile([128, 1024], f16, tag=f"xa{r}{rc}",
                               name=f"xa{r}{rc}")
                      for rc in range(RCN)] for r in "qkv"}
            VP = [pbc.tile([128, 520], f32r, tag=f"vp{kc}", name=f"vp{kc}")
                  for kc in range(KCN)]
            with tc.tile_pool(name="phb", bufs=1) as pb:
                SFt = pb.tile([128, TCN * 256], f16, tag="sf",
                              name="sf", bufs=1)
                SFv = SFt[:].rearrange("p (t n) -> p t n", t=TCN)
                XB = [pb.tile([128, 1024], dt.float32, tag=f"xb{rc}",
                              name=f"xb{rc}")
                      for rc in range(RCN)]
                # xc buffers rotate across routes (2 in flight)
                XC = {r: [pb.tile([128, 1024], dt.float32, tag=f"xc{rc}",
                                  name=f"xc{r}{rc}", bufs=2)
                          for rc in range(RCN)] for r in "qkv"}
                with (
                    tc.tile_pool(name="pha", bufs=1) as pa_,
                    tc.tile_pool(name="phx", bufs=3) as px,
                    tc.tile_pool(name="psA", bufs=4, space="PSUM") as psA,
                ):
                    W4t = pa_.tile([128, KCN * 256], f32r, tag="W4w", name="W4w")
                    nc.sync.dma_start(out=W4t[:], in_=wd["W4"][:].bitcast(f32r))
                    BIt = pa_.tile([128, KCN * 512], f32r, tag="BIw", name="BIw")
                    nc.scalar.dma_start(out=BIt[:], in_=wd["BI"][:].bitcast(f32r))
                    BI = BIt[:].rearrange("p (k n) -> p k n", k=KCN)
                    W4 = W4t[:].rearrange("p (k n) -> p k n", k=KCN)
                    for sweep in range(2):
                        ps_sf = [psA.tile([128, 256], dt.float32, tag="ps_asf",
                                          name="ps_asf") for _ in range(4)]
                        ps_xb = [psA.tile([128, 512], dt.float32, tag="ps_axb",
                                          name="ps_axb") for _ in range(4)]
                        for kc in range(KCN):
                            xt = px.tile([128, 1024], f32r, tag="xtc",
                                         name="xtc")
                            nc.sync.dma_start(out=xt[:],
                                              in_=XTv[:, kc, :].bitcast(f32r))
                            for ti in range(4):
                                nc.tensor.matmul(
                                    ps_sf[ti][:],
                                    xt[:, 128 * ti + 512 * sweep:
                                       128 * (ti + 1) + 512 * sweep],
                                    W4[:, kc, :],
                                    start=(kc == 0), stop=(kc == KCN - 1))
                            for i in range(4):
                                rc, th = 2 * sweep + i // 2, i % 2
                                nc.tensor.matmul(
                                    ps_xb[i][:],
                                    BI[:, kc, 128 * rc:128 * (rc + 1)],
                                    xt[:, 512 * th:512 * (th + 1)],
                                    start=(kc == 0), stop=(kc == KCN - 1))
                        for ti in range(4):
                            t = 4 * sweep + ti
                            nc.scalar.copy(SFv[:, t, :], ps_sf[ti][:])
                        for i in range(4):
                            rc, th = 2 * sweep + i // 2, i % 2
                            nc.vector.tensor_copy(
                                XB[rc][:, 512 * th:512 * (th + 1)], ps_xb[i][:])

                # ============ phase B: compress routes ============
                with (
                    tc.tile_pool(name="psB", bufs=1, space="PSUM") as pps,
                    tc.tile_pool(name="psBt", bufs=2, space="PSUM") as ppt,
                    tc.tile_pool(name="psBv", bufs=1, space="PSUM") as ppv,
                ):
                    def front_corr(ri, r):
                        """softmax_front + chat-correction for route r."""
                        CHT = softmax_front(
                            pb, ppt, ppv, r,
                            SFv[:, :, 64 * ri:64 * ri + 64],
                            SFv[:, :, 192:256], W["GIN"][:])
                        for rc in range(RCN):
                            for th in range(2):
                                ps = pps.tile([128, 512], dt.float32,
                                              tag="ps_mm", name="ps_mm", bufs=3)
                                nc.tensor.matmul(
                                    ps[:],
                                    W["NEGBH"][:][0:64, 128 * rc:128 * (rc + 1)],
                                    CHT[0:64, 512 * th:512 * (th + 1)],
                                    start=True, stop=True)
                                nc.vector.tensor_add(
                                    XC[r][rc][:, 512 * th:512 * (th + 1)],
                                    XB[rc][:, 512 * th:512 * (th + 1)], ps[:])

                    # software-pipelined routes: route r+1's front/correction
                    # is emitted between route r's chain_a and chain_b so every
                    # engine has fill work during r's serial recursion tail
                    front_corr(0, "q")
                    for ri, r in enumerate("qkv"):
                        st = hh_chain_a(pb, pps, ppt, r, WDR["QKV"[ri]], XC[r])
                        if ri < 2:
                            front_corr(ri + 1, "qkv"[ri + 1])
                        CT = hh_chain_b(ppt, st)
                        # chain correction: fp16 1-pass GEMM + DVE add -> fp16
                        for rc in range(RCN):
                            for th in range(2):
                                ps = pps.tile([128, 512], dt.float32,
                                              tag="ps_mm", name="ps_mm", bufs=3)
                                nc.tensor.matmul(
                                    ps[:], W["NEGP"][:][
                                        0:64, 128 * rc:128 * (rc + 1)],
                                    CT[0:64, 512 * th:512 * (th + 1)],
                                    start=True, stop=True)
                                nc.vector.tensor_add(
                                    XA[r][rc][:, 512 * th:512 * (th + 1)],
                                    XC[r][rc][:, 512 * th:512 * (th + 1)],
                                    ps[:])
                    # V -> N-domain V' (f32r) with per-head 65-col blocks+ones
                    for rc in range(RCN):
                        for t in range(TCN):
                            pt = ppt.tile([128, 128], f16, tag="ps_t16",
                                          name="ps_t16")
                            nc.tensor.transpose(
                                pt[:], XA["v"][rc][:, 128 * t:128 * (t + 1)],
                                ident16[:])
                            dst = bass.AP(
                                VP[t].tensor, VP[t].offset + 65 * (2 * rc),
                                [VP[t].ap[0], [65, 2], [1, 64]])
                            nc.scalar.copy(
                                dst, pt[:].rearrange("p (h n) -> p h n", h=2))
                    for t in range(TCN):
                        ones = VP[t][:].rearrange(
                            "p (h n) -> p h n", h=H)[:, :, 64:65]
                        nc.scalar.copy(ones, ones32[:, 0:8].rearrange(
                            "p (h n) -> p h n", h=H))

            # ================= phase C: attention =================
            with tc.tile_pool(name="att", bufs=1) as pa:
                # per-(head, token) softmax denominators, broadcast across each
                # 64-row head block by a stride-0-partition DMA from PSUM
                ZINV = [pa.tile([128, 1024], dt.float32, tag=f"zinv{rc}",
                                name=f"zinv{rc}") for rc in range(RCN)]
                ZRI = pa.tile([1, 16 * 512], dt.float32, tag="zri", name="zri")

                def new_pt(hi):
                    t = pa.tile([128, KCN * 512], f32r, tag=f"pt{hi}",
                                name=f"pt{hi}", bufs=2)
                    return t[:].rearrange("p (k n) -> p k n", k=KCN)
                with (
                    tc.tile_pool(name="psC", bufs=3, space="PSUM") as pps,
                    tc.tile_pool(name="psCv", bufs=2, space="PSUM") as ppv,
                ):
                    for hp in range(4):
                        for qh in range(2):
                            for hi, h in enumerate((2 * hp, 2 * hp + 1)):
                                hr = 64 * hi
                                ptv = new_pt(hi)
                                for g in range(4):
                                    ps2 = pps.tile([128, 1024], dt.float32,
                                                   tag="ps2", name="ps2",
                                                   bufs=2)
                                    for j in range(2):
                                        kc = 2 * g + j
                                        nc.tensor.matmul(
                                            ps2[:, 512 * j:512 * (j + 1)],
                                            XA["k"][hp][hr:hr + 64,
                                                        128 * kc:128 * (kc + 1)],
                                            XA["q"][hp][hr:hr + 64,
                                                        512 * qh:512 * (qh + 1)],
                                            start=True, stop=True)
                                    nc.scalar.activation(
                                        ptv[:, 2 * g:2 * g + 2, :], ps2[:],
                                        act.Exp, scale=0.125)
                                pv65 = ppv.tile([128, 512], dt.float32,
                                                tag="ps_pv", name="ps_pv")
                                for kc in range(KCN):
                                    nc.tensor.matmul(
                                        pv65[0:65, :],
                                        VP[kc][:, 65 * h:65 * h + 65],
                                        ptv[:, kc, :],
                                        start=(kc == 0), stop=(kc == KCN - 1))
                                nc.vector.tensor_copy(
                                    AOT[hp][hr:hr + 64,
                                            512 * qh:512 * (qh + 1)],
                                    pv65[0:64, :])
                                zofs = 512 * (2 * h + qh)
                                nc.vector.reciprocal(
                                    ZRI[0:1, zofs:zofs + 512], pv65[64:65, :])
                                zsrc = bass.AP(
                                    ZRI.tensor, ZRI.offset + zofs,
                                    [ZRI.ap[0], [0, 64], [1, 512]])
                                nc.sync.dma_start(
                                    out=ZINV[hp][64 * hi:64 * (hi + 1),
                                                 512 * qh:512 * (qh + 1)],
                                    in_=zsrc)
                    # normalize + fp16 cast
                    for rc in range(RCN):
                        for th in range(2):
                            nc.gpsimd.tensor_mul(
                                AOT[rc][:, 512 * th:512 * (th + 1)],
                                AOT[rc][:, 512 * th:512 * (th + 1)],
                                ZINV[rc][:, 512 * th:512 * (th + 1)])
                    for rc in range(RCN):
                        nc.gpsimd.tensor_copy(AOT16[rc][:], AOT[rc][:])

            _pbc_cm.__exit__(None, None, None)
            # ================= phase D: expand =================
            with tc.tile_pool(name="exp", bufs=1) as pe:
                BOUTt = pe.tile([128, RCN * 1024], f16, tag="boutw", name="boutw")
                nc.sync.dma_start(out=BOUTt[:], in_=wd["BOUT"][:])
                BOUT = BOUTt[:].rearrange("p (k n) -> p k n", k=RCN)
                WDRO2t = pe.tile([128, RCN * 128], f16, tag="wdro2w",
                                 name="wdro2w")
                nc.sync.dma_start(out=WDRO2t[:], in_=wd["WDRO2"][:])
                WDRO2 = WDRO2t[:].rearrange("p (k n) -> p k n", k=RCN)
                NEGPOWt = pe.tile([64, 128], f16, tag="negpoww", name="negpoww")
                nc.sync.dma_start(out=NEGPOWt[:], in_=wd["NEGPOW"][:])
                NPBOHHt = pe.tile([128, 1024], f16, tag="npbohhw",
                                  name="npbohhw")
                nc.sync.dma_start(out=NPBOHHt[:], in_=wd["NPBOHH"][:])
                STK = pe.tile([128, 1024], f16, tag="stk", name="stk")
                OUT1 = [pe.tile([128, 1024], dt.float32, tag=f"out1{dc}",
                                name=f"out1{dc}") for dc in range(KCN)]
                SO = pe.tile([128, TCN * 128], dt.float32, tag="so", name="so")
                SOv = SO[:].rearrange("p (t n) -> p t n", t=TCN)
                with (
                    tc.tile_pool(name="psD", bufs=1, space="PSUM") as pps,
                    tc.tile_pool(name="psDt", bufs=2, space="PSUM") as ppt,
                    tc.tile_pool(name="psDv", bufs=1, space="PSUM") as ppv,
                ):
                    CTo = hh_chain(pe, pps, ppt, "o", WDR["O"], AOT,
                                   ct_ap=STK[0:64, :])
                    for t in range(TCN):
                        ps = pps.tile([128, 128], dt.float32, tag="ps_sf",
                                      name="ps_sf", bufs=2)
                        for rc in range(RCN):
                            nc.tensor.matmul(
                                ps[:], AOT16[rc][:, 128 * t:128 * (t + 1)],
                                WDRO2[:, rc, :],
                                start=(rc == 0), stop=False)
                        nc.tensor.matmul(
                            ps[:], CTo[0:64, 128 * t:128 * (t + 1)],
                            NEGPOWt[0:64, :], start=False, stop=True)
                        nc.scalar.copy(SOv[:, t, :], ps[:, 0:128])
                    # ao @ base_out - emitted late so it back-fills PE stalls
                    for dc in range(KCN):
                        for th in range(2):
                            ps = pps.tile([128, 512], dt.float32, tag="ps_mm",
                                          name="ps_mm", bufs=3)
                            for rc in range(RCN):
                                nc.tensor.matmul(
                                    ps[:], BOUT[:, rc, 128 * dc:128 * (dc + 1)],
                                    AOT16[rc][:, 512 * th:512 * (th + 1)],
                                    start=(rc == 0), stop=(rc == RCN - 1))
                            nc.vector.tensor_copy(
                                OUT1[dc][:, 512 * th:512 * (th + 1)], ps[:])
                    softmax_front(
                        pe, ppt, ppv, "o",
                        SOv[:, :, 0:64], SOv[:, :, 64:128], W["GOUT"][:],
                        cht_dtype=f16, cht_ap=STK[64:128, :])
                    for dc in range(KCN):
                        for th in range(2):
                            ps = pps.tile([128, 512], dt.float32, tag="ps_mm",
                                          name="ps_mm", bufs=3)
                            nc.tensor.matmul(
                                ps[:], NPBOHHt[:][:, 128 * dc:128 * (dc + 1)],
                                STK[:, 512 * th:512 * (th + 1)],
                                start=True, stop=True)
                            ot = pe.tile([128, 512], dt.float32, tag="outsb",
                                         name="outsb", bufs=3)
                            nc.vector.tensor_add(
                                ot[:], OUT1[dc][:, 512 * th:512 * (th + 1)],
                                ps[:])
                            nc.sync.dma_start(
                                out=OUTd[128 * dc:128 * (dc + 1),
                                         512 * th:512 * (th + 1)],
                                in_=ot[:])
    _split_sync_waits(nc)
    return nc


def get_built():
    if "nc" not in _BUILT:
        _BUILT["nc"] = build()
    return _BUILT["nc"]


def _in_maps(inputs):
    x = np.asarray(inputs["x"], np.float32)
    w = prep_weights(inputs)
    in_maps = []
    for c in range(NCORES):
        m = dict(w)
        m["XT"] = _r12(_pack_kc(np.ascontiguousarray(x[c].T), KCN, 128))
        in_maps.append(m)
    return in_maps


def kernel(**inputs):
    from concourse.bass_utils import run_bass_kernel_spmd

    nc = get_built()
    res = run_bass_kernel_spmd(nc, _in_maps(inputs),
                               core_ids=list(range(NCORES)))
    out = np.stack([res.results[c]["OUT"].T for c in range(NCORES)], axis=0)
    return out.astype(np.float32)


def run_timed(inputs, trace=False):
    from concourse.bass_utils import run_bass_kernel_spmd
    nc = get_built()
    return run_bass_kernel_spmd(nc, _in_maps(inputs),
                                core_ids=list(range(NCORES)), trace=trace)


# revision 40
# speedup vs baseline: 1.7461x; 1.0601x over previous
"""Trainium2 Bass kernel for nn_NeuronAttention (moe_routing).

Sharding: data-parallel over batch B=8 across 8 NeuronCores (one batch row
per core); weights replicated; no collectives.

Per-core computation uses two layouts: "T-domain" [feature, token] for PE
GEMMs (contraction on partitions) and "N-domain" [token, small-free] for
routing math (softmax, top-k, Householder-chain recursion) on DVE/ACT.
The Householder chains are evaluated in 64-dim dot-space: with
d0 = xc@P.T, G = P@P.T, a = 1/(||P_k||^2+EPS), the 4 selected reflections
reduce to the scalar recursion beta_i = 2a_i(d0_i - sum_{j<i} beta_j G_ij)
and a rank-64 correction xc - (sum_i beta_i e_{idx_i})@P.

Precision plan (hw-measured: f32r keeps ~13 mantissa bits, fp16 11, and a
full-m12 emulation of this pipeline gives 5e-3 rel err vs the 2e-2 gate):
  - proc-router score GEMMs (feed top-k) stay true fp32 4-pass, reading
    fp32 xc tiles, so selections match the reference almost everywhere;
  - phase-A GEMMs (x@[routers|hh], x@base_in) run 1-pass f32r on host-
    pre-rounded operands;
  - the in-softmax chat correction runs f32r (chat rounded at ~2^-13);
  - attention QK runs fp16 on post-chain fp16 copies, exp'd scores are
    stored f32r (full fp32 exponent range - no overflow), and PV streams
    the exp'd matrix as the f32r moving operand, producing attention
    output directly in T-domain [feature, token];
  - softmax denominators use an appended all-ones stationary column and
    an exact reciprocal + 0/1-matmul partition-broadcast;
  - all post-selection / output GEMMs (one-hot gathers, chain corrections,
    x@base_out, final Householder correction) run fp16 1-pass.
"""

import numpy as np

B, S, D, R = 8, 1024, 1024, 512
NPROC, TOPK = 64, 4
H, DH = 8, 64
EPS = 1e-8
NCORES = 8
TCN = 8   # token chunks of 128
KCN = 8   # D chunks of 128
RCN = 4   # rank chunks of 128

_BUILT = {}


def _apply_tile_drain_patch():
    """walrus here rejects >1 sync-wait on CTRL-class instructions; split
    Tile's kernel-tail drain waits into a chain of single-wait nops."""
    import concourse.mybir as mybir
    from concourse.tile import TileContext
    from concourse.vector_clock import ScopedClock

    if getattr(TileContext, "_drain_patched", False):
        return

    def _patched(self, tick_clock, wait_clock):
        probe = self.nc.sync.nop()
        wait_clock.add_sem_waits(
            probe.ins, ScopedClock({None: tick_clock.global_clock}))
        si = probe.ins.sync_info
        waits = list(si.on_wait) if si is not None else []
        updates = list(si.on_update) if si is not None else []
        if len(waits) > 1:
            probe.ins.sync_info = mybir.SyncInfo(
                on_update=updates, on_wait=waits[:1])
            for ofs in range(1, len(waits)):
                extra = self.nc.sync.nop()
                extra.ins.sync_info = mybir.SyncInfo(
                    on_update=[], on_wait=waits[ofs:ofs + 1])
        self.nc.sync.drain()
        self.nc.all_engine_barrier()
        assert self.sems is not None
        popped = self.nc._tile_sem_poison_stack.pop()
        assert popped is self._sem_poison
        self.nc.clear_and_free_semaphores(list(self.sems.allocated().values()))
        self.nc.all_engine_barrier()

    TileContext._drain_and_barrier = _patched
    TileContext._drain_patched = True


def _split_sync_waits(nc):
    """walrus here accepts at most 1 sync-wait per instruction; hoist
    extra waits onto same-engine NoOps inserted just before."""
    import concourse.mybir as mybir

    ctr = [0]
    for f in nc.m.functions:
        for bb in f.blocks:
            insts = bb.instructions
            out = []
            for inst in insts:
                si = inst.sync_info
                if si is not None and len(si.on_wait) > 1:
                    waits = list(si.on_wait)
                    for w in waits[:-1]:
                        ctr[0] += 1
                        nop = mybir.InstNoOp(
                            name=f"I-sw{ctr[0]}", ins=[], outs=[])
                        nop.engine = inst.engine
                        nop.sync_info = mybir.SyncInfo(
                            on_update=[], on_wait=[w])
                        out.append(nop)
                    inst.sync_info = mybir.SyncInfo(
                        on_update=list(si.on_update), on_wait=[waits[-1]])
                out.append(inst)
            bb.instructions = out


def _pack_kc(a, nchunk, chunk, dtype=np.float32):
    # [nchunk*chunk, N] -> [chunk, nchunk*N], chunk-major partitions
    n = a.shape[1]
    return np.ascontiguousarray(
        a.reshape(nchunk, chunk, n).transpose(1, 0, 2).reshape(chunk, nchunk * n)
    ).astype(dtype)


def _r12(a, mbits=12):
    """Round to the f32r grid (12 explicit mantissa bits, RNE) so the PE's
    1-pass f32r read is exact."""
    a = np.asarray(a, np.float64)
    m, e = np.frexp(a)
    return np.ldexp(np.round(m * 2.0**mbits) / 2.0**mbits, e).astype(np.float32)


def prep_weights(inputs):
    f = {k: np.asarray(v, np.float64) for k, v in inputs.items()}
    P = f["process_hh"]
    G = P @ P.T
    alpha2 = 2.0 / ((P * P).sum(1) + EPS)
    ihh, ohh = f["input_hh"], f["output_hh"]
    base_in, base_out = f["base_input"], f["base_output"]
    Bo = ohh @ base_out.T

    w = {}
    w["BI"] = _r12(_pack_kc(base_in, KCN, 128))
    W4 = np.concatenate([f["q_in_router"].T, f["k_in_router"].T,
                         f["v_in_router"].T, ihh.T], axis=1)
    w["W4"] = _r12(_pack_kc(W4, KCN, 128))
    for nm, wp in (("WDRQ", "q_proc_router"), ("WDRK", "k_proc_router"),
                   ("WDRV", "v_proc_router"), ("WDRO", "o_proc_router")):
        w[nm] = _pack_kc(np.concatenate([P.T, f[wp].T], axis=1), RCN, 128)
    w["WDRO2"] = _pack_kc(
        np.concatenate([f["o_out_router"].T, Bo.T], axis=1), RCN, 128,
        np.float16)
    w["NEGBH"] = _r12(-(ihh @ base_in))
    w["NEGP"] = (-P).astype(np.float16)
    w["GIN"] = (ihh @ ihh.T).astype(np.float16)
    w["GOUT"] = (ohh @ ohh.T).astype(np.float16)
    BD = np.zeros((256, 196))
    for i in range(4):
        if i < 3:
            BD[64 * i:64 * i + 64, 64 * i:64 * i + 64] = -G
        BD[64 * i:64 * i + 64, 192 + i] = alpha2
    w["BD"] = _pack_kc(BD, 2, 128, np.float16)
    w["BOUT"] = _pack_kc(base_out, RCN, 128, np.float16)
    w["NEGPOW"] = (-(P @ np.concatenate(
        [f["o_out_router"].T, Bo.T], axis=1))).astype(np.float16)[:, 0:128]
    w["NPBOHH"] = np.concatenate(
        [-(P @ base_out), -ohh], axis=0).astype(np.float16)
    return w


def build():
    import concourse.bass as bass
    import concourse.mybir as mybir
    from concourse.tile import TileContext
    from concourse.masks import make_identity

    _apply_tile_drain_patch()
    dt = mybir.dt
    op = mybir.AluOpType
    act = mybir.ActivationFunctionType
    f32r = dt.float32r
    f16 = dt.float16

    nc = bass.Bass()
    XTd = nc.dram_tensor("XT", (128, KCN * 1024), dt.float32, kind="ExternalInput")
    wd = {}
    for nm, shape, wdt in (
        ("BI", (128, KCN * 512), dt.float32), ("W4", (128, KCN * 256), dt.float32),
        ("WDRQ", (128, RCN * 128), dt.float32), ("WDRK", (128, RCN * 128), dt.float32),
        ("WDRV", (128, RCN * 128), dt.float32), ("WDRO", (128, RCN * 128), dt.float32),
        ("WDRO2", (128, RCN * 128), f16),
        ("NEGBH", (64, 512), dt.float32), ("NEGP", (64, 512), f16),
        ("GIN", (64, 64), f16), ("GOUT", (64, 64), f16),
        ("BD", (128, 2 * 196), f16), ("BOUT", (128, RCN * 1024), f16),
        ("NEGPOW", (64, 128), f16), ("NPBOHH", (128, 1024), f16),
    ):
        wd[nm] = nc.dram_tensor(nm, shape, wdt, kind="ExternalInput")
    OUTd = nc.dram_tensor("OUT", (1024, 1024), dt.float32, kind="ExternalOutput")

    with TileContext(nc) as tc:
        with (
            tc.tile_pool(name="w", bufs=1) as pw,
            tc.tile_pool(name="live", bufs=1) as pl,
        ):
            # small weights: issue on the Pool sequencer's DMA queue so the
            # SP queue serves BI/W4/xt first (PE's critical path at start)
            W = {}
            for nm, dram in wd.items():
                if nm in ("BOUT", "WDRO2", "NEGPOW", "NPBOHH", "BI", "W4"):
                    continue
                if nm == "NEGBH":
                    # consumed by an f32r matmul; host pre-rounds, DMA as f32r
                    t = pw.tile(list(dram.shape), f32r, tag=nm)
                    nc.gpsimd.dma_start(out=t[:], in_=dram[:].bitcast(f32r))
                else:
                    t = pw.tile(list(dram.shape), dram.dtype, tag=nm)
                    nc.gpsimd.dma_start(out=t[:], in_=dram[:])
                W[nm] = t
            ident = pw.tile([128, 128], dt.float32, tag="ident", name="ident")
            make_identity(nc, ident[:])
            ident16 = pw.tile([128, 128], f16, tag="ident16", name="ident16")
            nc.scalar.copy(ident16[:], ident[:])
            ident_r = pw.tile([128, 128], f32r, tag="identr", name="identr")
            nc.scalar.copy(ident_r[:], ident[:])
            ones32 = pw.tile([128, 8], dt.float32, tag="ones32", name="ones32")
            nc.vector.memset(ones32[:], 1.0)

            BD = W["BD"][:].rearrange("p (k n) -> p k n", k=2)
            WDR = {r: W["WDR" + r][:].rearrange("p (k n) -> p k n", k=RCN)
                   for r in ("Q", "K", "V", "O")}
            XTv = XTd[:].rearrange("p (k n) -> p k n", k=KCN)

            # persistent activations: attention output, T-domain
            AOT = [pl.tile([128, 1024], dt.float32, tag=f"aot{rc}", name=f"aot{rc}")
                   for rc in range(RCN)]
            AOT16 = [pl.tile([128, 1024], f16, tag=f"aot16{rc}", name=f"aot16{rc}")
                     for rc in range(RCN)]

            # ---------- shared helpers ----------

            def softmax_front(pool, ppt, ppv, name, s_all, f_all, gram,
                              cht_dtype=None, cht_ap=None):
                """Batched over all 8 token chunks. s_all/f_all are
                [128, 8, 64] APs. Returns CHT [64, 1024] (chat^T)."""
                E = pool.tile([128, TCN * 64], f16, tag="E_sh",
                              name=f"E_{name}", bufs=2)
                Ev = E[:].rearrange("p (t n) -> p t n", t=TCN)
                ET = pool.tile([64, 1024], f16, tag="ET_sh",
                               name=f"ET_{name}", bufs=1)
                CH = pool.tile([128, TCN * 64], f16, tag="CH_sh",
                               name=f"CH_{name}", bufs=2)
                CHv = CH[:].rearrange("p (t n) -> p t n", t=TCN)
                CHT = cht_ap if cht_ap is not None else pool.tile(
                    [64, 1024], cht_dtype or f32r,
                    tag="CHT_sh", name=f"CHT_{name}", bufs=1)
                SC = pool.tile([128, 8 * 8], dt.float32, tag=f"sc1_{name}",
                               name=f"sc1_{name}")
                SCv = SC[:].rearrange("p (t n) -> p t n", t=8)
                scr = pool.tile([128, 512], dt.float32, tag="scr_sh",
                                name=f"scr_{name}", bufs=1)

                nc.scalar.activation(Ev[:, :, :], s_all, act.Exp)
                Z8 = SCv[:, :, 0:1]
                nc.vector.tensor_reduce(Z8, Ev[:, :, :],
                                        mybir.AxisListType.X, op.add)
                # u = e @ Gin per chunk, packed into one PSUM bank
                pu = ppv.tile([128, 512], dt.float32, tag="ps_u", name="ps_u")
                for t in range(TCN):
                    pt = ppt.tile([128, 128], f16, tag="ps_t16", name="ps_t16")
                    nc.tensor.transpose(pt[0:64, :], Ev[:, t, :], ident16[:])
                    nc.scalar.copy(ET[:, 128 * t:128 * (t + 1)], pt[0:64, :])
                    nc.tensor.matmul(pu[:, 64 * t:64 * (t + 1)],
                                     ET[:, 128 * t:128 * (t + 1)], gram,
                                     start=True, stop=True)
                puv = pu[:].rearrange("p (t n) -> p t n", t=TCN)
                pacc, qacc = SCv[:, :, 1:2], SCv[:, :, 2:3]
                nc.vector.tensor_mul(scr[:], Ev[:, :, :], f_all)
                nc.vector.tensor_reduce(
                    pacc, scr[:].rearrange("p (t n) -> p t n", t=TCN),
                    mybir.AxisListType.X, op.add)
                nc.vector.tensor_mul(scr[:], Ev[:, :, :], puv)
                nc.vector.tensor_reduce(
                    qacc, scr[:].rearrange("p (t n) -> p t n", t=TCN),
                    mybir.AxisListType.X, op.add)
                z2, den = SCv[:, :, 3:4], SCv[:, :, 4:5]
                rec, gam = SCv[:, :, 5:6], SCv[:, :, 6:7]
                nc.vector.tensor_mul(z2, Z8, Z8)
                nc.vector.scalar_tensor_tensor(out=den, in0=z2, scalar=EPS,
                                               in1=qacc, op0=op.mult, op1=op.add)
                nc.vector.reciprocal(rec, den)
                nc.vector.scalar_tensor_tensor(out=gam, in0=pacc, scalar=2.0,
                                               in1=rec, op0=op.mult, op1=op.mult)
                nc.vector.tensor_mul(CHv[:, :, :], Ev[:, :, :],
                                     gam.to_broadcast((128, TCN, 64)))
                for t in range(TCN):
                    pt2 = ppt.tile([128, 128], f16, tag="ps_t16", name="ps_t16")
                    nc.tensor.transpose(pt2[0:64, :], CHv[:, t, :], ident16[:])
                    nc.scalar.copy(CHT[:, 128 * t:128 * (t + 1)], pt2[0:64, :])
                return CHT

            def hh_chain_a(pool, pps, ppt, name, wdr, src_tiles,
                           ct_ap=None, ts=0, te=TCN):
                """Stage a: D0/R GEMM (true fp32 - feeds top-k), top-4
                one-hots, d0 selects, -G row gather. Returns tile dict."""
                D0 = pool.tile([128, TCN * 64], f16, tag="D0_sh",
                               name=f"D0_{name}", bufs=2)
                D0v = D0[:].rearrange("p (t n) -> p t n", t=TCN)
                RS = pool.tile([128, TCN * 64], dt.float32, tag="RS_sh",
                               name=f"RS_{name}", bufs=2)
                RSv = RS[:].rearrange("p (t n) -> p t n", t=TCN)
                M8 = pool.tile([128, TCN * 8], dt.float32, tag="M8_sh",
                               name=f"M8_{name}", bufs=2)
                M8v = M8[:].rearrange("p (t n) -> p t n", t=TCN)
                OH = pool.tile([128, TCN * 256], f16, tag="OH_sh",
                               name=f"OH_{name}", bufs=2)
                OHv = OH[:].rearrange("p (t n) -> p t n", t=TCN)
                OHT = pool.tile([128, 256], f16, tag="OHT_sh",
                                name=f"OHT_{name}", bufs=2)
                BT = pool.tile([128, TCN * 196], f16, tag="BT_sh",
                               name=f"BT_{name}", bufs=2)
                BTv = BT[:].rearrange("p (t n) -> p t n", t=TCN)
                DSA = pool.tile([128, TCN * 24], dt.float32, tag="DS_sh2",
                                name=f"DS_{name}", bufs=2)
                DSAv = DSA[:].rearrange("p (t n) -> p t n", t=TCN)
                BE = pool.tile([128, TCN * 4], dt.float32, tag=f"BE_{name}",
                               name=f"BE_{name}")
                BEv = BE[:].rearrange("p (t n) -> p t n", t=TCN)
                CC = pool.tile([128, TCN * 64], f16, tag="CC_sh",
                               name=f"CC_{name}", bufs=2)
                CCv = CC[:].rearrange("p (t n) -> p t n", t=TCN)
                CT = ct_ap if ct_ap is not None else pool.tile(
                    [64, 1024], f16, tag="CT_sh", name=f"CT_{name}", bufs=1)
                scr = pool.tile([128, 512], f16, tag="scr16_sh",
                                name=f"scr16_{name}", bufs=2)
                scrv = scr[:].rearrange("p (t n) -> p t n", t=TCN)

                tn = te - ts
                # D0/R: pack 4 chunks per PSUM bank (true fp32, 4-pass)
                psd = [pps.tile([128, 512], dt.float32, tag="ps_sf",
                                name="ps_sf", bufs=2)
                       for _ in range((tn + 3) // 4)]
                for t in range(ts, te):
                    tl = t - ts
                    for rc in range(RCN):
                        nc.tensor.matmul(
                            psd[tl // 4][:, 128 * (tl % 4):128 * (tl % 4 + 1)],
                            src_tiles[rc][:, 128 * t:128 * (t + 1)],
                            wdr[:, rc, :],
                            start=(rc == 0), stop=(rc == RCN - 1))
                for half in range(tn // 4):
                    pv = psd[half][:].rearrange("p (t n) -> p t n", t=4)
                    nc.scalar.copy(D0v[:, 4 * half:4 * half + 4, :],
                                   pv[:, :, 0:64])
                    nc.scalar.copy(RSv[:, 4 * half:4 * half + 4, :],
                                   pv[:, :, 64:128])
                for tl in range(tn):
                    nc.vector.max(M8v[:, tl, :], RSv[:, tl, :])
                # one-hots + d0 selects (batched over chunks)
                for i in range(4):
                    nc.vector.tensor_tensor(
                        OHv[:, 0:tn, 64 * i:64 * (i + 1)], RSv[:, 0:tn, :],
                        M8v[:, 0:tn, i:i + 1].to_broadcast((128, tn, 64)),
                        op.is_equal)
                    nc.vector.tensor_mul(scr[:, 0:64 * tn],
                                         OHv[:, 0:tn, 64 * i:64 * (i + 1)],
                                         D0v[:, 0:tn, :])
                    nc.vector.tensor_reduce(DSAv[:, 0:tn, i:i + 1],
                                            scrv[:, 0:tn, :],
                                            mybir.AxisListType.X, op.add)
                # gather -G rows + 2alpha via transposed one-hot GEMM (fp16)
                for t in range(ts, te):
                    tl = t - ts
                    psb = pps.tile([128, 196], dt.float32, tag="ps_mm",
                                   name="ps_b", bufs=3)
                    for half in range(2):
                        pt = ppt.tile([128, 128], f16, tag="ps_t16",
                                      name="ps_t16")
                        nc.tensor.transpose(
                            pt[:], OHv[:, tl, 128 * half:128 * (half + 1)],
                            ident16[:])
                        nc.scalar.copy(OHT[:, 128 * half:128 * (half + 1)], pt[:])
                        nc.tensor.matmul(
                            psb[:], OHT[:, 128 * half:128 * (half + 1)],
                            BD[:, half, :], start=(half == 0), stop=(half == 1))
                    nc.scalar.copy(BTv[:, tl, :], psb[:])
                return dict(OHv=OHv, BTv=BTv, DSAv=DSAv, BEv=BEv, CCv=CCv,
                            CC=CC, CT=CT, scr=scr, scrv=scrv, ts=ts, te=te)

            def hh_chain_b(ppt, st):
                OHv, BTv, DSAv = st["OHv"], st["BTv"], st["DSAv"]
                BEv, CCv, CC = st["BEv"], st["CCv"], st["CC"]
                CT, scr, scrv = st["CT"], st["scr"], st["scrv"]
                ts, te = st["ts"], st["te"]
                tn = te - ts
                # pair values -G[idx_i, idx_j]
                pair = {}
                pidx = 4
                for i in range(1, 4):
                    for j in range(i):
                        nc.vector.tensor_mul(
                            scr[:, 0:64 * tn], OHv[:, 0:tn, 64 * i:64 * (i + 1)],
                            BTv[:, 0:tn, 64 * j:64 * (j + 1)])
                        nc.vector.tensor_reduce(
                            DSAv[:, 0:tn, pidx:pidx + 1], scrv[:, 0:tn, :],
                            mybir.AxisListType.X, op.add)
                        pair[(i, j)] = DSAv[:, 0:tn, pidx:pidx + 1]
                        pidx += 1
                # recursion (batched [128, 8] ops)
                be = [BEv[:, 0:tn, i:i + 1] for i in range(4)]
                a2 = [BTv[:, 0:tn, 192 + i:193 + i] for i in range(4)]
                nc.vector.tensor_mul(be[0], DSAv[:, 0:tn, 0:1], a2[0])
                tmp = 10
                for i in range(1, 4):
                    cur = DSAv[:, 0:tn, i:i + 1]
                    for j in range(i):
                        t1 = DSAv[:, 0:tn, tmp:tmp + 1]; tmp += 1
                        nc.vector.tensor_mul(t1, pair[(i, j)], be[j])
                        t2 = DSAv[:, 0:tn, tmp:tmp + 1]; tmp += 1
                        nc.vector.tensor_add(t2, t1, cur)
                        cur = t2
                    nc.vector.tensor_mul(be[i], cur, a2[i])
                # c = sum beta_i * onehot_i
                nc.vector.tensor_mul(CCv[:, 0:tn, :], OHv[:, 0:tn, 0:64],
                                     be[0].to_broadcast((128, tn, 64)))
                for i in range(1, 4):
                    nc.vector.tensor_mul(
                        scr[:, 0:64 * tn], OHv[:, 0:tn, 64 * i:64 * (i + 1)],
                        be[i].to_broadcast((128, tn, 64)))
                    nc.vector.tensor_add(CC[:, 0:64 * tn], CC[:, 0:64 * tn],
                                         scr[:, 0:64 * tn])
                for t in range(ts, te):
                    tl = t - ts
                    ptc = ppt.tile([128, 128], f16, tag="ps_t16", name="ps_t16")
                    nc.tensor.transpose(ptc[0:64, :], CCv[:, tl, :], ident16[:])
                    nc.scalar.copy(CT[:, 128 * t:128 * (t + 1)], ptc[0:64, :])
                return CT

            def hh_chain(pool, pps, ppt, name, wdr, src_tiles, ct_ap=None):
                st = hh_chain_a(pool, pps, ppt, name, wdr, src_tiles,
                                ct_ap=ct_ap)
                return hh_chain_b(ppt, st)

            # ================= phase A: SF + xbT (f32r 1-pass) =============
            _pbc_cm = tc.tile_pool(name="bc", bufs=1)
            pbc = _pbc_cm.__enter__()
            # post-chain fp16 activations for attention
            XA = {r: [pbc.tile([128, 1024], f16, tag=f"xa{r}{rc}",
                               name=f"xa{r}{rc}")
                      for rc in range(RCN)] for r in "qkv"}
            VP = [pbc.tile([128, 520], f32r, tag=f"vp{kc}", name=f"vp{kc}")
                  for kc in range(KCN)]
            with tc.tile_pool(name="phb", bufs=1) as pb:
                SFt = pb.tile([128, TCN * 256], f16, tag="sf",
                              name="sf", bufs=1)
                SFv = SFt[:].rearrange("p (t n) -> p t n", t=TCN)
                XB = [pb.tile([128, 1024], f32r, tag=f"xb{rc}",
                              name=f"xb{rc}")
                      for rc in range(RCN)]
                # xc buffers rotate across routes (2 in flight)
                XC = {r: [pb.tile([128, 1024], dt.float32, tag=f"xc{rc}",
                                  name=f"xc{r}{rc}", bufs=2)
                          for rc in range(RCN)] for r in "qkv"}
                with (
                    tc.tile_pool(name="pha", bufs=1) as pa_,
                    tc.tile_pool(name="phx", bufs=3) as px,
                    tc.tile_pool(name="psA", bufs=4, space="PSUM") as psA,
                ):
                    W4t = pa_.tile([128, KCN * 256], f32r, tag="W4w", name="W4w")
                    nc.sync.dma_start(out=W4t[:], in_=wd["W4"][:].bitcast(f32r))
                    BIt = pa_.tile([128, KCN * 512], f32r, tag="BIw", name="BIw")
                    nc.scalar.dma_start(out=BIt[:], in_=wd["BI"][:].bitcast(f32r))
                    BI = BIt[:].rearrange("p (k n) -> p k n", k=KCN)
                    W4 = W4t[:].rearrange("p (k n) -> p k n", k=KCN)
                    for sweep in range(2):
                        ps_sf = [psA.tile([128, 256], dt.float32, tag="ps_asf",
                                          name="ps_asf") for _ in range(4)]
                        ps_xb = [psA.tile([128, 512], dt.float32, tag="ps_axb",
                                          name="ps_axb") for _ in range(4)]
                        for kc in range(KCN):
                            xt = px.tile([128, 1024], f32r, tag="xtc",
                                         name="xtc")
                            eng = nc.sync if kc % 2 == 0 else nc.scalar
                            eng.dma_start(out=xt[:],
                                          in_=XTv[:, kc, :].bitcast(f32r))
                            for ti in range(4):
                                nc.tensor.matmul(
                                    ps_sf[ti][:],
                                    xt[:, 128 * ti + 512 * sweep:
                                       128 * (ti + 1) + 512 * sweep],
                                    W4[:, kc, :],
                                    start=(kc == 0), stop=(kc == KCN - 1))
                            for i in range(4):
                                rc, th = 2 * sweep + i // 2, i % 2
                                nc.tensor.matmul(
                                    ps_xb[i][:],
                                    BI[:, kc, 128 * rc:128 * (rc + 1)],
                                    xt[:, 512 * th:512 * (th + 1)],
                                    start=(kc == 0), stop=(kc == KCN - 1))
                        for ti in range(4):
                            t = 4 * sweep + ti
                            nc.scalar.copy(SFv[:, t, :], ps_sf[ti][:])
                        for i in range(4):
                            rc, th = 2 * sweep + i // 2, i % 2
                            nc.scalar.copy(
                                XB[rc][:, 512 * th:512 * (th + 1)], ps_xb[i][:])

                # ============ phase B: compress routes ============
                with (
                    tc.tile_pool(name="psB", bufs=1, space="PSUM") as pps,
                    tc.tile_pool(name="psBt", bufs=2, space="PSUM") as ppt,
                    tc.tile_pool(name="psBv", bufs=1, space="PSUM") as ppv,
                ):
                    def front_corr(ri, r):
                        """softmax_front + chat-correction for route r."""
                        CHT = softmax_front(
                            pb, ppt, ppv, r,
                            SFv[:, :, 64 * ri:64 * ri + 64],
                            SFv[:, :, 192:256], W["GIN"][:])
                        for rc in range(RCN):
                            for th in range(2):
                                ps = pps.tile([128, 512], dt.float32,
                                              tag="ps_mm", name="ps_mm", bufs=3)
                                nc.tensor.matmul(
                                    ps[:],
                                    W["NEGBH"][:][0:64, 128 * rc:128 * (rc + 1)],
                                    CHT[0:64, 512 * th:512 * (th + 1)],
                                    start=True, stop=False)
                                nc.tensor.matmul(
                                    ps[:], ident_r[:],
                                    XB[rc][:, 512 * th:512 * (th + 1)],
                                    start=False, stop=True)
                                if th == 0:
                                    nc.scalar.copy(
                                        XC[r][rc][:, 512 * th:512 * (th + 1)],
                                        ps[:])
                                else:
                                    nc.vector.tensor_copy(
                                        XC[r][rc][:, 512 * th:512 * (th + 1)],
                                        ps[:])

                    # software-pipelined routes: route r+1's front/correction
                    # is emitted between route r's chain_a and chain_b so every
                    # engine has fill work during r's serial recursion tail
                    front_corr(0, "q")
                    front_corr(1, "k")
                    sts = {}
                    sts[0] = hh_chain_a(pb, pps, ppt, "q", WDR["Q"], XC["q"])
                    for ri, r in enumerate("qkv"):
                        if ri + 1 < 3:
                            sts[ri + 1] = hh_chain_a(
                                pb, pps, ppt, "qkv"[ri + 1],
                                WDR["QKV"[ri + 1]], XC["qkv"[ri + 1]])
                        CT = hh_chain_b(ppt, sts.pop(ri))
                        # chain correction: fp16 1-pass GEMM + DVE add -> fp16
                        for rc in range(RCN):
                            for th in range(2):
                                ps = pps.tile([128, 512], dt.float32,
                                              tag="ps_mm", name="ps_mm", bufs=3)
                                nc.tensor.matmul(
                                    ps[:], W["NEGP"][:][
                                        0:64, 128 * rc:128 * (rc + 1)],
                                    CT[0:64, 512 * th:512 * (th + 1)],
                                    start=True, stop=True)
                                nc.vector.tensor_add(
                                    XA[r][rc][:, 512 * th:512 * (th + 1)],
                                    XC[r][rc][:, 512 * th:512 * (th + 1)],
                                    ps[:])
                        if ri + 2 < 3:
                            front_corr(ri + 2, "qkv"[ri + 2])
                    # V -> N-domain V' (f32r) with per-head 65-col blocks+ones
                    for rc in range(RCN):
                        for t in range(TCN):
                            pt = ppt.tile([128, 128], f16, tag="ps_t16",
                                          name="ps_t16")
                            nc.tensor.transpose(
                                pt[:], XA["v"][rc][:, 128 * t:128 * (t + 1)],
                                ident16[:])
                            dst = bass.AP(
                                VP[t].tensor, VP[t].offset + 65 * (2 * rc),
                                [VP[t].ap[0], [65, 2], [1, 64]])
                            nc.scalar.copy(
                                dst, pt[:].rearrange("p (h n) -> p h n", h=2))
                    for t in range(TCN):
                        ones = VP[t][:].rearrange(
                            "p (h n) -> p h n", h=H)[:, :, 64:65]
                        nc.scalar.copy(ones, ones32[:, 0:8].rearrange(
                            "p (h n) -> p h n", h=H))

            # ================= phase C: attention =================
            with tc.tile_pool(name="att", bufs=1) as pa:
                # per-(head, token) softmax denominators, broadcast across each
                # 64-row head block by a stride-0-partition DMA from PSUM
                ZINV = [pa.tile([128, 1024], dt.float32, tag=f"zinv{rc}",
                                name=f"zinv{rc}") for rc in range(RCN)]
                ZRI = pa.tile([1, 16 * 512], dt.float32, tag="zri", name="zri")

                def new_pt(hi):
                    t = pa.tile([128, KCN * 512], f32r, tag=f"pt{hi}",
                                name=f"pt{hi}", bufs=2)
                    return t[:].rearrange("p (k n) -> p k n", k=KCN)
                with (
                    tc.tile_pool(name="psC", bufs=3, space="PSUM") as pps,
                    tc.tile_pool(name="psCv", bufs=2, space="PSUM") as ppv,
                ):
                    # software-pipelined: scores/exp of iteration n+1 are
                    # emitted before PV of n so ACT (exp) and PE (PV) overlap
                    iters = [(hp, qh, hi) for qh in range(2)
                             for hp in range(4) for hi in range(2)]
                    ptvs = {}

                    def scores_exp(n):
                        hp, qh, hi = iters[n]
                        hr = 64 * hi
                        ptv = ptvs[n] = new_pt(hi)
                        for g in range(4):
                            ps2 = pps.tile([128, 1024], dt.float32,
                                           tag="ps2", name="ps2", bufs=3)
                            for j in range(2):
                                kc = 2 * g + j
                                nc.tensor.matmul(
                                    ps2[:, 512 * j:512 * (j + 1)],
                                    XA["k"][hp][hr:hr + 64,
                                                128 * kc:128 * (kc + 1)],
                                    XA["q"][hp][hr:hr + 64,
                                                512 * qh:512 * (qh + 1)],
                                    start=True, stop=True)
                            nc.scalar.activation(
                                ptv[:, 2 * g:2 * g + 2, :], ps2[:],
                                act.Exp, scale=0.125)

                    def pv_stage(n):
                        hp, qh, hi = iters[n]
                        h, hr = 2 * hp + hi, 64 * hi
                        ptv = ptvs.pop(n)
                        pv65 = ppv.tile([128, 512], dt.float32,
                                        tag="ps_pv", name="ps_pv")
                        for kc in range(KCN):
                            nc.tensor.matmul(
                                pv65[0:65, :],
                                VP[kc][:, 65 * h:65 * h + 65],
                                ptv[:, kc, :],
                                start=(kc == 0), stop=(kc == KCN - 1))
                        nc.vector.tensor_copy(
                            AOT[hp][hr:hr + 64, 512 * qh:512 * (qh + 1)],
                            pv65[0:64, :])
                        zofs = 512 * (2 * h + qh)
                        nc.vector.reciprocal(
                            ZRI[0:1, zofs:zofs + 512], pv65[64:65, :])
                        zsrc = bass.AP(
                            ZRI.tensor, ZRI.offset + zofs,
                            [ZRI.ap[0], [0, 64], [1, 512]])
                        nc.sync.dma_start(
                            out=ZINV[hp][64 * hi:64 * (hi + 1),
                                         512 * qh:512 * (qh + 1)],
                            in_=zsrc)

                    scores_exp(0)
                    for n in range(16):
                        if n + 1 < 16:
                            scores_exp(n + 1)
                        pv_stage(n)
                        if n % 2 == 1:
                            th, rc = n // 8, (n % 8) // 2
                            nc.gpsimd.tensor_mul(
                                AOT[rc][:, 512 * th:512 * (th + 1)],
                                AOT[rc][:, 512 * th:512 * (th + 1)],
                                ZINV[rc][:, 512 * th:512 * (th + 1)])
                            nc.gpsimd.tensor_copy(
                                AOT16[rc][:, 512 * th:512 * (th + 1)],
                                AOT[rc][:, 512 * th:512 * (th + 1)])

            _pbc_cm.__exit__(None, None, None)
            # ================= phase D: expand =================
            with tc.tile_pool(name="exp", bufs=1) as pe:
                BOUTt = pe.tile([128, RCN * 1024], f16, tag="boutw", name="boutw")
                nc.sync.dma_start(out=BOUTt[:], in_=wd["BOUT"][:])
                BOUT = BOUTt[:].rearrange("p (k n) -> p k n", k=RCN)
                WDRO2t = pe.tile([128, RCN * 128], f16, tag="wdro2w",
                                 name="wdro2w")
                nc.sync.dma_start(out=WDRO2t[:], in_=wd["WDRO2"][:])
                WDRO2 = WDRO2t[:].rearrange("p (k n) -> p k n", k=RCN)
                NEGPOWt = pe.tile([64, 128], f16, tag="negpoww", name="negpoww")
                nc.sync.dma_start(out=NEGPOWt[:], in_=wd["NEGPOW"][:])
                NPBOHHt = pe.tile([128, 1024], f16, tag="npbohhw",
                                  name="npbohhw")
                nc.sync.dma_start(out=NPBOHHt[:], in_=wd["NPBOHH"][:])
                STK = pe.tile([128, 1024], f16, tag="stk", name="stk")
                SO = pe.tile([128, TCN * 128], dt.float32, tag="so", name="so")
                SOv = SO[:].rearrange("p (t n) -> p t n", t=TCN)
                with (
                    tc.tile_pool(name="psD", bufs=1, space="PSUM") as pps,
                    tc.tile_pool(name="psDt", bufs=2, space="PSUM") as ppt,
                    tc.tile_pool(name="psDv", bufs=1, space="PSUM") as ppv,
                ):
                    st1 = hh_chain_a(pe, pps, ppt, "o", WDR["O"], AOT,
                                     ct_ap=STK[0:64, :], ts=0, te=4)
                    st2 = hh_chain_a(pe, pps, ppt, "o2", WDR["O"], AOT,
                                     ct_ap=STK[0:64, :], ts=4, te=8)
                    hh_chain_b(ppt, st1)
                    CTo = hh_chain_b(ppt, st2)
                    for t in range(TCN):
                        ps = pps.tile([128, 128], dt.float32, tag="ps_sf",
                                      name="ps_sf", bufs=2)
                        for rc in range(RCN):
                            nc.tensor.matmul(
                                ps[:], AOT16[rc][:, 128 * t:128 * (t + 1)],
                                WDRO2[:, rc, :],
                                start=(rc == 0), stop=False)
                        nc.tensor.matmul(
                            ps[:], CTo[0:64, 128 * t:128 * (t + 1)],
                            NEGPOWt[0:64, :], start=False, stop=True)
                        nc.scalar.copy(SOv[:, t, :], ps[:, 0:128])
                    softmax_front(
                        pe, ppt, ppv, "o",
                        SOv[:, :, 0:64], SOv[:, :, 64:128], W["GOUT"][:],
                        cht_dtype=f16, cht_ap=STK[64:128, :])
                    # ao @ base_out accumulated with the Householder/chat
                    # correction in one PSUM group; the BOUT matmuls are ready
                    # early and back-fill PE stalls before STK lands
                    for dc in range(KCN):
                        for th in range(2):
                            ps = pps.tile([128, 512], dt.float32, tag="ps_mm",
                                          name="ps_mm", bufs=3)
                            for rc in range(RCN):
                                nc.tensor.matmul(
                                    ps[:], BOUT[:, rc, 128 * dc:128 * (dc + 1)],
                                    AOT16[rc][:, 512 * th:512 * (th + 1)],
                                    start=(rc == 0), stop=False)
                            nc.tensor.matmul(
                                ps[:], NPBOHHt[:][:, 128 * dc:128 * (dc + 1)],
                                STK[:, 512 * th:512 * (th + 1)],
                                start=False, stop=True)
                            ot = pe.tile([128, 512], dt.float32, tag="outsb",
                                         name="outsb", bufs=3)
                            if th == 0:
                                nc.scalar.copy(ot[:], ps[:])
                            else:
                                nc.vector.tensor_copy(ot[:], ps[:])
                            nc.sync.dma_start(
                                out=OUTd[128 * dc:128 * (dc + 1),
                                         512 * th:512 * (th + 1)],
                                in_=ot[:])
    _split_sync_waits(nc)
    return nc


def get_built():
    if "nc" not in _BUILT:
        _BUILT["nc"] = build()
    return _BUILT["nc"]


def _in_maps(inputs):
    x = np.asarray(inputs["x"], np.float32)
    w = prep_weights(inputs)
    in_maps = []
    for c in range(NCORES):
        m = dict(w)
        m["XT"] = _r12(_pack_kc(np.ascontiguousarray(x[c].T), KCN, 128))
        in_maps.append(m)
    return in_maps


def kernel(**inputs):
    from concourse.bass_utils import run_bass_kernel_spmd

    nc = get_built()
    res = run_bass_kernel_spmd(nc, _in_maps(inputs),
                               core_ids=list(range(NCORES)))
    out = np.stack([res.results[c]["OUT"].T for c in range(NCORES)], axis=0)
    return out.astype(np.float32)


def run_timed(inputs, trace=False):
    from concourse.bass_utils import run_bass_kernel_spmd
    nc = get_built()
    return run_bass_kernel_spmd(nc, _in_maps(inputs),
                                core_ids=list(range(NCORES)), trace=trace)


# revision 45
# speedup vs baseline: 1.7704x; 1.0139x over previous
"""Trainium2 Bass kernel for nn_NeuronAttention (moe_routing).

Sharding: data-parallel over batch B=8 across 8 NeuronCores (one batch row
per core); weights replicated; no collectives.

Per-core computation uses two layouts: "T-domain" [feature, token] for PE
GEMMs (contraction on partitions) and "N-domain" [token, small-free] for
routing math (softmax, top-k, Householder-chain recursion) on DVE/ACT.
The Householder chains are evaluated in 64-dim dot-space: with
d0 = xc@P.T, G = P@P.T, a = 1/(||P_k||^2+EPS), the 4 selected reflections
reduce to the scalar recursion beta_i = 2a_i(d0_i - sum_{j<i} beta_j G_ij)
and a rank-64 correction xc - (sum_i beta_i e_{idx_i})@P.

Precision plan (hw-measured: f32r keeps ~13 mantissa bits, fp16 11, and a
full-m12 emulation of this pipeline gives 5e-3 rel err vs the 2e-2 gate):
  - proc-router score GEMMs (feed top-k) stay true fp32 4-pass, reading
    fp32 xc tiles, so selections match the reference almost everywhere;
  - phase-A GEMMs (x@[routers|hh], x@base_in) run 1-pass f32r on host-
    pre-rounded operands;
  - the in-softmax chat correction runs f32r (chat rounded at ~2^-13);
  - attention QK runs fp16 on post-chain fp16 copies, exp'd scores are
    stored f32r (full fp32 exponent range - no overflow), and PV streams
    the exp'd matrix as the f32r moving operand, producing attention
    output directly in T-domain [feature, token];
  - softmax denominators use an appended all-ones stationary column and
    an exact reciprocal + a stride-0 free-dim DMA broadcast across each
    64-row head block;
  - all post-selection / output GEMMs (one-hot gathers, chain corrections,
    x@base_out, final Householder correction) run fp16 1-pass.
"""

import numpy as np

B, S, D, R = 8, 1024, 1024, 512
NPROC, TOPK = 64, 4
H, DH = 8, 64
EPS = 1e-8
NCORES = 8
TCN = 8   # token chunks of 128
KCN = 8   # D chunks of 128
RCN = 4   # rank chunks of 128

_BUILT = {}


def _apply_tile_drain_patch():
    """walrus here rejects >1 sync-wait on CTRL-class instructions; split
    Tile's kernel-tail drain waits into a chain of single-wait nops."""
    import concourse.mybir as mybir
    from concourse.tile import TileContext
    from concourse.vector_clock import ScopedClock

    if getattr(TileContext, "_drain_patched", False):
        return

    def _patched(self, tick_clock, wait_clock):
        probe = self.nc.sync.nop()
        wait_clock.add_sem_waits(
            probe.ins, ScopedClock({None: tick_clock.global_clock}))
        si = probe.ins.sync_info
        waits = list(si.on_wait) if si is not None else []
        updates = list(si.on_update) if si is not None else []
        if len(waits) > 1:
            probe.ins.sync_info = mybir.SyncInfo(
                on_update=updates, on_wait=waits[:1])
            for ofs in range(1, len(waits)):
                extra = self.nc.sync.nop()
                extra.ins.sync_info = mybir.SyncInfo(
                    on_update=[], on_wait=waits[ofs:ofs + 1])
        self.nc.sync.drain()
        self.nc.all_engine_barrier()
        assert self.sems is not None
        popped = self.nc._tile_sem_poison_stack.pop()
        assert popped is self._sem_poison
        self.nc.clear_and_free_semaphores(list(self.sems.allocated().values()))
        self.nc.all_engine_barrier()

    TileContext._drain_and_barrier = _patched
    TileContext._drain_patched = True


def _split_sync_waits(nc):
    """walrus here accepts at most 1 sync-wait per instruction; hoist
    extra waits onto same-engine NoOps inserted just before."""
    import concourse.mybir as mybir

    ctr = [0]
    for f in nc.m.functions:
        for bb in f.blocks:
            insts = bb.instructions
            out = []
            for inst in insts:
                si = inst.sync_info
                if si is not None and len(si.on_wait) > 1:
                    waits = list(si.on_wait)
                    for w in waits[:-1]:
                        ctr[0] += 1
                        nop = mybir.InstNoOp(
                            name=f"I-sw{ctr[0]}", ins=[], outs=[])
                        nop.engine = inst.engine
                        nop.sync_info = mybir.SyncInfo(
                            on_update=[], on_wait=[w])
                        out.append(nop)
                    inst.sync_info = mybir.SyncInfo(
                        on_update=list(si.on_update), on_wait=[waits[-1]])
                out.append(inst)
            bb.instructions = out


def _pack_kc(a, nchunk, chunk, dtype=np.float32):
    # [nchunk*chunk, N] -> [chunk, nchunk*N], chunk-major partitions
    n = a.shape[1]
    return np.ascontiguousarray(
        a.reshape(nchunk, chunk, n).transpose(1, 0, 2).reshape(chunk, nchunk * n)
    ).astype(dtype)


def _r12(a, mbits=12):
    """Round to the f32r grid (12 explicit mantissa bits, RNE) so the PE's
    1-pass f32r read is exact."""
    a = np.asarray(a, np.float64)
    m, e = np.frexp(a)
    return np.ldexp(np.round(m * 2.0**mbits) / 2.0**mbits, e).astype(np.float32)


def prep_weights(inputs):
    f = {k: np.asarray(v, np.float64) for k, v in inputs.items()}
    P = f["process_hh"]
    G = P @ P.T
    alpha2 = 2.0 / ((P * P).sum(1) + EPS)
    ihh, ohh = f["input_hh"], f["output_hh"]
    base_in, base_out = f["base_input"], f["base_output"]
    Bo = ohh @ base_out.T

    w = {}
    w["BI"] = _r12(_pack_kc(base_in, KCN, 128))
    W4 = np.concatenate([f["q_in_router"].T, f["k_in_router"].T,
                         f["v_in_router"].T, ihh.T], axis=1)
    w["W4"] = _r12(_pack_kc(W4, KCN, 128))
    for nm, wp in (("WDRQ", "q_proc_router"), ("WDRK", "k_proc_router"),
                   ("WDRV", "v_proc_router"), ("WDRO", "o_proc_router")):
        w[nm] = _pack_kc(np.concatenate([P.T, f[wp].T], axis=1), RCN, 128)
    w["WDRO2"] = _pack_kc(
        np.concatenate([f["o_out_router"].T, Bo.T], axis=1), RCN, 128,
        np.float16)
    w["NEGBH"] = _r12(-(ihh @ base_in))
    w["NEGP"] = (-P).astype(np.float16)
    w["GIN"] = (ihh @ ihh.T).astype(np.float16)
    w["GOUT"] = (ohh @ ohh.T).astype(np.float16)
    BD = np.zeros((256, 196))
    for i in range(4):
        if i < 3:
            BD[64 * i:64 * i + 64, 64 * i:64 * i + 64] = -G
        BD[64 * i:64 * i + 64, 192 + i] = alpha2
    w["BD"] = _pack_kc(BD, 2, 128, np.float16)
    w["BOUT"] = _pack_kc(base_out, RCN, 128, np.float16)
    w["NEGPOW"] = (-(P @ np.concatenate(
        [f["o_out_router"].T, Bo.T], axis=1))).astype(np.float16)[:, 0:128]
    w["NPBOHH"] = np.concatenate(
        [-(P @ base_out), -ohh], axis=0).astype(np.float16)
    return w


def build():
    import concourse.bass as bass
    import concourse.mybir as mybir
    from concourse.tile import TileContext
    from concourse.masks import make_identity

    _apply_tile_drain_patch()
    dt = mybir.dt
    op = mybir.AluOpType
    act = mybir.ActivationFunctionType
    f32r = dt.float32r
    f16 = dt.float16

    nc = bass.Bass()
    XTd = nc.dram_tensor("XT", (128, KCN * 1024), dt.float32, kind="ExternalInput")
    wd = {}
    for nm, shape, wdt in (
        ("BI", (128, KCN * 512), dt.float32), ("W4", (128, KCN * 256), dt.float32),
        ("WDRQ", (128, RCN * 128), dt.float32), ("WDRK", (128, RCN * 128), dt.float32),
        ("WDRV", (128, RCN * 128), dt.float32), ("WDRO", (128, RCN * 128), dt.float32),
        ("WDRO2", (128, RCN * 128), f16),
        ("NEGBH", (64, 512), dt.float32), ("NEGP", (64, 512), f16),
        ("GIN", (64, 64), f16), ("GOUT", (64, 64), f16),
        ("BD", (128, 2 * 196), f16), ("BOUT", (128, RCN * 1024), f16),
        ("NEGPOW", (64, 128), f16), ("NPBOHH", (128, 1024), f16),
    ):
        wd[nm] = nc.dram_tensor(nm, shape, wdt, kind="ExternalInput")
    OUTd = nc.dram_tensor("OUT", (1024, 1024), dt.float32, kind="ExternalOutput")

    with TileContext(nc) as tc:
        with (
            tc.tile_pool(name="w", bufs=1) as pw,
            tc.tile_pool(name="live", bufs=1) as pl,
        ):
            # small weights: issue on the Pool sequencer's DMA queue so the
            # SP queue serves BI/W4/xt first (PE's critical path at start)
            W = {}
            for nm, dram in wd.items():
                if nm in ("BOUT", "WDRO2", "NEGPOW", "NPBOHH", "BI", "W4"):
                    continue
                if nm == "NEGBH":
                    # consumed by an f32r matmul; host pre-rounds, DMA as f32r
                    t = pw.tile(list(dram.shape), f32r, tag=nm)
                    nc.gpsimd.dma_start(out=t[:], in_=dram[:].bitcast(f32r))
                else:
                    t = pw.tile(list(dram.shape), dram.dtype, tag=nm)
                    nc.gpsimd.dma_start(out=t[:], in_=dram[:])
                W[nm] = t
            ident = pw.tile([128, 128], dt.float32, tag="ident", name="ident")
            make_identity(nc, ident[:])
            ident16 = pw.tile([128, 128], f16, tag="ident16", name="ident16")
            nc.scalar.copy(ident16[:], ident[:])
            ident_r = pw.tile([128, 128], f32r, tag="identr", name="identr")
            nc.scalar.copy(ident_r[:], ident[:])
            ones32 = pw.tile([128, 8], dt.float32, tag="ones32", name="ones32")
            nc.vector.memset(ones32[:], 1.0)

            BD = W["BD"][:].rearrange("p (k n) -> p k n", k=2)
            WDR = {r: W["WDR" + r][:].rearrange("p (k n) -> p k n", k=RCN)
                   for r in ("Q", "K", "V", "O")}
            XTv = XTd[:].rearrange("p (k n) -> p k n", k=KCN)

            # persistent activations: attention output, T-domain
            AOT = [pl.tile([128, 1024], dt.float32, tag=f"aot{rc}", name=f"aot{rc}")
                   for rc in range(RCN)]
            AOT16 = [pl.tile([128, 1024], f16, tag=f"aot16{rc}", name=f"aot16{rc}")
                     for rc in range(RCN)]

            # ---------- shared helpers ----------

            def softmax_front(pool, ppt, ppv, name, s_all, f_all, gram,
                              cht_dtype=None, cht_ap=None):
                """Batched over all 8 token chunks. s_all/f_all are
                [128, 8, 64] APs. Returns CHT [64, 1024] (chat^T)."""
                E = pool.tile([128, TCN * 64], f16, tag="E_sh",
                              name=f"E_{name}", bufs=2)
                Ev = E[:].rearrange("p (t n) -> p t n", t=TCN)
                ET = pool.tile([64, 1024], f16, tag="ET_sh",
                               name=f"ET_{name}", bufs=1)
                CH = pool.tile([128, TCN * 64], f16, tag="CH_sh",
                               name=f"CH_{name}", bufs=2)
                CHv = CH[:].rearrange("p (t n) -> p t n", t=TCN)
                CHT = cht_ap if cht_ap is not None else pool.tile(
                    [64, 1024], cht_dtype or f32r,
                    tag="CHT_sh", name=f"CHT_{name}", bufs=1)
                SC = pool.tile([128, 8 * 8], dt.float32, tag=f"sc1_{name}",
                               name=f"sc1_{name}")
                SCv = SC[:].rearrange("p (t n) -> p t n", t=8)
                scr = pool.tile([128, 512], dt.float32, tag="scr_sh",
                                name=f"scr_{name}", bufs=1)

                nc.scalar.activation(Ev[:, :, :], s_all, act.Exp)
                Z8 = SCv[:, :, 0:1]
                nc.vector.tensor_reduce(Z8, Ev[:, :, :],
                                        mybir.AxisListType.X, op.add)
                # u = e @ Gin per chunk, packed into one PSUM bank
                pu = ppv.tile([128, 512], dt.float32, tag="ps_u", name="ps_u")
                for t in range(TCN):
                    pt = ppt.tile([128, 128], f16, tag="ps_t16", name="ps_t16")
                    nc.tensor.transpose(pt[0:64, :], Ev[:, t, :], ident16[:])
                    nc.scalar.copy(ET[:, 128 * t:128 * (t + 1)], pt[0:64, :])
                    nc.tensor.matmul(pu[:, 64 * t:64 * (t + 1)],
                                     ET[:, 128 * t:128 * (t + 1)], gram,
                                     start=True, stop=True)
                puv = pu[:].rearrange("p (t n) -> p t n", t=TCN)
                pacc, qacc = SCv[:, :, 1:2], SCv[:, :, 2:3]
                nc.vector.tensor_mul(scr[:], Ev[:, :, :], f_all)
                nc.vector.tensor_reduce(
                    pacc, scr[:].rearrange("p (t n) -> p t n", t=TCN),
                    mybir.AxisListType.X, op.add)
                nc.vector.tensor_mul(scr[:], Ev[:, :, :], puv)
                nc.vector.tensor_reduce(
                    qacc, scr[:].rearrange("p (t n) -> p t n", t=TCN),
                    mybir.AxisListType.X, op.add)
                z2, den = SCv[:, :, 3:4], SCv[:, :, 4:5]
                rec, gam = SCv[:, :, 5:6], SCv[:, :, 6:7]
                nc.vector.tensor_mul(z2, Z8, Z8)
                nc.vector.scalar_tensor_tensor(out=den, in0=z2, scalar=EPS,
                                               in1=qacc, op0=op.mult, op1=op.add)
                nc.vector.reciprocal(rec, den)
                nc.vector.scalar_tensor_tensor(out=gam, in0=pacc, scalar=2.0,
                                               in1=rec, op0=op.mult, op1=op.mult)
                nc.vector.tensor_mul(CHv[:, :, :], Ev[:, :, :],
                                     gam.to_broadcast((128, TCN, 64)))
                for t in range(TCN):
                    pt2 = ppt.tile([128, 128], f16, tag="ps_t16", name="ps_t16")
                    nc.tensor.transpose(pt2[0:64, :], CHv[:, t, :], ident16[:])
                    nc.scalar.copy(CHT[:, 128 * t:128 * (t + 1)], pt2[0:64, :])
                return CHT

            def hh_chain_a(pool, pps, ppt, name, wdr, src_tiles,
                           ct_ap=None, ts=0, te=TCN):
                """Stage a: D0/R GEMM (true fp32 - feeds top-k), top-4
                one-hots, d0 selects, -G row gather. Returns tile dict."""
                D0 = pool.tile([128, TCN * 64], f16, tag="D0_sh",
                               name=f"D0_{name}", bufs=2)
                D0v = D0[:].rearrange("p (t n) -> p t n", t=TCN)
                RS = pool.tile([128, TCN * 64], dt.float32, tag="RS_sh",
                               name=f"RS_{name}", bufs=2)
                RSv = RS[:].rearrange("p (t n) -> p t n", t=TCN)
                M8 = pool.tile([128, TCN * 8], dt.float32, tag="M8_sh",
                               name=f"M8_{name}", bufs=2)
                M8v = M8[:].rearrange("p (t n) -> p t n", t=TCN)
                OH = pool.tile([128, TCN * 256], f16, tag="OH_sh",
                               name=f"OH_{name}", bufs=2)
                OHv = OH[:].rearrange("p (t n) -> p t n", t=TCN)
                OHT = pool.tile([128, 256], f16, tag="OHT_sh",
                                name=f"OHT_{name}", bufs=2)
                BT = pool.tile([128, TCN * 196], f16, tag="BT_sh",
                               name=f"BT_{name}", bufs=2)
                BTv = BT[:].rearrange("p (t n) -> p t n", t=TCN)
                DSA = pool.tile([128, TCN * 24], dt.float32, tag="DS_sh2",
                                name=f"DS_{name}", bufs=2)
                DSAv = DSA[:].rearrange("p (t n) -> p t n", t=TCN)
                BE = pool.tile([128, TCN * 4], dt.float32, tag=f"BE_{name}",
                               name=f"BE_{name}")
                BEv = BE[:].rearrange("p (t n) -> p t n", t=TCN)
                CC = pool.tile([128, TCN * 64], f16, tag="CC_sh",
                               name=f"CC_{name}", bufs=2)
                CCv = CC[:].rearrange("p (t n) -> p t n", t=TCN)
                CT = ct_ap if ct_ap is not None else pool.tile(
                    [64, 1024], f16, tag="CT_sh", name=f"CT_{name}", bufs=1)
                scr = pool.tile([128, 512], f16, tag="scr16_sh",
                                name=f"scr16_{name}", bufs=2)
                scrv = scr[:].rearrange("p (t n) -> p t n", t=TCN)

                tn = te - ts
                # D0/R: pack 4 chunks per PSUM bank (true fp32, 4-pass)
                psd = [pps.tile([128, 512], dt.float32, tag="ps_sf",
                                name="ps_sf", bufs=2)
                       for _ in range((tn + 3) // 4)]
                for t in range(ts, te):
                    tl = t - ts
                    for rc in range(RCN):
                        nc.tensor.matmul(
                            psd[tl // 4][:, 128 * (tl % 4):128 * (tl % 4 + 1)],
                            src_tiles[rc][:, 128 * t:128 * (t + 1)],
                            wdr[:, rc, :],
                            start=(rc == 0), stop=(rc == RCN - 1))
                for half in range(tn // 4):
                    pv = psd[half][:].rearrange("p (t n) -> p t n", t=4)
                    nc.scalar.copy(D0v[:, 4 * half:4 * half + 4, :],
                                   pv[:, :, 0:64])
                    nc.scalar.copy(RSv[:, 4 * half:4 * half + 4, :],
                                   pv[:, :, 64:128])
                for tl in range(tn):
                    nc.vector.max(M8v[:, tl, :], RSv[:, tl, :])
                # one-hots + d0 selects (batched over chunks)
                for i in range(4):
                    nc.vector.tensor_tensor(
                        OHv[:, 0:tn, 64 * i:64 * (i + 1)], RSv[:, 0:tn, :],
                        M8v[:, 0:tn, i:i + 1].to_broadcast((128, tn, 64)),
                        op.is_equal)
                    nc.vector.tensor_mul(scr[:, 0:64 * tn],
                                         OHv[:, 0:tn, 64 * i:64 * (i + 1)],
                                         D0v[:, 0:tn, :])
                    nc.vector.tensor_reduce(DSAv[:, 0:tn, i:i + 1],
                                            scrv[:, 0:tn, :],
                                            mybir.AxisListType.X, op.add)
                # gather -G rows + 2alpha via transposed one-hot GEMM (fp16)
                for t in range(ts, te):
                    tl = t - ts
                    psb = pps.tile([128, 196], dt.float32, tag="ps_mm",
                                   name="ps_b", bufs=3)
                    for half in range(2):
                        pt = ppt.tile([128, 128], f16, tag="ps_t16",
                                      name="ps_t16")
                        nc.tensor.transpose(
                            pt[:], OHv[:, tl, 128 * half:128 * (half + 1)],
                            ident16[:])
                        nc.scalar.copy(OHT[:, 128 * half:128 * (half + 1)], pt[:])
                        nc.tensor.matmul(
                            psb[:], OHT[:, 128 * half:128 * (half + 1)],
                            BD[:, half, :], start=(half == 0), stop=(half == 1))
                    nc.scalar.copy(BTv[:, tl, :], psb[:])
                return dict(OHv=OHv, BTv=BTv, DSAv=DSAv, BEv=BEv, CCv=CCv,
                            CC=CC, CT=CT, scr=scr, scrv=scrv, ts=ts, te=te)

            def hh_chain_b(ppt, st):
                OHv, BTv, DSAv = st["OHv"], st["BTv"], st["DSAv"]
                BEv, CCv, CC = st["BEv"], st["CCv"], st["CC"]
                CT, scr, scrv = st["CT"], st["scr"], st["scrv"]
                ts, te = st["ts"], st["te"]
                tn = te - ts
                # pair values -G[idx_i, idx_j]
                pair = {}
                pidx = 4
                for i in range(1, 4):
                    for j in range(i):
                        nc.vector.tensor_mul(
                            scr[:, 0:64 * tn], OHv[:, 0:tn, 64 * i:64 * (i + 1)],
                            BTv[:, 0:tn, 64 * j:64 * (j + 1)])
                        nc.vector.tensor_reduce(
                            DSAv[:, 0:tn, pidx:pidx + 1], scrv[:, 0:tn, :],
                            mybir.AxisListType.X, op.add)
                        pair[(i, j)] = DSAv[:, 0:tn, pidx:pidx + 1]
                        pidx += 1
                # recursion (batched [128, 8] ops)
                be = [BEv[:, 0:tn, i:i + 1] for i in range(4)]
                a2 = [BTv[:, 0:tn, 192 + i:193 + i] for i in range(4)]
                nc.vector.tensor_mul(be[0], DSAv[:, 0:tn, 0:1], a2[0])
                tmp = 10
                for i in range(1, 4):
                    cur = DSAv[:, 0:tn, i:i + 1]
                    for j in range(i):
                        t1 = DSAv[:, 0:tn, tmp:tmp + 1]; tmp += 1
                        nc.vector.tensor_mul(t1, pair[(i, j)], be[j])
                        t2 = DSAv[:, 0:tn, tmp:tmp + 1]; tmp += 1
                        nc.vector.tensor_add(t2, t1, cur)
                        cur = t2
                    nc.vector.tensor_mul(be[i], cur, a2[i])
                # c = sum beta_i * onehot_i
                nc.vector.tensor_mul(CCv[:, 0:tn, :], OHv[:, 0:tn, 0:64],
                                     be[0].to_broadcast((128, tn, 64)))
                for i in range(1, 4):
                    nc.vector.tensor_mul(
                        scr[:, 0:64 * tn], OHv[:, 0:tn, 64 * i:64 * (i + 1)],
                        be[i].to_broadcast((128, tn, 64)))
                    nc.vector.tensor_add(CC[:, 0:64 * tn], CC[:, 0:64 * tn],
                                         scr[:, 0:64 * tn])
                for t in range(ts, te):
                    tl = t - ts
                    ptc = ppt.tile([128, 128], f16, tag="ps_t16", name="ps_t16")
                    nc.tensor.transpose(ptc[0:64, :], CCv[:, tl, :], ident16[:])
                    nc.scalar.copy(CT[:, 128 * t:128 * (t + 1)], ptc[0:64, :])
                return CT

            def hh_chain(pool, pps, ppt, name, wdr, src_tiles, ct_ap=None):
                st = hh_chain_a(pool, pps, ppt, name, wdr, src_tiles,
                                ct_ap=ct_ap)
                return hh_chain_b(ppt, st)

            # ================= phase A: SF + xbT (f32r 1-pass) =============
            _pbc_cm = tc.tile_pool(name="bc", bufs=1)
            pbc = _pbc_cm.__enter__()
            # post-chain fp16 activations for attention
            XA = {r: [pbc.tile([128, 1024], f16, tag=f"xa{r}{rc}",
                               name=f"xa{r}{rc}")
                      for rc in range(RCN)] for r in "qkv"}
            VP = [pbc.tile([128, 520], f32r, tag=f"vp{kc}", name=f"vp{kc}")
                  for kc in range(KCN)]
            with tc.tile_pool(name="phb", bufs=1) as pb:
                SFt = pb.tile([128, TCN * 256], f16, tag="sf",
                              name="sf", bufs=1)
                SFv = SFt[:].rearrange("p (t n) -> p t n", t=TCN)
                XB = [pb.tile([128, 1024], f32r, tag=f"xb{rc}",
                              name=f"xb{rc}")
                      for rc in range(RCN)]
                # xc buffers rotate across routes (2 in flight)
                XC = {r: [pb.tile([128, 1024], dt.float32, tag=f"xc{rc}",
                                  name=f"xc{r}{rc}", bufs=2)
                          for rc in range(RCN)] for r in "qkv"}
                with (
                    tc.tile_pool(name="pha", bufs=1) as pa_,
                    tc.tile_pool(name="phx", bufs=3) as px,
                    tc.tile_pool(name="psA", bufs=4, space="PSUM") as psA,
                ):
                    W4t = pa_.tile([128, KCN * 256], f32r, tag="W4w", name="W4w")
                    nc.sync.dma_start(out=W4t[:, 0:1024],
                                      in_=wd["W4"][:, 0:1024].bitcast(f32r))
                    nc.scalar.dma_start(out=W4t[:, 1024:2048],
                                        in_=wd["W4"][:, 1024:2048].bitcast(f32r))
                    BIt = pa_.tile([128, KCN * 512], f32r, tag="BIw", name="BIw")
                    BI = BIt[:].rearrange("p (k n) -> p k n", k=KCN)
                    W4 = W4t[:].rearrange("p (k n) -> p k n", k=KCN)
                    for sweep in range(2):
                        ps_sf = [psA.tile([128, 256], dt.float32, tag="ps_asf",
                                          name="ps_asf") for _ in range(4)]
                        ps_xb = [psA.tile([128, 512], dt.float32, tag="ps_axb",
                                          name="ps_axb") for _ in range(4)]
                        for kc in range(KCN):
                            xt = px.tile([128, 1024], f32r, tag="xtc",
                                         name="xtc")
                            eng = nc.sync if kc % 2 == 0 else nc.scalar
                            eng.dma_start(out=xt[:],
                                          in_=XTv[:, kc, :].bitcast(f32r))
                            if sweep == 0 and kc in (0, 2):
                                half = kc // 2
                                nc.scalar.dma_start(
                                    out=BIt[:, 2048 * half:2048 * (half + 1)],
                                    in_=wd["BI"][:, 2048 * half:
                                                 2048 * (half + 1)]
                                    .bitcast(f32r))
                            for ti in range(4):
                                nc.tensor.matmul(
                                    ps_sf[ti][:],
                                    xt[:, 128 * ti + 512 * sweep:
                                       128 * (ti + 1) + 512 * sweep],
                                    W4[:, kc, :],
                                    start=(kc == 0), stop=(kc == KCN - 1))
                            for i in range(4):
                                rc, th = 2 * sweep + i // 2, i % 2
                                nc.tensor.matmul(
                                    ps_xb[i][:],
                                    BI[:, kc, 128 * rc:128 * (rc + 1)],
                                    xt[:, 512 * th:512 * (th + 1)],
                                    start=(kc == 0), stop=(kc == KCN - 1))
                        for ti in range(4):
                            t = 4 * sweep + ti
                            nc.scalar.copy(SFv[:, t, :], ps_sf[ti][:])
                        for i in range(4):
                            rc, th = 2 * sweep + i // 2, i % 2
                            nc.scalar.copy(
                                XB[rc][:, 512 * th:512 * (th + 1)], ps_xb[i][:])

                # ============ phase B: compress routes ============
                with (
                    tc.tile_pool(name="psB", bufs=1, space="PSUM") as pps,
                    tc.tile_pool(name="psBt", bufs=2, space="PSUM") as ppt,
                    tc.tile_pool(name="psBv", bufs=1, space="PSUM") as ppv,
                ):
                    def front_corr(ri, r):
                        """softmax_front + chat-correction for route r."""
                        CHT = softmax_front(
                            pb, ppt, ppv, r,
                            SFv[:, :, 64 * ri:64 * ri + 64],
                            SFv[:, :, 192:256], W["GIN"][:])
                        for rc in range(RCN):
                            for th in range(2):
                                ps = pps.tile([128, 512], dt.float32,
                                              tag="ps_mm", name="ps_mm", bufs=3)
                                nc.tensor.matmul(
                                    ps[:],
                                    W["NEGBH"][:][0:64, 128 * rc:128 * (rc + 1)],
                                    CHT[0:64, 512 * th:512 * (th + 1)],
                                    start=True, stop=False)
                                nc.tensor.matmul(
                                    ps[:], ident_r[:],
                                    XB[rc][:, 512 * th:512 * (th + 1)],
                                    start=False, stop=True)
                                if th == 0:
                                    nc.scalar.copy(
                                        XC[r][rc][:, 512 * th:512 * (th + 1)],
                                        ps[:])
                                else:
                                    nc.vector.tensor_copy(
                                        XC[r][rc][:, 512 * th:512 * (th + 1)],
                                        ps[:])

                    # software-pipelined routes: route r+1's front/correction
                    # is emitted between route r's chain_a and chain_b so every
                    # engine has fill work during r's serial recursion tail
                    ROUTES = [("v", 2), ("q", 0), ("k", 1)]
                    front_corr(ROUTES[0][1], ROUTES[0][0])
                    front_corr(ROUTES[1][1], ROUTES[1][0])
                    sts = {}
                    sts[0] = hh_chain_a(pb, pps, ppt, ROUTES[0][0],
                                        WDR[ROUTES[0][0].upper()],
                                        XC[ROUTES[0][0]])
                    for ri, (r, sfi) in enumerate(ROUTES):
                        if ri + 1 < 3:
                            nr = ROUTES[ri + 1][0]
                            sts[ri + 1] = hh_chain_a(
                                pb, pps, ppt, nr, WDR[nr.upper()], XC[nr])
                        CT = hh_chain_b(ppt, sts.pop(ri))
                        # chain correction: fp16 1-pass GEMM + DVE add -> fp16
                        for rc in range(RCN):
                            for th in range(2):
                                ps = pps.tile([128, 512], dt.float32,
                                              tag="ps_mm", name="ps_mm", bufs=3)
                                nc.tensor.matmul(
                                    ps[:], W["NEGP"][:][
                                        0:64, 128 * rc:128 * (rc + 1)],
                                    CT[0:64, 512 * th:512 * (th + 1)],
                                    start=True, stop=True)
                                nc.vector.tensor_add(
                                    XA[r][rc][:, 512 * th:512 * (th + 1)],
                                    XC[r][rc][:, 512 * th:512 * (th + 1)],
                                    ps[:])
                        if ri + 2 < 3:
                            front_corr(ROUTES[ri + 2][1], ROUTES[ri + 2][0])
                        if r == "v":
                            # V ready first: build N-domain V' now so the
                            # attention entry does not stall on route tails
                            for rc in range(RCN):
                                for t in range(TCN):
                                    pt = ppt.tile([128, 128], f16,
                                                  tag="ps_t16", name="ps_t16")
                                    nc.tensor.transpose(
                                        pt[:],
                                        XA["v"][rc][:, 128 * t:128 * (t + 1)],
                                        ident16[:])
                                    dst = bass.AP(
                                        VP[t].tensor,
                                        VP[t].offset + 65 * (2 * rc),
                                        [VP[t].ap[0], [65, 2], [1, 64]])
                                    nc.scalar.copy(
                                        dst,
                                        pt[:].rearrange("p (h n) -> p h n",
                                                        h=2))
                    # ones columns for the softmax denominators
                    for t in range(TCN):
                        ones = VP[t][:].rearrange(
                            "p (h n) -> p h n", h=H)[:, :, 64:65]
                        nc.scalar.copy(ones, ones32[:, 0:8].rearrange(
                            "p (h n) -> p h n", h=H))

            # ================= phase C: attention =================
            with tc.tile_pool(name="att", bufs=1) as pa:
                # per-(head, token) softmax denominators, broadcast across each
                # 64-row head block by a stride-0-partition DMA from PSUM
                ZINV = [pa.tile([128, 1024], dt.float32, tag=f"zinv{rc}",
                                name=f"zinv{rc}") for rc in range(RCN)]
                ZRI = pa.tile([1, 16 * 512], dt.float32, tag="zri", name="zri")

                def new_pt(hi):
                    t = pa.tile([128, KCN * 512], f32r, tag=f"pt{hi}",
                                name=f"pt{hi}", bufs=2)
                    return t[:].rearrange("p (k n) -> p k n", k=KCN)
                with (
                    tc.tile_pool(name="psC", bufs=3, space="PSUM") as pps,
                    tc.tile_pool(name="psCv", bufs=2, space="PSUM") as ppv,
                ):
                    # software-pipelined: scores/exp of iteration n+1 are
                    # emitted before PV of n so ACT (exp) and PE (PV) overlap
                    iters = [(hp, qh, hi) for qh in range(2)
                             for hp in range(4) for hi in range(2)]
                    ptvs = {}

                    def scores_exp(n):
                        hp, qh, hi = iters[n]
                        hr = 64 * hi
                        ptv = ptvs[n] = new_pt(hi)
                        for g in range(4):
                            ps2 = pps.tile([128, 1024], dt.float32,
                                           tag="ps2", name="ps2", bufs=3)
                            for j in range(2):
                                kc = 2 * g + j
                                nc.tensor.matmul(
                                    ps2[:, 512 * j:512 * (j + 1)],
                                    XA["k"][hp][hr:hr + 64,
                                                128 * kc:128 * (kc + 1)],
                                    XA["q"][hp][hr:hr + 64,
                                                512 * qh:512 * (qh + 1)],
                                    start=True, stop=True)
                            nc.scalar.activation(
                                ptv[:, 2 * g:2 * g + 2, :], ps2[:],
                                act.Exp, scale=0.125)

                    def pv_stage(n):
                        hp, qh, hi = iters[n]
                        h, hr = 2 * hp + hi, 64 * hi
                        ptv = ptvs.pop(n)
                        pv65 = ppv.tile([128, 512], dt.float32,
                                        tag="ps_pv", name="ps_pv")
                        for kc in range(KCN):
                            nc.tensor.matmul(
                                pv65[0:65, :],
                                VP[kc][:, 65 * h:65 * h + 65],
                                ptv[:, kc, :],
                                start=(kc == 0), stop=(kc == KCN - 1))
                        nc.vector.tensor_copy(
                            AOT[hp][hr:hr + 64, 512 * qh:512 * (qh + 1)],
                            pv65[0:64, :])
                        zofs = 512 * (2 * h + qh)
                        nc.vector.reciprocal(
                            ZRI[0:1, zofs:zofs + 512], pv65[64:65, :])
                        zsrc = bass.AP(
                            ZRI.tensor, ZRI.offset + zofs,
                            [ZRI.ap[0], [0, 64], [1, 512]])
                        nc.sync.dma_start(
                            out=ZINV[hp][64 * hi:64 * (hi + 1),
                                         512 * qh:512 * (qh + 1)],
                            in_=zsrc)

                    scores_exp(0)
                    for n in range(16):
                        if n + 1 < 16:
                            scores_exp(n + 1)
                        pv_stage(n)
                        if n % 2 == 1:
                            th, rc = n // 8, (n % 8) // 2
                            nc.gpsimd.tensor_mul(
                                AOT[rc][:, 512 * th:512 * (th + 1)],
                                AOT[rc][:, 512 * th:512 * (th + 1)],
                                ZINV[rc][:, 512 * th:512 * (th + 1)])
                            nc.gpsimd.tensor_copy(
                                AOT16[rc][:, 512 * th:512 * (th + 1)],
                                AOT[rc][:, 512 * th:512 * (th + 1)])

            _pbc_cm.__exit__(None, None, None)
            # ================= phase D: expand =================
            with tc.tile_pool(name="exp", bufs=1) as pe:
                BOUTt = pe.tile([128, RCN * 1024], f16, tag="boutw", name="boutw")
                nc.sync.dma_start(out=BOUTt[:], in_=wd["BOUT"][:])
                BOUT = BOUTt[:].rearrange("p (k n) -> p k n", k=RCN)
                WDRO2t = pe.tile([128, RCN * 128], f16, tag="wdro2w",
                                 name="wdro2w")
                nc.sync.dma_start(out=WDRO2t[:], in_=wd["WDRO2"][:])
                WDRO2 = WDRO2t[:].rearrange("p (k n) -> p k n", k=RCN)
                NEGPOWt = pe.tile([64, 128], f16, tag="negpoww", name="negpoww")
                nc.sync.dma_start(out=NEGPOWt[:], in_=wd["NEGPOW"][:])
                NPBOHHt = pe.tile([128, 1024], f16, tag="npbohhw",
                                  name="npbohhw")
                nc.sync.dma_start(out=NPBOHHt[:], in_=wd["NPBOHH"][:])
                STK = pe.tile([128, 1024], f16, tag="stk", name="stk")
                SO = pe.tile([128, TCN * 128], dt.float32, tag="so", name="so")
                SOv = SO[:].rearrange("p (t n) -> p t n", t=TCN)
                with (
                    tc.tile_pool(name="psD", bufs=1, space="PSUM") as pps,
                    tc.tile_pool(name="psDt", bufs=2, space="PSUM") as ppt,
                    tc.tile_pool(name="psDv", bufs=1, space="PSUM") as ppv,
                ):
                    st1 = hh_chain_a(pe, pps, ppt, "o", WDR["O"], AOT,
                                     ct_ap=STK[0:64, :], ts=0, te=4)
                    st2 = hh_chain_a(pe, pps, ppt, "o2", WDR["O"], AOT,
                                     ct_ap=STK[0:64, :], ts=4, te=8)
                    hh_chain_b(ppt, st1)
                    CTo = hh_chain_b(ppt, st2)
                    for t in range(TCN):
                        ps = pps.tile([128, 128], dt.float32, tag="ps_sf",
                                      name="ps_sf", bufs=2)
                        for rc in range(RCN):
                            nc.tensor.matmul(
                                ps[:], AOT16[rc][:, 128 * t:128 * (t + 1)],
                                WDRO2[:, rc, :],
                                start=(rc == 0), stop=False)
                        nc.tensor.matmul(
                            ps[:], CTo[0:64, 128 * t:128 * (t + 1)],
                            NEGPOWt[0:64, :], start=False, stop=True)
                        nc.scalar.copy(SOv[:, t, :], ps[:, 0:128])
                    softmax_front(
                        pe, ppt, ppv, "o",
                        SOv[:, :, 0:64], SOv[:, :, 64:128], W["GOUT"][:],
                        cht_dtype=f16, cht_ap=STK[64:128, :])
                    # ao @ base_out accumulated with the Householder/chat
                    # correction in one PSUM group; the BOUT matmuls are ready
                    # early and back-fill PE stalls before STK lands
                    for dc in range(KCN):
                        for th in range(2):
                            ps = pps.tile([128, 512], dt.float32, tag="ps_mm",
                                          name="ps_mm", bufs=3)
                            for rc in range(RCN):
                                nc.tensor.matmul(
                                    ps[:], BOUT[:, rc, 128 * dc:128 * (dc + 1)],
                                    AOT16[rc][:, 512 * th:512 * (th + 1)],
                                    start=(rc == 0), stop=False)
                            nc.tensor.matmul(
                                ps[:], NPBOHHt[:][:, 128 * dc:128 * (dc + 1)],
                                STK[:, 512 * th:512 * (th + 1)],
                                start=False, stop=True)
                            ot = pe.tile([128, 512], dt.float32, tag="outsb",
                                         name="outsb", bufs=3)
                            if th == 0:
                                nc.scalar.copy(ot[:], ps[:])
                            else:
                                nc.vector.tensor_copy(ot[:], ps[:])
                            oeng = nc.sync if th == 0 else nc.gpsimd
                            oeng.dma_start(
                                out=OUTd[128 * dc:128 * (dc + 1),
                                         512 * th:512 * (th + 1)],
                                in_=ot[:])
    _split_sync_waits(nc)
    return nc


def get_built():
    if "nc" not in _BUILT:
        _BUILT["nc"] = build()
    return _BUILT["nc"]


def _in_maps(inputs):
    x = np.asarray(inputs["x"], np.float32)
    w = prep_weights(inputs)
    in_maps = []
    for c in range(NCORES):
        m = dict(w)
        m["XT"] = _r12(_pack_kc(np.ascontiguousarray(x[c].T), KCN, 128))
        in_maps.append(m)
    return in_maps


def kernel(**inputs):
    from concourse.bass_utils import run_bass_kernel_spmd

    nc = get_built()
    res = run_bass_kernel_spmd(nc, _in_maps(inputs),
                               core_ids=list(range(NCORES)))
    out = np.stack([res.results[c]["OUT"].T for c in range(NCORES)], axis=0)
    return out.astype(np.float32)


def run_timed(inputs, trace=False):
    from concourse.bass_utils import run_bass_kernel_spmd
    nc = get_built()
    return run_bass_kernel_spmd(nc, _in_maps(inputs),
                                core_ids=list(range(NCORES)), trace=trace)


# revision 48
# speedup vs baseline: 1.8017x; 1.0177x over previous
"""Trainium2 Bass kernel for nn_NeuronAttention (moe_routing).

Sharding: data-parallel over batch B=8 across 8 NeuronCores (one batch row
per core); weights replicated; no collectives.

Per-core computation uses two layouts: "T-domain" [feature, token] for PE
GEMMs (contraction on partitions) and "N-domain" [token, small-free] for
routing math (softmax, top-k, Householder-chain recursion) on DVE/ACT.
The Householder chains are evaluated in 64-dim dot-space: with
d0 = xc@P.T, G = P@P.T, a = 1/(||P_k||^2+EPS), the 4 selected reflections
reduce to the scalar recursion beta_i = 2a_i(d0_i - sum_{j<i} beta_j G_ij)
and a rank-64 correction xc - (sum_i beta_i e_{idx_i})@P.

Precision plan (hw-measured: f32r keeps ~13 mantissa bits, fp16 11, and a
full-m12 emulation of this pipeline gives 5e-3 rel err vs the 2e-2 gate):
  - proc-router score GEMMs (feed top-k) stay true fp32 4-pass, reading
    fp32 xc tiles, so selections match the reference almost everywhere;
  - phase-A GEMMs (x@[routers|hh], x@base_in) run 1-pass f32r on host-
    pre-rounded operands;
  - the in-softmax chat correction runs f32r (chat rounded at ~2^-13);
  - attention QK runs fp16 on post-chain fp16 copies, exp'd scores are
    stored f32r (full fp32 exponent range - no overflow), and PV streams
    the exp'd matrix as the f32r moving operand, producing attention
    output directly in T-domain [feature, token];
  - softmax denominators use an appended all-ones stationary column and
    an exact reciprocal + a stride-0 free-dim DMA broadcast across each
    64-row head block;
  - all post-selection / output GEMMs (one-hot gathers, chain corrections,
    x@base_out, final Householder correction) run fp16 1-pass.
"""

import numpy as np

B, S, D, R = 8, 1024, 1024, 512
NPROC, TOPK = 64, 4
H, DH = 8, 64
EPS = 1e-8
NCORES = 8
TCN = 8   # token chunks of 128
KCN = 8   # D chunks of 128
RCN = 4   # rank chunks of 128

_BUILT = {}


def _apply_tile_drain_patch():
    """walrus here rejects >1 sync-wait on CTRL-class instructions; split
    Tile's kernel-tail drain waits into a chain of single-wait nops."""
    import concourse.mybir as mybir
    from concourse.tile import TileContext
    from concourse.vector_clock import ScopedClock

    if getattr(TileContext, "_drain_patched", False):
        return

    def _patched(self, tick_clock, wait_clock):
        probe = self.nc.sync.nop()
        wait_clock.add_sem_waits(
            probe.ins, ScopedClock({None: tick_clock.global_clock}))
        si = probe.ins.sync_info
        waits = list(si.on_wait) if si is not None else []
        updates = list(si.on_update) if si is not None else []
        if len(waits) > 1:
            probe.ins.sync_info = mybir.SyncInfo(
                on_update=updates, on_wait=waits[:1])
            for ofs in range(1, len(waits)):
                extra = self.nc.sync.nop()
                extra.ins.sync_info = mybir.SyncInfo(
                    on_update=[], on_wait=waits[ofs:ofs + 1])
        self.nc.sync.drain()
        self.nc.all_engine_barrier()
        assert self.sems is not None
        popped = self.nc._tile_sem_poison_stack.pop()
        assert popped is self._sem_poison
        self.nc.clear_and_free_semaphores(list(self.sems.allocated().values()))
        self.nc.all_engine_barrier()

    TileContext._drain_and_barrier = _patched
    TileContext._drain_patched = True


def _split_sync_waits(nc):
    """walrus here accepts at most 1 sync-wait per instruction; hoist
    extra waits onto same-engine NoOps inserted just before."""
    import concourse.mybir as mybir

    ctr = [0]
    for f in nc.m.functions:
        for bb in f.blocks:
            insts = bb.instructions
            out = []
            for inst in insts:
                si = inst.sync_info
                if si is not None and len(si.on_wait) > 1:
                    waits = list(si.on_wait)
                    for w in waits[:-1]:
                        ctr[0] += 1
                        nop = mybir.InstNoOp(
                            name=f"I-sw{ctr[0]}", ins=[], outs=[])
                        nop.engine = inst.engine
                        nop.sync_info = mybir.SyncInfo(
                            on_update=[], on_wait=[w])
                        out.append(nop)
                    inst.sync_info = mybir.SyncInfo(
                        on_update=list(si.on_update), on_wait=[waits[-1]])
                out.append(inst)
            bb.instructions = out


def _pack_kc(a, nchunk, chunk, dtype=np.float32):
    # [nchunk*chunk, N] -> [chunk, nchunk*N], chunk-major partitions
    n = a.shape[1]
    return np.ascontiguousarray(
        a.reshape(nchunk, chunk, n).transpose(1, 0, 2).reshape(chunk, nchunk * n)
    ).astype(dtype)


def _r12(a, mbits=12):
    """Round to the f32r grid (12 explicit mantissa bits, RNE) so the PE's
    1-pass f32r read is exact."""
    a = np.asarray(a, np.float64)
    m, e = np.frexp(a)
    return np.ldexp(np.round(m * 2.0**mbits) / 2.0**mbits, e).astype(np.float32)


def prep_weights(inputs):
    f = {k: np.asarray(v, np.float64) for k, v in inputs.items()}
    P = f["process_hh"]
    G = P @ P.T
    alpha2 = 2.0 / ((P * P).sum(1) + EPS)
    ihh, ohh = f["input_hh"], f["output_hh"]
    base_in, base_out = f["base_input"], f["base_output"]
    Bo = ohh @ base_out.T

    w = {}
    w["BI"] = _r12(_pack_kc(base_in, KCN, 128))
    W4 = np.concatenate([f["q_in_router"].T, f["k_in_router"].T,
                         f["v_in_router"].T, ihh.T], axis=1)
    w["W4"] = _r12(_pack_kc(W4, KCN, 128))
    for nm, wp in (("WDRQ", "q_proc_router"), ("WDRK", "k_proc_router"),
                   ("WDRV", "v_proc_router"), ("WDRO", "o_proc_router")):
        w[nm] = _pack_kc(np.concatenate([P.T, f[wp].T], axis=1), RCN, 128)
    w["WDRO2"] = _pack_kc(
        np.concatenate([f["o_out_router"].T, Bo.T], axis=1), RCN, 128,
        np.float16)
    w["NEGBH"] = _r12(-(ihh @ base_in))
    w["NEGP"] = (-P).astype(np.float16)
    w["GIN"] = (ihh @ ihh.T).astype(np.float16)
    w["GOUT"] = (ohh @ ohh.T).astype(np.float16)
    BD = np.zeros((256, 196))
    for i in range(4):
        if i < 3:
            BD[64 * i:64 * i + 64, 64 * i:64 * i + 64] = -G
        BD[64 * i:64 * i + 64, 192 + i] = alpha2
    w["BD"] = _pack_kc(BD, 2, 128, np.float16)
    w["BOUT"] = _pack_kc(base_out, RCN, 128, np.float16)
    w["NEGPOW"] = (-(P @ np.concatenate(
        [f["o_out_router"].T, Bo.T], axis=1))).astype(np.float16)[:, 0:128]
    w["NPBOHH"] = np.concatenate(
        [-(P @ base_out), -ohh], axis=0).astype(np.float16)
    return w


def build():
    import concourse.bass as bass
    import concourse.mybir as mybir
    from concourse.tile import TileContext
    from concourse.masks import make_identity

    _apply_tile_drain_patch()
    dt = mybir.dt
    op = mybir.AluOpType
    act = mybir.ActivationFunctionType
    f32r = dt.float32r
    f16 = dt.float16

    nc = bass.Bass()
    XTd = nc.dram_tensor("XT", (128, KCN * 1024), dt.float32, kind="ExternalInput")
    wd = {}
    for nm, shape, wdt in (
        ("BI", (128, KCN * 512), dt.float32), ("W4", (128, KCN * 256), dt.float32),
        ("WDRQ", (128, RCN * 128), dt.float32), ("WDRK", (128, RCN * 128), dt.float32),
        ("WDRV", (128, RCN * 128), dt.float32), ("WDRO", (128, RCN * 128), dt.float32),
        ("WDRO2", (128, RCN * 128), f16),
        ("NEGBH", (64, 512), dt.float32), ("NEGP", (64, 512), f16),
        ("GIN", (64, 64), f16), ("GOUT", (64, 64), f16),
        ("BD", (128, 2 * 196), f16), ("BOUT", (128, RCN * 1024), f16),
        ("NEGPOW", (64, 128), f16), ("NPBOHH", (128, 1024), f16),
    ):
        wd[nm] = nc.dram_tensor(nm, shape, wdt, kind="ExternalInput")
    OUTd = nc.dram_tensor("OUT", (1024, 1024), dt.float32, kind="ExternalOutput")

    with TileContext(nc) as tc:
        with (
            tc.tile_pool(name="w", bufs=1) as pw,
            tc.tile_pool(name="live", bufs=1) as pl,
        ):
            # small weights: issue on the Pool sequencer's DMA queue so the
            # SP queue serves BI/W4/xt first (PE's critical path at start)
            W = {}
            for nm, dram in wd.items():
                if nm in ("BOUT", "WDRO2", "NEGPOW", "NPBOHH", "BI", "W4"):
                    continue
                if nm == "NEGBH":
                    # consumed by an f32r matmul; host pre-rounds, DMA as f32r
                    t = pw.tile(list(dram.shape), f32r, tag=nm)
                    nc.gpsimd.dma_start(out=t[:], in_=dram[:].bitcast(f32r))
                else:
                    t = pw.tile(list(dram.shape), dram.dtype, tag=nm)
                    nc.gpsimd.dma_start(out=t[:], in_=dram[:])
                W[nm] = t
            ident = pw.tile([128, 128], dt.float32, tag="ident", name="ident")
            make_identity(nc, ident[:])
            ident16 = pw.tile([128, 128], f16, tag="ident16", name="ident16")
            nc.scalar.copy(ident16[:], ident[:])
            ident_r = pw.tile([128, 128], f32r, tag="identr", name="identr")
            nc.scalar.copy(ident_r[:], ident[:])
            ones32 = pw.tile([128, 8], dt.float32, tag="ones32", name="ones32")
            nc.vector.memset(ones32[:], 1.0)

            BD = W["BD"][:].rearrange("p (k n) -> p k n", k=2)
            WDR = {r: W["WDR" + r][:].rearrange("p (k n) -> p k n", k=RCN)
                   for r in ("Q", "K", "V", "O")}
            XTv = XTd[:].rearrange("p (k n) -> p k n", k=KCN)

            # persistent activations: attention output, T-domain
            AOT = [pl.tile([128, 1024], dt.float32, tag=f"aot{rc}", name=f"aot{rc}")
                   for rc in range(RCN)]
            AOT16 = [pl.tile([128, 1024], f16, tag=f"aot16{rc}", name=f"aot16{rc}")
                     for rc in range(RCN)]

            # ---------- shared helpers ----------

            def softmax_front(pool, ppt, ppv, name, s_all, f_all, gram,
                              cht_dtype=None, cht_ap=None):
                """Batched over all 8 token chunks. s_all/f_all are
                [128, 8, 64] APs. Returns CHT [64, 1024] (chat^T)."""
                E = pool.tile([128, TCN * 64], f16, tag="E_sh",
                              name=f"E_{name}", bufs=1)
                Ev = E[:].rearrange("p (t n) -> p t n", t=TCN)
                ET = pool.tile([64, 1024], f16, tag="ET_sh",
                               name=f"ET_{name}", bufs=1)
                CH = pool.tile([128, TCN * 64], f16, tag="CH_sh",
                               name=f"CH_{name}", bufs=1)
                CHv = CH[:].rearrange("p (t n) -> p t n", t=TCN)
                CHT = cht_ap if cht_ap is not None else pool.tile(
                    [64, 1024], cht_dtype or f32r,
                    tag="CHT_sh", name=f"CHT_{name}", bufs=1)
                SC = pool.tile([128, 8 * 8], dt.float32, tag=f"sc1_{name}",
                               name=f"sc1_{name}")
                SCv = SC[:].rearrange("p (t n) -> p t n", t=8)
                scr = pool.tile([128, 512], dt.float32, tag="scr_sh",
                                name=f"scr_{name}", bufs=1)

                nc.scalar.activation(Ev[:, :, :], s_all, act.Exp)
                Z8 = SCv[:, :, 0:1]
                nc.vector.tensor_reduce(Z8, Ev[:, :, :],
                                        mybir.AxisListType.X, op.add)
                # u = e @ Gin per chunk, packed into one PSUM bank
                pu = ppv.tile([128, 512], dt.float32, tag="ps_u", name="ps_u")
                for t in range(TCN):
                    pt = ppt.tile([128, 128], f16, tag="ps_t16", name="ps_t16")
                    nc.tensor.transpose(pt[0:64, :], Ev[:, t, :], ident16[:])
                    nc.scalar.copy(ET[:, 128 * t:128 * (t + 1)], pt[0:64, :])
                    nc.tensor.matmul(pu[:, 64 * t:64 * (t + 1)],
                                     ET[:, 128 * t:128 * (t + 1)], gram,
                                     start=True, stop=True)
                puv = pu[:].rearrange("p (t n) -> p t n", t=TCN)
                pacc, qacc = SCv[:, :, 1:2], SCv[:, :, 2:3]
                nc.vector.tensor_mul(scr[:], Ev[:, :, :], f_all)
                nc.vector.tensor_reduce(
                    pacc, scr[:].rearrange("p (t n) -> p t n", t=TCN),
                    mybir.AxisListType.X, op.add)
                nc.vector.tensor_mul(scr[:], Ev[:, :, :], puv)
                nc.vector.tensor_reduce(
                    qacc, scr[:].rearrange("p (t n) -> p t n", t=TCN),
                    mybir.AxisListType.X, op.add)
                z2, den = SCv[:, :, 3:4], SCv[:, :, 4:5]
                rec, gam = SCv[:, :, 5:6], SCv[:, :, 6:7]
                nc.vector.tensor_mul(z2, Z8, Z8)
                nc.vector.scalar_tensor_tensor(out=den, in0=z2, scalar=EPS,
                                               in1=qacc, op0=op.mult, op1=op.add)
                nc.vector.reciprocal(rec, den)
                nc.vector.scalar_tensor_tensor(out=gam, in0=pacc, scalar=2.0,
                                               in1=rec, op0=op.mult, op1=op.mult)
                nc.vector.tensor_mul(CHv[:, :, :], Ev[:, :, :],
                                     gam.to_broadcast((128, TCN, 64)))
                for t in range(TCN):
                    pt2 = ppt.tile([128, 128], f16, tag="ps_t16", name="ps_t16")
                    nc.tensor.transpose(pt2[0:64, :], CHv[:, t, :], ident16[:])
                    nc.scalar.copy(CHT[:, 128 * t:128 * (t + 1)], pt2[0:64, :])
                return CHT

            def hh_chain_a(pool, pps, ppt, name, wdr, src_tiles,
                           ct_ap=None, ts=0, te=TCN):
                """Stage a: D0/R GEMM (true fp32 - feeds top-k), top-4
                one-hots, d0 selects, -G row gather. Returns tile dict."""
                D0 = pool.tile([128, TCN * 64], f16, tag="D0_sh",
                               name=f"D0_{name}", bufs=2)
                D0v = D0[:].rearrange("p (t n) -> p t n", t=TCN)
                RS = pool.tile([128, TCN * 64], dt.float32, tag="RS_sh",
                               name=f"RS_{name}", bufs=2)
                RSv = RS[:].rearrange("p (t n) -> p t n", t=TCN)
                M8 = pool.tile([128, TCN * 8], dt.float32, tag="M8_sh",
                               name=f"M8_{name}", bufs=2)
                M8v = M8[:].rearrange("p (t n) -> p t n", t=TCN)
                OH = pool.tile([128, TCN * 256], f16, tag="OH_sh",
                               name=f"OH_{name}", bufs=2)
                OHv = OH[:].rearrange("p (t n) -> p t n", t=TCN)
                OHT = pool.tile([128, 256], f16, tag="OHT_sh",
                                name=f"OHT_{name}", bufs=2)
                BT = pool.tile([128, TCN * 196], f16, tag="BT_sh",
                               name=f"BT_{name}", bufs=2)
                BTv = BT[:].rearrange("p (t n) -> p t n", t=TCN)
                DSA = pool.tile([128, TCN * 24], dt.float32, tag="DS_sh2",
                                name=f"DS_{name}", bufs=2)
                DSAv = DSA[:].rearrange("p (t n) -> p t n", t=TCN)
                BE = pool.tile([128, TCN * 4], dt.float32, tag=f"BE_{name}",
                               name=f"BE_{name}")
                BEv = BE[:].rearrange("p (t n) -> p t n", t=TCN)
                CC = pool.tile([128, TCN * 64], f16, tag="CC_sh",
                               name=f"CC_{name}", bufs=2)
                CCv = CC[:].rearrange("p (t n) -> p t n", t=TCN)
                CT = ct_ap if ct_ap is not None else pool.tile(
                    [64, 1024], f16, tag="CT_sh", name=f"CT_{name}", bufs=1)
                scr = pool.tile([128, 512], f16, tag="scr16_sh",
                                name=f"scr16_{name}", bufs=1)
                scrv = scr[:].rearrange("p (t n) -> p t n", t=TCN)

                tn = te - ts
                # D0/R: pack 4 chunks per PSUM bank (true fp32, 4-pass)
                psd = [pps.tile([128, 512], dt.float32, tag="ps_sf",
                                name="ps_sf", bufs=2)
                       for _ in range((tn + 3) // 4)]
                for t in range(ts, te):
                    tl = t - ts
                    for rc in range(RCN):
                        nc.tensor.matmul(
                            psd[tl // 4][:, 128 * (tl % 4):128 * (tl % 4 + 1)],
                            src_tiles[rc][:, 128 * t:128 * (t + 1)],
                            wdr[:, rc, :],
                            start=(rc == 0), stop=(rc == RCN - 1))
                for half in range(tn // 4):
                    pv = psd[half][:].rearrange("p (t n) -> p t n", t=4)
                    nc.scalar.copy(D0v[:, 4 * half:4 * half + 4, :],
                                   pv[:, :, 0:64])
                    nc.scalar.copy(RSv[:, 4 * half:4 * half + 4, :],
                                   pv[:, :, 64:128])
                for tl in range(tn):
                    nc.vector.max(M8v[:, tl, :], RSv[:, tl, :])
                # one-hots + d0 selects (batched over chunks)
                for i in range(4):
                    nc.vector.tensor_tensor(
                        OHv[:, 0:tn, 64 * i:64 * (i + 1)], RSv[:, 0:tn, :],
                        M8v[:, 0:tn, i:i + 1].to_broadcast((128, tn, 64)),
                        op.is_equal)
                    nc.vector.tensor_mul(scr[:, 0:64 * tn],
                                         OHv[:, 0:tn, 64 * i:64 * (i + 1)],
                                         D0v[:, 0:tn, :])
                    nc.vector.tensor_reduce(DSAv[:, 0:tn, i:i + 1],
                                            scrv[:, 0:tn, :],
                                            mybir.AxisListType.X, op.add)
                # gather -G rows + 2alpha via transposed one-hot GEMM (fp16)
                for t in range(ts, te):
                    tl = t - ts
                    psb = pps.tile([128, 196], dt.float32, tag="ps_mm",
                                   name="ps_b", bufs=3)
                    for half in range(2):
                        pt = ppt.tile([128, 128], f16, tag="ps_t16",
                                      name="ps_t16")
                        nc.tensor.transpose(
                            pt[:], OHv[:, tl, 128 * half:128 * (half + 1)],
                            ident16[:])
                        nc.scalar.copy(OHT[:, 128 * half:128 * (half + 1)], pt[:])
                        nc.tensor.matmul(
                            psb[:], OHT[:, 128 * half:128 * (half + 1)],
                            BD[:, half, :], start=(half == 0), stop=(half == 1))
                    nc.scalar.copy(BTv[:, tl, :], psb[:])
                return dict(OHv=OHv, BTv=BTv, DSAv=DSAv, BEv=BEv, CCv=CCv,
                            CC=CC, CT=CT, scr=scr, scrv=scrv, ts=ts, te=te)

            def hh_chain_b(ppt, st):
                OHv, BTv, DSAv = st["OHv"], st["BTv"], st["DSAv"]
                BEv, CCv, CC = st["BEv"], st["CCv"], st["CC"]
                CT, scr, scrv = st["CT"], st["scr"], st["scrv"]
                ts, te = st["ts"], st["te"]
                tn = te - ts
                # pair values -G[idx_i, idx_j]
                pair = {}
                pidx = 4
                for i in range(1, 4):
                    for j in range(i):
                        nc.vector.tensor_mul(
                            scr[:, 0:64 * tn], OHv[:, 0:tn, 64 * i:64 * (i + 1)],
                            BTv[:, 0:tn, 64 * j:64 * (j + 1)])
                        nc.vector.tensor_reduce(
                            DSAv[:, 0:tn, pidx:pidx + 1], scrv[:, 0:tn, :],
                            mybir.AxisListType.X, op.add)
                        pair[(i, j)] = DSAv[:, 0:tn, pidx:pidx + 1]
                        pidx += 1
                # recursion (batched [128, 8] ops)
                be = [BEv[:, 0:tn, i:i + 1] for i in range(4)]
                a2 = [BTv[:, 0:tn, 192 + i:193 + i] for i in range(4)]
                nc.vector.tensor_mul(be[0], DSAv[:, 0:tn, 0:1], a2[0])
                tmp = 10
                for i in range(1, 4):
                    cur = DSAv[:, 0:tn, i:i + 1]
                    for j in range(i):
                        t1 = DSAv[:, 0:tn, tmp:tmp + 1]; tmp += 1
                        nc.vector.tensor_mul(t1, pair[(i, j)], be[j])
                        t2 = DSAv[:, 0:tn, tmp:tmp + 1]; tmp += 1
                        nc.vector.tensor_add(t2, t1, cur)
                        cur = t2
                    nc.vector.tensor_mul(be[i], cur, a2[i])
                # c = sum beta_i * onehot_i
                nc.vector.tensor_mul(CCv[:, 0:tn, :], OHv[:, 0:tn, 0:64],
                                     be[0].to_broadcast((128, tn, 64)))
                for i in range(1, 4):
                    nc.vector.tensor_mul(
                        scr[:, 0:64 * tn], OHv[:, 0:tn, 64 * i:64 * (i + 1)],
                        be[i].to_broadcast((128, tn, 64)))
                    nc.vector.tensor_add(CC[:, 0:64 * tn], CC[:, 0:64 * tn],
                                         scr[:, 0:64 * tn])
                for t in range(ts, te):
                    tl = t - ts
                    ptc = ppt.tile([128, 128], f16, tag="ps_t16", name="ps_t16")
                    nc.tensor.transpose(ptc[0:64, :], CCv[:, tl, :], ident16[:])
                    nc.scalar.copy(CT[:, 128 * t:128 * (t + 1)], ptc[0:64, :])
                return CT

            def hh_chain(pool, pps, ppt, name, wdr, src_tiles, ct_ap=None):
                st = hh_chain_a(pool, pps, ppt, name, wdr, src_tiles,
                                ct_ap=ct_ap)
                return hh_chain_b(ppt, st)

            # ================= phase A: SF + xbT (f32r 1-pass) =============
            _pbc_cm = tc.tile_pool(name="bc", bufs=1)
            pbc = _pbc_cm.__enter__()
            # post-chain fp16 activations for attention
            XA = {r: [pbc.tile([128, 1024], f16, tag=f"xa{r}{rc}",
                               name=f"xa{r}{rc}")
                      for rc in range(RCN)] for r in "qkv"}
            VP = [pbc.tile([128, 520], f32r, tag=f"vp{kc}", name=f"vp{kc}")
                  for kc in range(KCN)]
            with tc.tile_pool(name="phb", bufs=1) as pb:
                SFt = pb.tile([128, TCN * 256], f16, tag="sf",
                              name="sf", bufs=1)
                SFv = SFt[:].rearrange("p (t n) -> p t n", t=TCN)
                XB = [pb.tile([128, 1024], f32r, tag=f"xb{rc}",
                              name=f"xb{rc}")
                      for rc in range(RCN)]
                # xc buffers rotate across routes (2 in flight)
                XC = {r: [pb.tile([128, 1024], dt.float32, tag=f"xc{rc}",
                                  name=f"xc{r}{rc}", bufs=2)
                          for rc in range(RCN)] for r in "qkv"}
                with (
                    tc.tile_pool(name="pha", bufs=1) as pa_,
                    tc.tile_pool(name="phx", bufs=3) as px,
                    tc.tile_pool(name="psA", bufs=4, space="PSUM") as psA,
                ):
                    W4t = pa_.tile([128, KCN * 256], f32r, tag="W4w", name="W4w")
                    nc.sync.dma_start(out=W4t[:, 0:1024],
                                      in_=wd["W4"][:, 0:1024].bitcast(f32r))
                    nc.scalar.dma_start(out=W4t[:, 1024:2048],
                                        in_=wd["W4"][:, 1024:2048].bitcast(f32r))
                    BIt = pa_.tile([128, KCN * 512], f32r, tag="BIw", name="BIw")
                    BI = BIt[:].rearrange("p (k n) -> p k n", k=KCN)
                    W4 = W4t[:].rearrange("p (k n) -> p k n", k=KCN)
                    for sweep in range(2):
                        ps_sf = [psA.tile([128, 256], dt.float32, tag="ps_asf",
                                          name="ps_asf") for _ in range(4)]
                        ps_xb = [psA.tile([128, 512], dt.float32, tag="ps_axb",
                                          name="ps_axb") for _ in range(4)]
                        for kc in range(KCN):
                            xt = px.tile([128, 1024], f32r, tag="xtc",
                                         name="xtc")
                            eng = nc.sync if kc % 2 == 0 else nc.scalar
                            eng.dma_start(out=xt[:],
                                          in_=XTv[:, kc, :].bitcast(f32r))
                            if sweep == 0 and kc in (0, 2):
                                half = kc // 2
                                nc.scalar.dma_start(
                                    out=BIt[:, 2048 * half:2048 * (half + 1)],
                                    in_=wd["BI"][:, 2048 * half:
                                                 2048 * (half + 1)]
                                    .bitcast(f32r))
                            for ti in range(4):
                                nc.tensor.matmul(
                                    ps_sf[ti][:],
                                    xt[:, 128 * ti + 512 * sweep:
                                       128 * (ti + 1) + 512 * sweep],
                                    W4[:, kc, :],
                                    start=(kc == 0), stop=(kc == KCN - 1))
                            for i in range(4):
                                rc, th = 2 * sweep + i // 2, i % 2
                                nc.tensor.matmul(
                                    ps_xb[i][:],
                                    BI[:, kc, 128 * rc:128 * (rc + 1)],
                                    xt[:, 512 * th:512 * (th + 1)],
                                    start=(kc == 0), stop=(kc == KCN - 1))
                        for ti in range(4):
                            t = 4 * sweep + ti
                            nc.scalar.copy(SFv[:, t, :], ps_sf[ti][:])
                        for i in range(4):
                            rc, th = 2 * sweep + i // 2, i % 2
                            nc.scalar.copy(
                                XB[rc][:, 512 * th:512 * (th + 1)], ps_xb[i][:])

                # ============ phase B: compress routes ============
                with (
                    tc.tile_pool(name="psB", bufs=1, space="PSUM") as pps,
                    tc.tile_pool(name="psBt", bufs=2, space="PSUM") as ppt,
                    tc.tile_pool(name="psBv", bufs=1, space="PSUM") as ppv,
                ):
                    def front_corr(ri, r):
                        """softmax_front + chat-correction for route r."""
                        CHT = softmax_front(
                            pb, ppt, ppv, r,
                            SFv[:, :, 64 * ri:64 * ri + 64],
                            SFv[:, :, 192:256], W["GIN"][:])
                        for rc in range(RCN):
                            for th in range(2):
                                ps = pps.tile([128, 512], dt.float32,
                                              tag="ps_mm", name="ps_mm", bufs=3)
                                nc.tensor.matmul(
                                    ps[:],
                                    W["NEGBH"][:][0:64, 128 * rc:128 * (rc + 1)],
                                    CHT[0:64, 512 * th:512 * (th + 1)],
                                    start=True, stop=False)
                                nc.tensor.matmul(
                                    ps[:], ident_r[:],
                                    XB[rc][:, 512 * th:512 * (th + 1)],
                                    start=False, stop=True)
                                if th == 0:
                                    nc.scalar.copy(
                                        XC[r][rc][:, 512 * th:512 * (th + 1)],
                                        ps[:])
                                else:
                                    nc.vector.tensor_copy(
                                        XC[r][rc][:, 512 * th:512 * (th + 1)],
                                        ps[:])

                    # software-pipelined routes: route r+1's front/correction
                    # is emitted between route r's chain_a and chain_b so every
                    # engine has fill work during r's serial recursion tail
                    ROUTES = [("v", 2), ("q", 0), ("k", 1)]
                    front_corr(ROUTES[0][1], ROUTES[0][0])
                    front_corr(ROUTES[1][1], ROUTES[1][0])
                    sts = {}
                    sts[0] = hh_chain_a(pb, pps, ppt, ROUTES[0][0],
                                        WDR[ROUTES[0][0].upper()],
                                        XC[ROUTES[0][0]])
                    CTL = pb.tile([64, 1024], f16, tag="ctl", name="ctl")
                    for ri, (r, sfi) in enumerate(ROUTES):
                        if ri + 1 < 2:
                            nr = ROUTES[ri + 1][0]
                            sts[ri + 1] = hh_chain_a(
                                pb, pps, ppt, nr, WDR[nr.upper()], XC[nr])
                        elif ri + 1 == 2:
                            # last route: split into token halves so the
                            # qh=0 attention entry starts one half earlier
                            nr = ROUTES[2][0]
                            sts[2] = [hh_chain_a(pb, pps, ppt, nr + str(hf),
                                                 WDR[nr.upper()], XC[nr],
                                                 ct_ap=CTL[:], ts=4 * hf,
                                                 te=4 * hf + 4)
                                      for hf in range(2)]
                        stc = sts.pop(ri)
                        if ri == 2:
                            for hf in range(2):
                                hh_chain_b(ppt, stc[hf])
                                for rc in range(RCN):
                                    th = hf
                                    ps = pps.tile([128, 512], dt.float32,
                                                  tag="ps_mm", name="ps_mm",
                                                  bufs=3)
                                    nc.tensor.matmul(
                                        ps[:], W["NEGP"][:][
                                            0:64, 128 * rc:128 * (rc + 1)],
                                        CTL[0:64, 512 * th:512 * (th + 1)],
                                        start=True, stop=True)
                                    nc.vector.tensor_add(
                                        XA[r][rc][:, 512 * th:512 * (th + 1)],
                                        XC[r][rc][:, 512 * th:512 * (th + 1)],
                                        ps[:])
                            continue
                        CT = hh_chain_b(ppt, stc)
                        # chain correction: fp16 1-pass GEMM + DVE add -> fp16
                        for rc in range(RCN):
                            for th in range(2):
                                ps = pps.tile([128, 512], dt.float32,
                                              tag="ps_mm", name="ps_mm", bufs=3)
                                nc.tensor.matmul(
                                    ps[:], W["NEGP"][:][
                                        0:64, 128 * rc:128 * (rc + 1)],
                                    CT[0:64, 512 * th:512 * (th + 1)],
                                    start=True, stop=True)
                                nc.vector.tensor_add(
                                    XA[r][rc][:, 512 * th:512 * (th + 1)],
                                    XC[r][rc][:, 512 * th:512 * (th + 1)],
                                    ps[:])
                        if ri + 2 < 3:
                            front_corr(ROUTES[ri + 2][1], ROUTES[ri + 2][0])
                        if r == "v":
                            # V ready first: build N-domain V' now so the
                            # attention entry does not stall on route tails
                            for rc in range(RCN):
                                for t in range(TCN):
                                    pt = ppt.tile([128, 128], f16,
                                                  tag="ps_t16", name="ps_t16")
                                    nc.tensor.transpose(
                                        pt[:],
                                        XA["v"][rc][:, 128 * t:128 * (t + 1)],
                                        ident16[:])
                                    dst = bass.AP(
                                        VP[t].tensor,
                                        VP[t].offset + 65 * (2 * rc),
                                        [VP[t].ap[0], [65, 2], [1, 64]])
                                    nc.scalar.copy(
                                        dst,
                                        pt[:].rearrange("p (h n) -> p h n",
                                                        h=2))
                    # ones columns for the softmax denominators
                    for t in range(TCN):
                        ones = VP[t][:].rearrange(
                            "p (h n) -> p h n", h=H)[:, :, 64:65]
                        nc.scalar.copy(ones, ones32[:, 0:8].rearrange(
                            "p (h n) -> p h n", h=H))

            # ================= phase C: attention =================
            with tc.tile_pool(name="att", bufs=1) as pa:
                # per-(head, token) softmax denominators, broadcast across each
                # 64-row head block by a stride-0-partition DMA from PSUM
                ZINV = [pa.tile([128, 1024], dt.float32, tag=f"zinv{rc}",
                                name=f"zinv{rc}") for rc in range(RCN)]
                ZRI = pa.tile([1, 16 * 512], dt.float32, tag="zri", name="zri")

                def new_pt(hi):
                    t = pa.tile([128, KCN * 512], f32r, tag=f"pt{hi}",
                                name=f"pt{hi}", bufs=2)
                    return t[:].rearrange("p (k n) -> p k n", k=KCN)
                with (
                    tc.tile_pool(name="psC", bufs=3, space="PSUM") as pps,
                    tc.tile_pool(name="psCv", bufs=2, space="PSUM") as ppv,
                ):
                    # software-pipelined: scores/exp of iteration n+1 are
                    # emitted before PV of n so ACT (exp) and PE (PV) overlap
                    iters = [(hp, qh, hi) for qh in range(2)
                             for hp in range(4) for hi in range(2)]
                    ptvs = {}

                    def scores_exp(n):
                        hp, qh, hi = iters[n]
                        hr = 64 * hi
                        ptv = ptvs[n] = new_pt(hi)
                        for g in range(4):
                            ps2 = pps.tile([128, 1024], dt.float32,
                                           tag="ps2", name="ps2", bufs=3)
                            for j in range(2):
                                kc = 2 * g + j
                                nc.tensor.matmul(
                                    ps2[:, 512 * j:512 * (j + 1)],
                                    XA["k"][hp][hr:hr + 64,
                                                128 * kc:128 * (kc + 1)],
                                    XA["q"][hp][hr:hr + 64,
                                                512 * qh:512 * (qh + 1)],
                                    start=True, stop=True)
                            nc.scalar.activation(
                                ptv[:, 2 * g:2 * g + 2, :], ps2[:],
                                act.Exp, scale=0.125)

                    def pv_stage(n):
                        hp, qh, hi = iters[n]
                        h, hr = 2 * hp + hi, 64 * hi
                        ptv = ptvs.pop(n)
                        pv65 = ppv.tile([128, 512], dt.float32,
                                        tag="ps_pv", name="ps_pv")
                        for kc in range(KCN):
                            nc.tensor.matmul(
                                pv65[0:65, :],
                                VP[kc][:, 65 * h:65 * h + 65],
                                ptv[:, kc, :],
                                start=(kc == 0), stop=(kc == KCN - 1))
                        nc.vector.tensor_copy(
                            AOT[hp][hr:hr + 64, 512 * qh:512 * (qh + 1)],
                            pv65[0:64, :])
                        zofs = 512 * (2 * h + qh)
                        nc.vector.reciprocal(
                            ZRI[0:1, zofs:zofs + 512], pv65[64:65, :])
                        zsrc = bass.AP(
                            ZRI.tensor, ZRI.offset + zofs,
                            [ZRI.ap[0], [0, 64], [1, 512]])
                        nc.sync.dma_start(
                            out=ZINV[hp][64 * hi:64 * (hi + 1),
                                         512 * qh:512 * (qh + 1)],
                            in_=zsrc)

                    scores_exp(0)
                    for n in range(16):
                        if n + 1 < 16:
                            scores_exp(n + 1)
                        pv_stage(n)
                        if n % 2 == 1:
                            th, rc = n // 8, (n % 8) // 2
                            nc.gpsimd.tensor_mul(
                                AOT[rc][:, 512 * th:512 * (th + 1)],
                                AOT[rc][:, 512 * th:512 * (th + 1)],
                                ZINV[rc][:, 512 * th:512 * (th + 1)])
                            nc.gpsimd.tensor_copy(
                                AOT16[rc][:, 512 * th:512 * (th + 1)],
                                AOT[rc][:, 512 * th:512 * (th + 1)])

            _pbc_cm.__exit__(None, None, None)
            # ================= phase D: expand =================
            with tc.tile_pool(name="exp", bufs=1) as pe:
                BOUTt = pe.tile([128, RCN * 1024], f16, tag="boutw", name="boutw")
                nc.sync.dma_start(out=BOUTt[:], in_=wd["BOUT"][:])
                BOUT = BOUTt[:].rearrange("p (k n) -> p k n", k=RCN)
                WDRO2t = pe.tile([128, RCN * 128], f16, tag="wdro2w",
                                 name="wdro2w")
                nc.sync.dma_start(out=WDRO2t[:], in_=wd["WDRO2"][:])
                WDRO2 = WDRO2t[:].rearrange("p (k n) -> p k n", k=RCN)
                NEGPOWt = pe.tile([64, 128], f16, tag="negpoww", name="negpoww")
                nc.sync.dma_start(out=NEGPOWt[:], in_=wd["NEGPOW"][:])
                NPBOHHt = pe.tile([128, 1024], f16, tag="npbohhw",
                                  name="npbohhw")
                nc.sync.dma_start(out=NPBOHHt[:], in_=wd["NPBOHH"][:])
                STK = pe.tile([128, 1024], f16, tag="stk", name="stk")
                SO = pe.tile([128, TCN * 128], dt.float32, tag="so", name="so")
                SOv = SO[:].rearrange("p (t n) -> p t n", t=TCN)
                with (
                    tc.tile_pool(name="psD", bufs=1, space="PSUM") as pps,
                    tc.tile_pool(name="psDt", bufs=2, space="PSUM") as ppt,
                    tc.tile_pool(name="psDv", bufs=1, space="PSUM") as ppv,
                ):
                    st1 = hh_chain_a(pe, pps, ppt, "o", WDR["O"], AOT,
                                     ct_ap=STK[0:64, :], ts=0, te=4)
                    st2 = hh_chain_a(pe, pps, ppt, "o2", WDR["O"], AOT,
                                     ct_ap=STK[0:64, :], ts=4, te=8)
                    hh_chain_b(ppt, st1)
                    CTo = hh_chain_b(ppt, st2)
                    for t in range(TCN):
                        ps = pps.tile([128, 128], dt.float32, tag="ps_sf",
                                      name="ps_sf", bufs=2)
                        for rc in range(RCN):
                            nc.tensor.matmul(
                                ps[:], AOT16[rc][:, 128 * t:128 * (t + 1)],
                                WDRO2[:, rc, :],
                                start=(rc == 0), stop=False)
                        nc.tensor.matmul(
                            ps[:], CTo[0:64, 128 * t:128 * (t + 1)],
                            NEGPOWt[0:64, :], start=False, stop=True)
                        nc.scalar.copy(SOv[:, t, :], ps[:, 0:128])
                    softmax_front(
                        pe, ppt, ppv, "o",
                        SOv[:, :, 0:64], SOv[:, :, 64:128], W["GOUT"][:],
                        cht_dtype=f16, cht_ap=STK[64:128, :])
                    # ao @ base_out accumulated with the Householder/chat
                    # correction in one PSUM group; the BOUT matmuls are ready
                    # early and back-fill PE stalls before STK lands
                    for dc in range(KCN):
                        for th in range(2):
                            ps = pps.tile([128, 512], dt.float32, tag="ps_mm",
                                          name="ps_mm", bufs=3)
                            for rc in range(RCN):
                                nc.tensor.matmul(
                                    ps[:], BOUT[:, rc, 128 * dc:128 * (dc + 1)],
                                    AOT16[rc][:, 512 * th:512 * (th + 1)],
                                    start=(rc == 0), stop=False)
                            nc.tensor.matmul(
                                ps[:], NPBOHHt[:][:, 128 * dc:128 * (dc + 1)],
                                STK[:, 512 * th:512 * (th + 1)],
                                start=False, stop=True)
                            ot = pe.tile([128, 512], dt.float32, tag="outsb",
                                         name="outsb", bufs=3)
                            if th == 0:
                                nc.scalar.copy(ot[:], ps[:])
                            else:
                                nc.vector.tensor_copy(ot[:], ps[:])
                            oeng = nc.sync if th == 0 else nc.gpsimd
                            oeng.dma_start(
                                out=OUTd[128 * dc:128 * (dc + 1),
                                         512 * th:512 * (th + 1)],
                                in_=ot[:])
    _split_sync_waits(nc)
    return nc


def get_built():
    if "nc" not in _BUILT:
        _BUILT["nc"] = build()
    return _BUILT["nc"]


def _in_maps(inputs):
    x = np.asarray(inputs["x"], np.float32)
    w = prep_weights(inputs)
    in_maps = []
    for c in range(NCORES):
        m = dict(w)
        m["XT"] = _r12(_pack_kc(np.ascontiguousarray(x[c].T), KCN, 128))
        in_maps.append(m)
    return in_maps


def kernel(**inputs):
    from concourse.bass_utils import run_bass_kernel_spmd

    nc = get_built()
    res = run_bass_kernel_spmd(nc, _in_maps(inputs),
                               core_ids=list(range(NCORES)))
    out = np.stack([res.results[c]["OUT"].T for c in range(NCORES)], axis=0)
    return out.astype(np.float32)


def run_timed(inputs, trace=False):
    from concourse.bass_utils import run_bass_kernel_spmd
    nc = get_built()
    return run_bass_kernel_spmd(nc, _in_maps(inputs),
                                core_ids=list(range(NCORES)), trace=trace)


# revision 51
# speedup vs baseline: 1.8099x; 1.0046x over previous
"""Trainium2 Bass kernel for nn_NeuronAttention (moe_routing).

Sharding: data-parallel over batch B=8 across 8 NeuronCores (one batch row
per core); weights replicated; no collectives.

Per-core computation uses two layouts: "T-domain" [feature, token] for PE
GEMMs (contraction on partitions) and "N-domain" [token, small-free] for
routing math (softmax, top-k, Householder-chain recursion) on DVE/ACT.
The Householder chains are evaluated in 64-dim dot-space: with
d0 = xc@P.T, G = P@P.T, a = 1/(||P_k||^2+EPS), the 4 selected reflections
reduce to the scalar recursion beta_i = 2a_i(d0_i - sum_{j<i} beta_j G_ij)
and a rank-64 correction xc - (sum_i beta_i e_{idx_i})@P.

Precision plan (hw-measured: f32r keeps ~13 mantissa bits, fp16 11, and a
full-m12 emulation of this pipeline gives 5e-3 rel err vs the 2e-2 gate):
  - proc-router score GEMMs (feed top-k) stay true fp32 4-pass, reading
    fp32 xc tiles, so selections match the reference almost everywhere;
  - phase-A GEMMs (x@[routers|hh], x@base_in) run 1-pass f32r on host-
    pre-rounded operands;
  - the in-softmax chat correction runs f32r (chat rounded at ~2^-13);
  - attention QK runs fp16 on post-chain fp16 copies, exp'd scores are
    stored f32r (full fp32 exponent range - no overflow), and PV streams
    the exp'd matrix as the f32r moving operand, producing attention
    output directly in T-domain [feature, token];
  - softmax denominators use an appended all-ones stationary column and
    an exact reciprocal + a stride-0 free-dim DMA broadcast across each
    64-row head block;
  - all post-selection / output GEMMs (one-hot gathers, chain corrections,
    x@base_out, final Householder correction) run fp16 1-pass.
"""

import numpy as np

B, S, D, R = 8, 1024, 1024, 512
NPROC, TOPK = 64, 4
H, DH = 8, 64
EPS = 1e-8
NCORES = 8
TCN = 8   # token chunks of 128
KCN = 8   # D chunks of 128
RCN = 4   # rank chunks of 128

_BUILT = {}


def _apply_tile_drain_patch():
    """walrus here rejects >1 sync-wait on CTRL-class instructions; split
    Tile's kernel-tail drain waits into a chain of single-wait nops."""
    import concourse.mybir as mybir
    from concourse.tile import TileContext
    from concourse.vector_clock import ScopedClock

    if getattr(TileContext, "_drain_patched", False):
        return

    def _patched(self, tick_clock, wait_clock):
        probe = self.nc.sync.nop()
        wait_clock.add_sem_waits(
            probe.ins, ScopedClock({None: tick_clock.global_clock}))
        si = probe.ins.sync_info
        waits = list(si.on_wait) if si is not None else []
        updates = list(si.on_update) if si is not None else []
        if len(waits) > 1:
            probe.ins.sync_info = mybir.SyncInfo(
                on_update=updates, on_wait=waits[:1])
            for ofs in range(1, len(waits)):
                extra = self.nc.sync.nop()
                extra.ins.sync_info = mybir.SyncInfo(
                    on_update=[], on_wait=waits[ofs:ofs + 1])
        self.nc.sync.drain()
        self.nc.all_engine_barrier()
        assert self.sems is not None
        popped = self.nc._tile_sem_poison_stack.pop()
        assert popped is self._sem_poison
        self.nc.clear_and_free_semaphores(list(self.sems.allocated().values()))
        self.nc.all_engine_barrier()

    TileContext._drain_and_barrier = _patched
    TileContext._drain_patched = True


def _split_sync_waits(nc):
    """walrus here accepts at most 1 sync-wait per instruction; hoist
    extra waits onto same-engine NoOps inserted just before."""
    import concourse.mybir as mybir

    ctr = [0]
    for f in nc.m.functions:
        for bb in f.blocks:
            insts = bb.instructions
            out = []
            for inst in insts:
                si = inst.sync_info
                if si is not None and len(si.on_wait) > 1:
                    waits = list(si.on_wait)
                    for w in waits[:-1]:
                        ctr[0] += 1
                        nop = mybir.InstNoOp(
                            name=f"I-sw{ctr[0]}", ins=[], outs=[])
                        nop.engine = inst.engine
                        nop.sync_info = mybir.SyncInfo(
                            on_update=[], on_wait=[w])
                        out.append(nop)
                    inst.sync_info = mybir.SyncInfo(
                        on_update=list(si.on_update), on_wait=[waits[-1]])
                out.append(inst)
            bb.instructions = out


def _pack_kc(a, nchunk, chunk, dtype=np.float32):
    # [nchunk*chunk, N] -> [chunk, nchunk*N], chunk-major partitions
    n = a.shape[1]
    return np.ascontiguousarray(
        a.reshape(nchunk, chunk, n).transpose(1, 0, 2).reshape(chunk, nchunk * n)
    ).astype(dtype)


def _r12(a, mbits=12):
    """Round to the f32r grid (12 explicit mantissa bits, RNE) so the PE's
    1-pass f32r read is exact."""
    a = np.asarray(a, np.float64)
    m, e = np.frexp(a)
    return np.ldexp(np.round(m * 2.0**mbits) / 2.0**mbits, e).astype(np.float32)


def prep_weights(inputs):
    f = {k: np.asarray(v, np.float64) for k, v in inputs.items()}
    P = f["process_hh"]
    G = P @ P.T
    alpha2 = 2.0 / ((P * P).sum(1) + EPS)
    ihh, ohh = f["input_hh"], f["output_hh"]
    base_in, base_out = f["base_input"], f["base_output"]
    Bo = ohh @ base_out.T

    w = {}
    w["BI"] = _r12(_pack_kc(base_in, KCN, 128))
    W4 = np.concatenate([f["q_in_router"].T, f["k_in_router"].T,
                         f["v_in_router"].T, ihh.T], axis=1)
    w["W4"] = _r12(_pack_kc(W4, KCN, 128))
    for nm, wp in (("WDRQ", "q_proc_router"), ("WDRK", "k_proc_router"),
                   ("WDRV", "v_proc_router"), ("WDRO", "o_proc_router")):
        w[nm] = _pack_kc(np.concatenate([P.T, f[wp].T], axis=1), RCN, 128)
    w["WDRO2"] = _pack_kc(
        np.concatenate([f["o_out_router"].T, Bo.T], axis=1), RCN, 128,
        np.float16)
    w["NEGBH"] = _r12(-(ihh @ base_in))
    w["NEGP"] = (-P).astype(np.float16)
    w["GIN"] = (ihh @ ihh.T).astype(np.float16)
    w["GOUT"] = (ohh @ ohh.T).astype(np.float16)
    BD = np.zeros((256, 196))
    for i in range(4):
        if i < 3:
            BD[64 * i:64 * i + 64, 64 * i:64 * i + 64] = -G
        BD[64 * i:64 * i + 64, 192 + i] = alpha2
    w["BD"] = _pack_kc(BD, 2, 128, np.float16)
    w["BOUT"] = _pack_kc(base_out, RCN, 128, np.float16)
    w["NEGPOW"] = (-(P @ np.concatenate(
        [f["o_out_router"].T, Bo.T], axis=1))).astype(np.float16)[:, 0:128]
    w["NPBOHH"] = np.concatenate(
        [-(P @ base_out), -ohh], axis=0).astype(np.float16)
    return w


def build():
    import concourse.bass as bass
    import concourse.mybir as mybir
    from concourse.tile import TileContext
    from concourse.masks import make_identity

    _apply_tile_drain_patch()
    dt = mybir.dt
    op = mybir.AluOpType
    act = mybir.ActivationFunctionType
    f32r = dt.float32r
    f16 = dt.float16

    nc = bass.Bass()
    XTd = nc.dram_tensor("XT", (128, KCN * 1024), dt.float32, kind="ExternalInput")
    wd = {}
    for nm, shape, wdt in (
        ("BI", (128, KCN * 512), dt.float32), ("W4", (128, KCN * 256), dt.float32),
        ("WDRQ", (128, RCN * 128), dt.float32), ("WDRK", (128, RCN * 128), dt.float32),
        ("WDRV", (128, RCN * 128), dt.float32), ("WDRO", (128, RCN * 128), dt.float32),
        ("WDRO2", (128, RCN * 128), f16),
        ("NEGBH", (64, 512), dt.float32), ("NEGP", (64, 512), f16),
        ("GIN", (64, 64), f16), ("GOUT", (64, 64), f16),
        ("BD", (128, 2 * 196), f16), ("BOUT", (128, RCN * 1024), f16),
        ("NEGPOW", (64, 128), f16), ("NPBOHH", (128, 1024), f16),
    ):
        wd[nm] = nc.dram_tensor(nm, shape, wdt, kind="ExternalInput")
    OUTd = nc.dram_tensor("OUT", (1024, 1024), dt.float32, kind="ExternalOutput")

    with TileContext(nc) as tc:
        with (
            tc.tile_pool(name="w", bufs=1) as pw,
            tc.tile_pool(name="live", bufs=1) as pl,
        ):
            # small weights: issue on the Pool sequencer's DMA queue so the
            # SP queue serves BI/W4/xt first (PE's critical path at start)
            W = {}
            for nm, dram in wd.items():
                if nm in ("BOUT", "WDRO2", "NEGPOW", "NPBOHH", "BI", "W4"):
                    continue
                if nm == "NEGBH":
                    # consumed by an f32r matmul; host pre-rounds, DMA as f32r
                    t = pw.tile(list(dram.shape), f32r, tag=nm)
                    nc.gpsimd.dma_start(out=t[:], in_=dram[:].bitcast(f32r))
                else:
                    t = pw.tile(list(dram.shape), dram.dtype, tag=nm)
                    nc.gpsimd.dma_start(out=t[:], in_=dram[:])
                W[nm] = t
            ident = pw.tile([128, 128], dt.float32, tag="ident", name="ident")
            make_identity(nc, ident[:])
            ident16 = pw.tile([128, 128], f16, tag="ident16", name="ident16")
            nc.scalar.copy(ident16[:], ident[:])
            ident_r = pw.tile([128, 128], f32r, tag="identr", name="identr")
            nc.scalar.copy(ident_r[:], ident[:])
            ones32 = pw.tile([128, 8], dt.float32, tag="ones32", name="ones32")
            nc.vector.memset(ones32[:], 1.0)

            BD = W["BD"][:].rearrange("p (k n) -> p k n", k=2)
            WDR = {r: W["WDR" + r][:].rearrange("p (k n) -> p k n", k=RCN)
                   for r in ("Q", "K", "V", "O")}
            XTv = XTd[:].rearrange("p (k n) -> p k n", k=KCN)

            # persistent activations: attention output, T-domain
            AOT = [pl.tile([128, 1024], dt.float32, tag=f"aot{rc}", name=f"aot{rc}")
                   for rc in range(RCN)]
            AOT16 = [pl.tile([128, 1024], f16, tag=f"aot16{rc}", name=f"aot16{rc}")
                     for rc in range(RCN)]

            # ---------- shared helpers ----------

            def softmax_front(pool, ppt, ppv, name, s_all, f_all, gram,
                              cht_dtype=None, cht_ap=None):
                """Batched over all 8 token chunks. s_all/f_all are
                [128, 8, 64] APs. Returns CHT [64, 1024] (chat^T)."""
                E = pool.tile([128, TCN * 64], f16, tag="E_sh",
                              name=f"E_{name}", bufs=1)
                Ev = E[:].rearrange("p (t n) -> p t n", t=TCN)
                ET = pool.tile([64, 1024], f16, tag="ET_sh",
                               name=f"ET_{name}", bufs=1)
                CH = pool.tile([128, TCN * 64], f16, tag="CH_sh",
                               name=f"CH_{name}", bufs=1)
                CHv = CH[:].rearrange("p (t n) -> p t n", t=TCN)
                CHT = cht_ap if cht_ap is not None else pool.tile(
                    [64, 1024], cht_dtype or f32r,
                    tag="CHT_sh", name=f"CHT_{name}", bufs=1)
                SC = pool.tile([128, 8 * 8], dt.float32, tag=f"sc1_{name}",
                               name=f"sc1_{name}")
                SCv = SC[:].rearrange("p (t n) -> p t n", t=8)
                scr = pool.tile([128, 512], dt.float32, tag="scr_sh",
                                name=f"scr_{name}", bufs=1)

                nc.scalar.activation(Ev[:, :, :], s_all, act.Exp)
                Z8 = SCv[:, :, 0:1]
                nc.vector.tensor_reduce(Z8, Ev[:, :, :],
                                        mybir.AxisListType.X, op.add)
                # u = e @ Gin per chunk, packed into one PSUM bank
                pu = ppv.tile([128, 512], dt.float32, tag="ps_u", name="ps_u")
                for t in range(TCN):
                    pt = ppt.tile([128, 128], f16, tag="ps_t16", name="ps_t16")
                    nc.tensor.transpose(pt[0:64, :], Ev[:, t, :], ident16[:])
                    nc.scalar.copy(ET[:, 128 * t:128 * (t + 1)], pt[0:64, :])
                    nc.tensor.matmul(pu[:, 64 * t:64 * (t + 1)],
                                     ET[:, 128 * t:128 * (t + 1)], gram,
                                     start=True, stop=True)
                puv = pu[:].rearrange("p (t n) -> p t n", t=TCN)
                pacc, qacc = SCv[:, :, 1:2], SCv[:, :, 2:3]
                nc.vector.tensor_mul(scr[:], Ev[:, :, :], f_all)
                nc.vector.tensor_reduce(
                    pacc, scr[:].rearrange("p (t n) -> p t n", t=TCN),
                    mybir.AxisListType.X, op.add)
                nc.vector.tensor_mul(scr[:], Ev[:, :, :], puv)
                nc.vector.tensor_reduce(
                    qacc, scr[:].rearrange("p (t n) -> p t n", t=TCN),
                    mybir.AxisListType.X, op.add)
                z2, den = SCv[:, :, 3:4], SCv[:, :, 4:5]
                rec, gam = SCv[:, :, 5:6], SCv[:, :, 6:7]
                nc.vector.tensor_mul(z2, Z8, Z8)
                nc.vector.scalar_tensor_tensor(out=den, in0=z2, scalar=EPS,
                                               in1=qacc, op0=op.mult, op1=op.add)
                nc.vector.reciprocal(rec, den)
                nc.vector.scalar_tensor_tensor(out=gam, in0=pacc, scalar=2.0,
                                               in1=rec, op0=op.mult, op1=op.mult)
                nc.vector.tensor_mul(CHv[:, :, :], Ev[:, :, :],
                                     gam.to_broadcast((128, TCN, 64)))
                for t in range(TCN):
                    pt2 = ppt.tile([128, 128], f16, tag="ps_t16", name="ps_t16")
                    nc.tensor.transpose(pt2[0:64, :], CHv[:, t, :], ident16[:])
                    nc.scalar.copy(CHT[:, 128 * t:128 * (t + 1)], pt2[0:64, :])
                return CHT

            def hh_chain_a(pool, pps, ppt, name, wdr, src_tiles,
                           ct_ap=None, ts=0, te=TCN):
                """Stage a: D0/R GEMM (true fp32 - feeds top-k), top-4
                one-hots, d0 selects, -G row gather. Returns tile dict."""
                D0 = pool.tile([128, TCN * 64], f16, tag="D0_sh",
                               name=f"D0_{name}", bufs=2)
                D0v = D0[:].rearrange("p (t n) -> p t n", t=TCN)
                RS = pool.tile([128, TCN * 64], dt.float32, tag="RS_sh",
                               name=f"RS_{name}", bufs=2)
                RSv = RS[:].rearrange("p (t n) -> p t n", t=TCN)
                M8 = pool.tile([128, TCN * 8], dt.float32, tag="M8_sh",
                               name=f"M8_{name}", bufs=2)
                M8v = M8[:].rearrange("p (t n) -> p t n", t=TCN)
                OH = pool.tile([128, TCN * 256], f16, tag="OH_sh",
                               name=f"OH_{name}", bufs=2)
                OHv = OH[:].rearrange("p (t n) -> p t n", t=TCN)
                OHT = pool.tile([128, 256], f16, tag="OHT_sh",
                                name=f"OHT_{name}", bufs=2)
                BT = pool.tile([128, TCN * 196], f16, tag="BT_sh",
                               name=f"BT_{name}", bufs=2)
                BTv = BT[:].rearrange("p (t n) -> p t n", t=TCN)
                DSA = pool.tile([128, TCN * 24], dt.float32, tag="DS_sh2",
                                name=f"DS_{name}", bufs=2)
                DSAv = DSA[:].rearrange("p (t n) -> p t n", t=TCN)
                BE = pool.tile([128, TCN * 4], dt.float32, tag=f"BE_{name}",
                               name=f"BE_{name}")
                BEv = BE[:].rearrange("p (t n) -> p t n", t=TCN)
                CC = pool.tile([128, TCN * 64], f16, tag="CC_sh",
                               name=f"CC_{name}", bufs=2)
                CCv = CC[:].rearrange("p (t n) -> p t n", t=TCN)
                CT = ct_ap if ct_ap is not None else pool.tile(
                    [64, 1024], f16, tag="CT_sh", name=f"CT_{name}", bufs=1)
                scr = pool.tile([128, 512], f16, tag="scr16_sh",
                                name=f"scr16_{name}", bufs=1)
                scrv = scr[:].rearrange("p (t n) -> p t n", t=TCN)

                tn = te - ts
                # D0/R: pack 4 chunks per PSUM bank (true fp32, 4-pass)
                psd = [pps.tile([128, 512], dt.float32, tag="ps_sf",
                                name="ps_sf", bufs=2)
                       for _ in range((tn + 3) // 4)]
                for t in range(ts, te):
                    tl = t - ts
                    for rc in range(RCN):
                        nc.tensor.matmul(
                            psd[tl // 4][:, 128 * (tl % 4):128 * (tl % 4 + 1)],
                            src_tiles[rc][:, 128 * t:128 * (t + 1)],
                            wdr[:, rc, :],
                            start=(rc == 0), stop=(rc == RCN - 1))
                for half in range(tn // 4):
                    pv = psd[half][:].rearrange("p (t n) -> p t n", t=4)
                    nc.scalar.copy(D0v[:, 4 * half:4 * half + 4, :],
                                   pv[:, :, 0:64])
                    nc.scalar.copy(RSv[:, 4 * half:4 * half + 4, :],
                                   pv[:, :, 64:128])
                for tl in range(tn):
                    nc.vector.max(M8v[:, tl, :], RSv[:, tl, :])
                # one-hots + d0 selects (batched over chunks)
                for i in range(4):
                    nc.vector.tensor_tensor(
                        OHv[:, 0:tn, 64 * i:64 * (i + 1)], RSv[:, 0:tn, :],
                        M8v[:, 0:tn, i:i + 1].to_broadcast((128, tn, 64)),
                        op.is_equal)
                    nc.vector.tensor_mul(scr[:, 0:64 * tn],
                                         OHv[:, 0:tn, 64 * i:64 * (i + 1)],
                                         D0v[:, 0:tn, :])
                    nc.vector.tensor_reduce(DSAv[:, 0:tn, i:i + 1],
                                            scrv[:, 0:tn, :],
                                            mybir.AxisListType.X, op.add)
                # gather -G rows + 2alpha via transposed one-hot GEMM (fp16)
                for t in range(ts, te):
                    tl = t - ts
                    psb = pps.tile([128, 196], dt.float32, tag="ps_mm",
                                   name="ps_b", bufs=3)
                    for half in range(2):
                        pt = ppt.tile([128, 128], f16, tag="ps_t16",
                                      name="ps_t16")
                        nc.tensor.transpose(
                            pt[:], OHv[:, tl, 128 * half:128 * (half + 1)],
                            ident16[:])
                        nc.scalar.copy(OHT[:, 128 * half:128 * (half + 1)], pt[:])
                        nc.tensor.matmul(
                            psb[:], OHT[:, 128 * half:128 * (half + 1)],
                            BD[:, half, :], start=(half == 0), stop=(half == 1))
                    nc.scalar.copy(BTv[:, tl, :], psb[:])
                return dict(OHv=OHv, BTv=BTv, DSAv=DSAv, BEv=BEv, CCv=CCv,
                            CC=CC, CT=CT, scr=scr, scrv=scrv, ts=ts, te=te)

            def hh_chain_b(ppt, st):
                OHv, BTv, DSAv = st["OHv"], st["BTv"], st["DSAv"]
                BEv, CCv, CC = st["BEv"], st["CCv"], st["CC"]
                CT, scr, scrv = st["CT"], st["scr"], st["scrv"]
                ts, te = st["ts"], st["te"]
                tn = te - ts
                # pair values -G[idx_i, idx_j]
                pair = {}
                pidx = 4
                for i in range(1, 4):
                    for j in range(i):
                        nc.vector.tensor_mul(
                            scr[:, 0:64 * tn], OHv[:, 0:tn, 64 * i:64 * (i + 1)],
                            BTv[:, 0:tn, 64 * j:64 * (j + 1)])
                        nc.vector.tensor_reduce(
                            DSAv[:, 0:tn, pidx:pidx + 1], scrv[:, 0:tn, :],
                            mybir.AxisListType.X, op.add)
                        pair[(i, j)] = DSAv[:, 0:tn, pidx:pidx + 1]
                        pidx += 1
                # recursion (batched [128, 8] ops)
                be = [BEv[:, 0:tn, i:i + 1] for i in range(4)]
                a2 = [BTv[:, 0:tn, 192 + i:193 + i] for i in range(4)]
                nc.vector.tensor_mul(be[0], DSAv[:, 0:tn, 0:1], a2[0])
                tmp = 10
                for i in range(1, 4):
                    cur = DSAv[:, 0:tn, i:i + 1]
                    for j in range(i):
                        t1 = DSAv[:, 0:tn, tmp:tmp + 1]; tmp += 1
                        nc.vector.tensor_mul(t1, pair[(i, j)], be[j])
                        t2 = DSAv[:, 0:tn, tmp:tmp + 1]; tmp += 1
                        nc.vector.tensor_add(t2, t1, cur)
                        cur = t2
                    nc.vector.tensor_mul(be[i], cur, a2[i])
                # c = sum beta_i * onehot_i
                nc.vector.tensor_mul(CCv[:, 0:tn, :], OHv[:, 0:tn, 0:64],
                                     be[0].to_broadcast((128, tn, 64)))
                for i in range(1, 4):
                    nc.vector.tensor_mul(
                        scr[:, 0:64 * tn], OHv[:, 0:tn, 64 * i:64 * (i + 1)],
                        be[i].to_broadcast((128, tn, 64)))
                    nc.vector.tensor_add(CC[:, 0:64 * tn], CC[:, 0:64 * tn],
                                         scr[:, 0:64 * tn])
                for t in range(ts, te):
                    tl = t - ts
                    ptc = ppt.tile([128, 128], f16, tag="ps_t16", name="ps_t16")
                    nc.tensor.transpose(ptc[0:64, :], CCv[:, tl, :], ident16[:])
                    nc.scalar.copy(CT[:, 128 * t:128 * (t + 1)], ptc[0:64, :])
                return CT

            def hh_chain(pool, pps, ppt, name, wdr, src_tiles, ct_ap=None):
                st = hh_chain_a(pool, pps, ppt, name, wdr, src_tiles,
                                ct_ap=ct_ap)
                return hh_chain_b(ppt, st)

            # ================= phase A: SF + xbT (f32r 1-pass) =============
            _pbc_cm = tc.tile_pool(name="bc", bufs=1)
            pbc = _pbc_cm.__enter__()
            # post-chain fp16 activations for attention
            XA = {r: [pbc.tile([128, 1024], f16, tag=f"xa{r}{rc}",
                               name=f"xa{r}{rc}")
                      for rc in range(RCN)] for r in "qkv"}
            VP = [pbc.tile([128, 520], f32r, tag=f"vp{kc}", name=f"vp{kc}")
                  for kc in range(KCN)]
            with tc.tile_pool(name="phb", bufs=1) as pb:
                SFt = pb.tile([128, TCN * 256], f16, tag="sf",
                              name="sf", bufs=1)
                SFv = SFt[:].rearrange("p (t n) -> p t n", t=TCN)
                XB = [pb.tile([128, 1024], f32r, tag=f"xb{rc}",
                              name=f"xb{rc}")
                      for rc in range(RCN)]
                # xc buffers rotate across routes (2 in flight)
                XC = {r: [pb.tile([128, 1024], dt.float32, tag=f"xc{rc}",
                                  name=f"xc{r}{rc}", bufs=2)
                          for rc in range(RCN)] for r in "qkv"}
                with (
                    tc.tile_pool(name="pha", bufs=1) as pa_,
                    tc.tile_pool(name="phx", bufs=3) as px,
                    tc.tile_pool(name="psA", bufs=4, space="PSUM") as psA,
                ):
                    W4t = pa_.tile([128, KCN * 256], f32r, tag="W4w", name="W4w")
                    nc.sync.dma_start(out=W4t[:, 0:1024],
                                      in_=wd["W4"][:, 0:1024].bitcast(f32r))
                    nc.scalar.dma_start(out=W4t[:, 1024:2048],
                                        in_=wd["W4"][:, 1024:2048].bitcast(f32r))
                    BIt = pa_.tile([128, KCN * 512], f32r, tag="BIw", name="BIw")
                    BI = BIt[:].rearrange("p (k n) -> p k n", k=KCN)
                    W4 = W4t[:].rearrange("p (k n) -> p k n", k=KCN)
                    xts0 = {}
                    for sweep in range(2):
                        ps_sf = [psA.tile([128, 256], dt.float32, tag="ps_asf",
                                          name="ps_asf") for _ in range(4)]
                        ps_xb = [psA.tile([128, 512], dt.float32, tag="ps_axb",
                                          name="ps_axb") for _ in range(4)]
                        # sweep 1 runs kc reversed: the last 3 chunks are
                        # still resident in the 3-deep prefetch rotation
                        order = (list(range(KCN)) if sweep == 0
                                 else list(range(KCN - 1, -1, -1)))
                        for ki, kc in enumerate(order):
                            if sweep == 1 and kc >= KCN - 3:
                                xt = xts0[kc]
                            else:
                                xt = px.tile([128, 1024], f32r, tag="xtc",
                                             name="xtc")
                                eng = nc.sync if kc % 2 == 0 else nc.scalar
                                eng.dma_start(
                                    out=xt[:], in_=XTv[:, kc, :].bitcast(f32r))
                                if sweep == 0:
                                    xts0[kc] = xt
                            if sweep == 0 and kc in (0, 2):
                                half = kc // 2
                                nc.scalar.dma_start(
                                    out=BIt[:, 2048 * half:2048 * (half + 1)],
                                    in_=wd["BI"][:, 2048 * half:
                                                 2048 * (half + 1)]
                                    .bitcast(f32r))
                            for ti in range(4):
                                nc.tensor.matmul(
                                    ps_sf[ti][:],
                                    xt[:, 128 * ti + 512 * sweep:
                                       128 * (ti + 1) + 512 * sweep],
                                    W4[:, kc, :],
                                    start=(ki == 0), stop=(ki == KCN - 1))
                            for i in range(4):
                                rc, th = 2 * sweep + i // 2, i % 2
                                nc.tensor.matmul(
                                    ps_xb[i][:],
                                    BI[:, kc, 128 * rc:128 * (rc + 1)],
                                    xt[:, 512 * th:512 * (th + 1)],
                                    start=(ki == 0), stop=(ki == KCN - 1))
                        for ti in range(4):
                            t = 4 * sweep + ti
                            nc.scalar.copy(SFv[:, t, :], ps_sf[ti][:])
                        for i in range(4):
                            rc, th = 2 * sweep + i // 2, i % 2
                            nc.scalar.copy(
                                XB[rc][:, 512 * th:512 * (th + 1)], ps_xb[i][:])

                # ============ phase B: compress routes ============
                with (
                    tc.tile_pool(name="psB", bufs=1, space="PSUM") as pps,
                    tc.tile_pool(name="psBt", bufs=2, space="PSUM") as ppt,
                    tc.tile_pool(name="psBv", bufs=1, space="PSUM") as ppv,
                ):
                    def front_corr(ri, r):
                        """softmax_front + chat-correction for route r."""
                        CHT = softmax_front(
                            pb, ppt, ppv, r,
                            SFv[:, :, 64 * ri:64 * ri + 64],
                            SFv[:, :, 192:256], W["GIN"][:])
                        for rc in range(RCN):
                            for th in range(2):
                                ps = pps.tile([128, 512], dt.float32,
                                              tag="ps_mm", name="ps_mm", bufs=3)
                                nc.tensor.matmul(
                                    ps[:],
                                    W["NEGBH"][:][0:64, 128 * rc:128 * (rc + 1)],
                                    CHT[0:64, 512 * th:512 * (th + 1)],
                                    start=True, stop=False)
                                nc.tensor.matmul(
                                    ps[:], ident_r[:],
                                    XB[rc][:, 512 * th:512 * (th + 1)],
                                    start=False, stop=True)
                                if th == 0:
                                    nc.scalar.copy(
                                        XC[r][rc][:, 512 * th:512 * (th + 1)],
                                        ps[:])
                                else:
                                    nc.vector.tensor_copy(
                                        XC[r][rc][:, 512 * th:512 * (th + 1)],
                                        ps[:])

                    # software-pipelined routes: route r+1's front/correction
                    # is emitted between route r's chain_a and chain_b so every
                    # engine has fill work during r's serial recursion tail
                    ROUTES = [("v", 2), ("q", 0), ("k", 1)]
                    front_corr(ROUTES[0][1], ROUTES[0][0])
                    front_corr(ROUTES[1][1], ROUTES[1][0])
                    sts = {}
                    sts[0] = hh_chain_a(pb, pps, ppt, ROUTES[0][0],
                                        WDR[ROUTES[0][0].upper()],
                                        XC[ROUTES[0][0]])
                    CTL = pb.tile([64, 1024], f16, tag="ctl", name="ctl")
                    for ri, (r, sfi) in enumerate(ROUTES):
                        if ri + 1 < 2:
                            nr = ROUTES[ri + 1][0]
                            sts[ri + 1] = hh_chain_a(
                                pb, pps, ppt, nr, WDR[nr.upper()], XC[nr])
                        elif ri + 1 == 2:
                            # last route: split into token halves so the
                            # qh=0 attention entry starts one half earlier
                            nr = ROUTES[2][0]
                            sts[2] = [hh_chain_a(pb, pps, ppt, nr + str(hf),
                                                 WDR[nr.upper()], XC[nr],
                                                 ct_ap=CTL[:], ts=4 * hf,
                                                 te=4 * hf + 4)
                                      for hf in range(2)]
                        stc = sts.pop(ri)
                        if ri == 2:
                            for hf in range(2):
                                hh_chain_b(ppt, stc[hf])
                                for rc in range(RCN):
                                    th = hf
                                    ps = pps.tile([128, 512], dt.float32,
                                                  tag="ps_mm", name="ps_mm",
                                                  bufs=3)
                                    nc.tensor.matmul(
                                        ps[:], W["NEGP"][:][
                                            0:64, 128 * rc:128 * (rc + 1)],
                                        CTL[0:64, 512 * th:512 * (th + 1)],
                                        start=True, stop=True)
                                    nc.vector.tensor_add(
                                        XA[r][rc][:, 512 * th:512 * (th + 1)],
                                        XC[r][rc][:, 512 * th:512 * (th + 1)],
                                        ps[:])
                            continue
                        CT = hh_chain_b(ppt, stc)
                        # chain correction: fp16 1-pass GEMM + DVE add -> fp16
                        for rc in range(RCN):
                            for th in range(2):
                                ps = pps.tile([128, 512], dt.float32,
                                              tag="ps_mm", name="ps_mm", bufs=3)
                                nc.tensor.matmul(
                                    ps[:], W["NEGP"][:][
                                        0:64, 128 * rc:128 * (rc + 1)],
                                    CT[0:64, 512 * th:512 * (th + 1)],
                                    start=True, stop=True)
                                nc.vector.tensor_add(
                                    XA[r][rc][:, 512 * th:512 * (th + 1)],
                                    XC[r][rc][:, 512 * th:512 * (th + 1)],
                                    ps[:])
                        if ri + 2 < 3:
                            front_corr(ROUTES[ri + 2][1], ROUTES[ri + 2][0])
                        if r == "v":
                            # V ready first: build N-domain V' now so the
                            # attention entry does not stall on route tails
                            for rc in range(RCN):
                                for t in range(TCN):
                                    pt = ppt.tile([128, 128], f16,
                                                  tag="ps_t16", name="ps_t16")
                                    nc.tensor.transpose(
                                        pt[:],
                                        XA["v"][rc][:, 128 * t:128 * (t + 1)],
                                        ident16[:])
                                    dst = bass.AP(
                                        VP[t].tensor,
                                        VP[t].offset + 65 * (2 * rc),
                                        [VP[t].ap[0], [65, 2], [1, 64]])
                                    nc.scalar.copy(
                                        dst,
                                        pt[:].rearrange("p (h n) -> p h n",
                                                        h=2))
                    # ones columns for the softmax denominators
                    for t in range(TCN):
                        ones = VP[t][:].rearrange(
                            "p (h n) -> p h n", h=H)[:, :, 64:65]
                        nc.scalar.copy(ones, ones32[:, 0:8].rearrange(
                            "p (h n) -> p h n", h=H))

            # ================= phase C: attention =================
            with tc.tile_pool(name="att", bufs=1) as pa:
                # per-(head, token) softmax denominators, broadcast across each
                # 64-row head block by a stride-0-partition DMA from PSUM
                ZINV = [pa.tile([128, 1024], dt.float32, tag=f"zinv{rc}",
                                name=f"zinv{rc}") for rc in range(RCN)]
                ZRI = pa.tile([1, 16 * 512], dt.float32, tag="zri", name="zri")

                def new_pt(hi):
                    t = pa.tile([128, KCN * 512], f32r, tag=f"pt{hi}",
                                name=f"pt{hi}", bufs=2)
                    return t[:].rearrange("p (k n) -> p k n", k=KCN)
                with (
                    tc.tile_pool(name="psC", bufs=3, space="PSUM") as pps,
                    tc.tile_pool(name="psCv", bufs=2, space="PSUM") as ppv,
                ):
                    # software-pipelined: scores/exp of iteration n+1 are
                    # emitted before PV of n so ACT (exp) and PE (PV) overlap
                    iters = [(hp, qh, hi) for qh in range(2)
                             for hp in range(4) for hi in range(2)]
                    ptvs = {}

                    def scores_exp(n):
                        hp, qh, hi = iters[n]
                        hr = 64 * hi
                        ptv = ptvs[n] = new_pt(hi)
                        for g in range(4):
                            ps2 = pps.tile([128, 1024], dt.float32,
                                           tag="ps2", name="ps2", bufs=3)
                            for j in range(2):
                                kc = 2 * g + j
                                nc.tensor.matmul(
                                    ps2[:, 512 * j:512 * (j + 1)],
                                    XA["k"][hp][hr:hr + 64,
                                                128 * kc:128 * (kc + 1)],
                                    XA["q"][hp][hr:hr + 64,
                                                512 * qh:512 * (qh + 1)],
                                    start=True, stop=True)
                            nc.scalar.activation(
                                ptv[:, 2 * g:2 * g + 2, :], ps2[:],
                                act.Exp, scale=0.125)

                    def pv_stage(n):
                        hp, qh, hi = iters[n]
                        h, hr = 2 * hp + hi, 64 * hi
                        ptv = ptvs.pop(n)
                        pv65 = ppv.tile([128, 512], dt.float32,
                                        tag="ps_pv", name="ps_pv")
                        for kc in range(KCN):
                            nc.tensor.matmul(
                                pv65[0:65, :],
                                VP[kc][:, 65 * h:65 * h + 65],
                                ptv[:, kc, :],
                                start=(kc == 0), stop=(kc == KCN - 1))
                        nc.vector.tensor_copy(
                            AOT[hp][hr:hr + 64, 512 * qh:512 * (qh + 1)],
                            pv65[0:64, :])
                        zofs = 512 * (2 * h + qh)
                        nc.vector.reciprocal(
                            ZRI[0:1, zofs:zofs + 512], pv65[64:65, :])
                        zsrc = bass.AP(
                            ZRI.tensor, ZRI.offset + zofs,
                            [ZRI.ap[0], [0, 64], [1, 512]])
                        nc.sync.dma_start(
                            out=ZINV[hp][64 * hi:64 * (hi + 1),
                                         512 * qh:512 * (qh + 1)],
                            in_=zsrc)

                    scores_exp(0)
                    for n in range(16):
                        if n + 1 < 16:
                            scores_exp(n + 1)
                        pv_stage(n)
                        if n % 2 == 1:
                            th, rc = n // 8, (n % 8) // 2
                            nc.gpsimd.tensor_mul(
                                AOT[rc][:, 512 * th:512 * (th + 1)],
                                AOT[rc][:, 512 * th:512 * (th + 1)],
                                ZINV[rc][:, 512 * th:512 * (th + 1)])
                            nc.gpsimd.tensor_copy(
                                AOT16[rc][:, 512 * th:512 * (th + 1)],
                                AOT[rc][:, 512 * th:512 * (th + 1)])

            _pbc_cm.__exit__(None, None, None)
            # ================= phase D: expand =================
            with tc.tile_pool(name="exp", bufs=1) as pe:
                BOUTt = pe.tile([128, RCN * 1024], f16, tag="boutw", name="boutw")
                nc.sync.dma_start(out=BOUTt[:], in_=wd["BOUT"][:])
                BOUT = BOUTt[:].rearrange("p (k n) -> p k n", k=RCN)
                WDRO2t = pe.tile([128, RCN * 128], f16, tag="wdro2w",
                                 name="wdro2w")
                nc.sync.dma_start(out=WDRO2t[:], in_=wd["WDRO2"][:])
                WDRO2 = WDRO2t[:].rearrange("p (k n) -> p k n", k=RCN)
                NEGPOWt = pe.tile([64, 128], f16, tag="negpoww", name="negpoww")
                nc.sync.dma_start(out=NEGPOWt[:], in_=wd["NEGPOW"][:])
                NPBOHHt = pe.tile([128, 1024], f16, tag="npbohhw",
                                  name="npbohhw")
                nc.sync.dma_start(out=NPBOHHt[:], in_=wd["NPBOHH"][:])
                STK = pe.tile([128, 1024], f16, tag="stk", name="stk")
                SO = pe.tile([128, TCN * 128], dt.float32, tag="so", name="so")
                SOv = SO[:].rearrange("p (t n) -> p t n", t=TCN)
                with (
                    tc.tile_pool(name="psD", bufs=1, space="PSUM") as pps,
                    tc.tile_pool(name="psDt", bufs=2, space="PSUM") as ppt,
                    tc.tile_pool(name="psDv", bufs=1, space="PSUM") as ppv,
                ):
                    st1 = hh_chain_a(pe, pps, ppt, "o", WDR["O"], AOT,
                                     ct_ap=STK[0:64, :], ts=0, te=4)
                    st2 = hh_chain_a(pe, pps, ppt, "o2", WDR["O"], AOT,
                                     ct_ap=STK[0:64, :], ts=4, te=8)
                    hh_chain_b(ppt, st1)
                    CTo = hh_chain_b(ppt, st2)
                    for t in range(TCN):
                        ps = pps.tile([128, 128], dt.float32, tag="ps_sf",
                                      name="ps_sf", bufs=2)
                        for rc in range(RCN):
                            nc.tensor.matmul(
                                ps[:], AOT16[rc][:, 128 * t:128 * (t + 1)],
                                WDRO2[:, rc, :],
                                start=(rc == 0), stop=False)
                        nc.tensor.matmul(
                            ps[:], CTo[0:64, 128 * t:128 * (t + 1)],
                            NEGPOWt[0:64, :], start=False, stop=True)
                        nc.scalar.copy(SOv[:, t, :], ps[:, 0:128])
                    softmax_front(
                        pe, ppt, ppv, "o",
                        SOv[:, :, 0:64], SOv[:, :, 64:128], W["GOUT"][:],
                        cht_dtype=f16, cht_ap=STK[64:128, :])
                    # ao @ base_out accumulated with the Householder/chat
                    # correction in one PSUM group; the BOUT matmuls are ready
                    # early and back-fill PE stalls before STK lands
                    for dc in range(KCN):
                        for th in range(2):
                            ps = pps.tile([128, 512], dt.float32, tag="ps_mm",
                                          name="ps_mm", bufs=3)
                            for rc in range(RCN):
                                nc.tensor.matmul(
                                    ps[:], BOUT[:, rc, 128 * dc:128 * (dc + 1)],
                                    AOT16[rc][:, 512 * th:512 * (th + 1)],
                                    start=(rc == 0), stop=False)
                            nc.tensor.matmul(
                                ps[:], NPBOHHt[:][:, 128 * dc:128 * (dc + 1)],
                                STK[:, 512 * th:512 * (th + 1)],
                                start=False, stop=True)
                            ot = pe.tile([128, 512], dt.float32, tag="outsb",
                                         name="outsb", bufs=3)
                            if th == 0:
                                nc.scalar.copy(ot[:], ps[:])
                            else:
                                nc.vector.tensor_copy(ot[:], ps[:])
                            oeng = nc.sync if th == 0 else nc.gpsimd
                            oeng.dma_start(
                                out=OUTd[128 * dc:128 * (dc + 1),
                                         512 * th:512 * (th + 1)],
                                in_=ot[:])
    _split_sync_waits(nc)
    return nc


def get_built():
    if "nc" not in _BUILT:
        _BUILT["nc"] = build()
    return _BUILT["nc"]


def _in_maps(inputs):
    x = np.asarray(inputs["x"], np.float32)
    w = prep_weights(inputs)
    in_maps = []
    for c in range(NCORES):
        m = dict(w)
        m["XT"] = _r12(_pack_kc(np.ascontiguousarray(x[c].T), KCN, 128))
        in_maps.append(m)
    return in_maps


def kernel(**inputs):
    from concourse.bass_utils import run_bass_kernel_spmd

    nc = get_built()
    res = run_bass_kernel_spmd(nc, _in_maps(inputs),
                               core_ids=list(range(NCORES)))
    out = np.stack([res.results[c]["OUT"].T for c in range(NCORES)], axis=0)
    return out.astype(np.float32)


def run_timed(inputs, trace=False):
    from concourse.bass_utils import run_bass_kernel_spmd
    nc = get_built()
    return run_bass_kernel_spmd(nc, _in_maps(inputs),
                                core_ids=list(range(NCORES)), trace=trace)
